# revision 13
# baseline (speedup 1.0000x reference)
"""S4D AddingModel — Bass/Tile kernel for 8 Trainium2 NeuronCores.

Strategy (data-parallel over batch B=8, one batch element per core):
  encoder matmul -> packed complex z (even/odd samples) -> four-step
  FFT_8192 (stage A over j1 via reverse-matmul, twiddle, stage B over j2)
  -> fused pointwise  Zv[k] = A[k]*Z[k] + B[k]*conj(Z[8192-k])  where the
  host-precomputed A/B fields absorb the rfft unpack, the S4D kernel
  transfer function (incl. the D skip term), and the repack -> mirrored
  inverse four-step -> gelu -> GLU projection -> mean-pool partial sums.

The S4D kernel construction + its rFFT + the A/B fields are tiny
parameter-only computations done on host (numpy).  All O(B*H*L) work runs
on the NeuronCores in one NEFF.

Shapes hardcoded: B=8, L=8192, H=128, N=32.
"""
import numpy as np
import ml_dtypes

B, L, H = 8, 8192, 128
M = 8192          # packed complex FFT length
M1, M2 = 128, 64  # j = j1*64 + j2 ; k = k2*128 + k1
G = 8             # g-chunks per group
NG = 8            # number of groups (NG*G = 64 chunks of 128 cols)

_BF = ml_dtypes.bfloat16


# ---------------------------------------------------------------------------
# host-side constants
# ---------------------------------------------------------------------------

def _host_fields(log_dt, log_A_real, A_imag, C_re, C_im, D):
    """S4D kernel K, its 2L rfft, and the packed-pointwise A/B fields."""
    dt = np.exp(log_dt.astype(np.float64))
    A = -np.exp(log_A_real.astype(np.float64)) + 1j * A_imag.astype(np.float64)
    C = C_re.astype(np.float64) + 1j * C_im.astype(np.float64)
    dtA = dt[:, None] * A
    K_coef = C * (np.exp(dtA) - 1.0) / A
    w = np.exp(dtA)
    Tb = 128
    J = L // Tb
    v_lo = w[:, :, None] ** np.arange(Tb)
    v_hi = (w ** Tb)[:, :, None] ** np.arange(J)
    K = 2.0 * np.matmul(K_coef[:, None, :] * v_hi.transpose(0, 2, 1),
                        v_lo).real.reshape(H, L)

    Khat = np.fft.rfft(K, 2 * L, axis=-1)              # (H, 8193)
    Khat = Khat + D.astype(np.float64)[:, None]        # fold skip y += D*u
    k = np.arange(M)
    P = Khat[:, :M]
    idx = (M - k) % (2 * L)
    Q = np.conj(Khat[:, idx])
    Q[:, 0] = Khat[:, M]
    th = 2.0 * np.pi * k / (2 * L)
    Afld = 0.5 * (P + Q) - 0.5 * (P - Q) * np.sin(th)[None, :]
    Bfld = 0.5j * (P - Q) * np.cos(th)[None, :]
    return Afld, Bfld                                   # (H, 8192) complex


def _pack_field(F):
    """(H, 8192) field -> device plane [128=(h',k2), 8192=(g,k1)]."""
    Fg = F.reshape(H, M2, M1)                           # [h, k2, k1]
    P = Fg.reshape(64, 2, M2, M1).transpose(1, 2, 0, 3)  # [h', k2, g, k1]
    return np.ascontiguousarray(P.reshape(128, 8192))


def _dup(mat):
    """[64, X] -> [128, X] duplicated halves (for base-partition 0/64 use)."""
    return np.concatenate([mat, mat], axis=0)


def host_prep(inputs):
    """Returns (shared_map, per_core_maps, dec_w, dec_b)."""
    f32 = np.float32
    x = np.asarray(inputs["x"], f32)
    Afld, Bfld = _host_fields(inputs["log_dt"], inputs["log_A_real"],
                              inputs["A_imag"], inputs["C_re"],
                              inputs["C_im"], inputs["D"])

    j1 = np.arange(64)
    k1 = np.arange(M1)
    j2 = np.arange(M2)
    k2 = np.arange(M2)

    def bf(a):
        return np.ascontiguousarray(a, dtype=np.float32).astype(_BF)

    shared = {}
    shared["enc_lhsT"] = bf(inputs["enc_w"])                      # [2, 128]
    shared["enc_bias"] = np.asarray(inputs["enc_b"], f32).reshape(128, 1)

    th = 2 * np.pi * np.outer(j1, k1) / M1                        # [64, 128]
    shared["d1m_r"] = bf(np.cos(th))
    shared["d1m_i"] = bf(-np.sin(th))
    shared["d1m_in"] = bf(np.sin(th))

    p = np.arange(128) % 64
    th = 2 * np.pi * np.outer(p, k1) / M                          # [128, 128]
    shared["twc"] = bf(np.cos(th))
    shared["tws"] = bf(-np.sin(th))

    th = 2 * np.pi * np.outer(j2, k2) / M2                        # [64, 64]
    shared["d2m_r"] = bf(_dup(np.cos(th)))
    shared["d2m_i"] = bf(_dup(-np.sin(th)))
    shared["d2m_in"] = bf(_dup(np.sin(th)))
    shared["d2m_rn"] = bf(_dup(-np.cos(th)))

    th = 2 * np.pi * np.outer(j2, 63 - k2) / M2                   # [64, 64]
    shared["f1_r"] = bf(_dup(np.cos(th)))
    shared["f1_i"] = bf(_dup(np.sin(th)))
    shared["f1_rn"] = bf(_dup(-np.cos(th)))

    th = 2 * np.pi * np.outer(k2, j2) / M2                        # [64, 64]
    shared["d2i_r"] = bf(_dup(np.cos(th)))
    shared["d2i_i"] = bf(_dup(np.sin(th)))
    shared["d2i_in"] = bf(_dup(-np.sin(th)))

    th = 2 * np.pi * np.outer(np.arange(M1), j2) / M              # [128, 64]
    shared["twic"] = bf(np.cos(th))
    shared["twis"] = bf(np.sin(th))

    th = 2 * np.pi * np.outer(np.arange(M1), j1) / M1             # [128, 64]
    shared["d1i_r"] = bf(np.cos(th) / M)
    shared["d1i_i"] = bf(np.sin(th) / M)
    shared["d1i_in"] = bf(-np.sin(th) / M)

    shared["glu_lhsT"] = bf(np.asarray(inputs["out_w"], f32).T)   # [128, 256]
    ob = np.asarray(inputs["out_b"], f32)
    shared["glu_ba"] = ob[:128].reshape(128, 1).astype(f32)
    shared["glu_bg"] = ob[128:].reshape(128, 1).astype(f32)
    shared["ones_c"] = np.ones((128, 1), f32)
    shared["half_c"] = np.full((128, 1), 0.5, f32)

    shared["apl_r"] = bf(_pack_field(Afld.real))
    shared["apl_i"] = bf(_pack_field(Afld.imag))
    shared["bpl_r"] = bf(_pack_field(Bfld.real))
    shared["bpl_i"] = bf(_pack_field(Bfld.imag))

    per_core = []
    for b in range(B):
        xb = x[b]                                                 # (8192, 2)
        per_core.append({
            "xe": bf(xb[0::2, :].T),                              # [2, 4096]
            "xo": bf(xb[1::2, :].T),                              # [2, 4096]
        })
    return shared, per_core


# ---------------------------------------------------------------------------
# device program
# ---------------------------------------------------------------------------

_SHARED_SPECS = [
    ("enc_lhsT", (2, 128), "bf"), ("enc_bias", (128, 1), "f32"),
    ("d1m_r", (64, 128), "bf"), ("d1m_i", (64, 128), "bf"),
    ("d1m_in", (64, 128), "bf"),
    ("twc", (128, 128), "bf"), ("tws", (128, 128), "bf"),
    ("d2m_r", (128, 64), "bf"), ("d2m_i", (128, 64), "bf"),
    ("d2m_in", (128, 64), "bf"), ("d2m_rn", (128, 64), "bf"),
    ("f1_r", (128, 64), "bf"), ("f1_i", (128, 64), "bf"),
    ("f1_rn", (128, 64), "bf"),
    ("d2i_r", (128, 64), "bf"), ("d2i_i", (128, 64), "bf"),
    ("d2i_in", (128, 64), "bf"),
    ("twic", (128, 64), "bf"), ("twis", (128, 64), "bf"),
    ("d1i_r", (128, 64), "bf"), ("d1i_i", (128, 64), "bf"),
    ("d1i_in", (128, 64), "bf"),
    ("glu_lhsT", (128, 256), "bf"), ("glu_ba", (128, 1), "f32"),
    ("glu_bg", (128, 1), "f32"), ("ones_c", (128, 1), "f32"),
    ("half_c", (128, 1), "f32"),
    ("apl_r", (128, 8192), "bf"), ("apl_i", (128, 8192), "bf"),
    ("bpl_r", (128, 8192), "bf"), ("bpl_i", (128, 8192), "bf"),
]


def build_program(debug_taps=False):
    """Build + compile the single-core SPMD bass program."""
    import concourse.bass as bass
    import concourse.tile as tile
    from concourse import bacc, mybir

    bf = mybir.dt.bfloat16
    f32 = mybir.dt.float32
    AF = mybir.ActivationFunctionType
    ALU = mybir.AluOpType

    nc = bacc.Bacc("TRN2", target_bir_lowering=False, debug=False,
                   num_devices=B)

    dram = {}
    for name, shape, dt_ in _SHARED_SPECS:
        dram[name] = nc.dram_tensor(name, list(shape),
                                    bf if dt_ == "bf" else f32,
                                    kind="ExternalInput").ap()
    dram["xe"] = nc.dram_tensor("xe", [2, 4096], bf, kind="ExternalInput").ap()
    dram["xo"] = nc.dram_tensor("xo", [2, 4096], bf, kind="ExternalInput").ap()
    pool_out = nc.dram_tensor("pool", [128, 1], f32, kind="ExternalOutput").ap()
    taps = {}
    if debug_taps:
        for nm in ("t_ct_r", "t_ct_i", "t_z_r", "t_z_i", "t_zc_r", "t_zc_i",
                   "t_zv_r", "t_zv_i", "t_c3_r", "t_c3_i"):
            taps[nm] = nc.dram_tensor(nm, [128, 8192], bf,
                                      kind="ExternalOutput").ap()
        for nm in ("t_ye", "t_yo"):
            taps[nm] = nc.dram_tensor(nm, [128, 4096], bf,
                                      kind="ExternalOutput").ap()

    with tile.TileContext(nc) as tc:
        from contextlib import ExitStack
        _stack = ExitStack()
        cpool = _stack.enter_context(tc.tile_pool(name="consts", bufs=1))
        C = {}
        for name, shape, dt_ in _SHARED_SPECS:
            C[name] = cpool.tile(list(shape), bf if dt_ == "bf" else f32,
                                 name="c_" + name, tag=name)
            nc.sync.dma_start(C[name][:], dram[name][:])

        persist = _stack.enter_context(tc.tile_pool(name="persist", bufs=1))
        t1r = persist.tile([64, 8192], bf, tag="t1r")
        t1i = persist.tile([64, 8192], bf, tag="t1i")
        yE = persist.tile([128, 4096], bf, tag="yE")
        yO = persist.tile([128, 4096], bf, tag="yO")
        pool_cols = persist.tile([128, 16], f32, tag="pool_cols")



        work = _stack.enter_context(tc.tile_pool(name="work", bufs=2))
        pw = _stack.enter_context(tc.tile_pool(name="pw", bufs=2))
        pp = _stack.enter_context(tc.tile_pool(name="pp", bufs=1, space="PSUM"))

        # ---- encoder + transpose into T1[j1, (h,j2)] -------------------
        for dst, src in ((t1r, dram["xe"]), (t1i, dram["xo"])):
            for c in range(8):
                xch = work.tile([2, 512], bf, tag="xch")
                nc.sync.dma_start(xch[:], src[:, c * 512:(c + 1) * 512])
                ps = pp.tile([128, 512], f32, name="ps", tag="pbig", bufs=3)
                nc.tensor.matmul(ps[:], C["enc_lhsT"][:], xch[:],
                                 start=True, stop=True)
                zch = work.tile([128, 512], bf, tag="zch")
                nc.scalar.activation(zch[:], ps[:], AF.Identity,
                                     bias=C["enc_bias"][:, 0:1], scale=1.0)
                for j1l in range(8):
                    nc.sync.dma_start(
                        dst[8 * c + j1l:8 * c + j1l + 1, :]
                           .rearrange("a (h b) -> a h b", h=128),
                        zch[:, j1l * 64:(j1l + 1) * 64])

        # ---- main groups ----------------------------------------------
        for gg in range(NG):
            g0 = gg * G
            # A-rev: Ct[(h',j2), (g,k1)] chunks
            ctr = work.tile([128, 1024], bf, tag="ctr")
            cti = work.tile([128, 1024], bf, tag="cti")
            for gl in range(G):
                g = g0 + gl
                ps_re = pp.tile([128, 128], f32, name="psA_re", tag="pa_re", bufs=1)
                ps_im = pp.tile([128, 128], f32, name="psA_im", tag="pa_im", bufs=1)
                lr = t1r[:, g * 128:(g + 1) * 128]
                li = t1i[:, g * 128:(g + 1) * 128]
                nc.tensor.matmul(ps_re[:], lr, C["d1m_r"][:], start=True, stop=False)
                nc.tensor.matmul(ps_im[:], lr, C["d1m_i"][:], start=True, stop=False)
                nc.tensor.matmul(ps_re[:], li, C["d1m_in"][:], start=False, stop=True)
                nc.tensor.matmul(ps_im[:], li, C["d1m_r"][:], start=False, stop=True)
                nc.scalar.copy(ctr[:, gl * 128:(gl + 1) * 128], ps_re[:])
                nc.scalar.copy(cti[:, gl * 128:(gl + 1) * 128], ps_im[:])

            # forward twiddle (broadcast [128,128] tile over g)
            twc_b = C["twc"][:].unsqueeze(1).broadcast_to((128, G, 128))
            tws_b = C["tws"][:].unsqueeze(1).broadcast_to((128, G, 128))
            cttr = work.tile([128, 1024], bf, tag="cttr", bufs=1)
            ctti = work.tile([128, 1024], bf, tag="ctti", bufs=1)
            v3 = lambda t: t[:].rearrange("p (a b) -> p a b", a=G)
            tA = work.tile([128, 1024], bf, name="tA", tag="s1")
            tB = work.tile([128, 1024], bf, name="tB", tag="s2")
            nc.vector.tensor_mul(v3(tA), v3(ctr), twc_b)
            nc.gpsimd.tensor_mul(v3(tB), v3(cti), tws_b)
            nc.vector.tensor_sub(cttr[:], tA[:], tB[:])
            nc.gpsimd.tensor_mul(v3(tA), v3(ctr), tws_b)
            nc.vector.tensor_mul(v3(tB), v3(cti), twc_b)
            nc.gpsimd.tensor_add(ctti[:], tA[:], tB[:])
            if debug_taps:
                nc.sync.dma_start(taps["t_ct_r"][:, g0*128:(g0+G)*128], cttr[:])
                nc.sync.dma_start(taps["t_ct_i"][:, g0*128:(g0+G)*128], ctti[:])

            # stage B -> Z ; Zc via F1/k0 path
            zr = work.tile([128, 1024], bf, tag="zr", bufs=1)
            zi = work.tile([128, 1024], bf, tag="zi", bufs=1)
            for (dst, m1a, m1b, m2a, m2b) in (
                    (zr, "d2m_r", "d2m_in", None, None),
                    (zi, "d2m_i", "d2m_r", None, None)):
                for c in range(2):
                    ps = pp.tile([128, 512], f32, name="psb", tag="pbig", bufs=3)
                    cols = slice(c * 512, (c + 1) * 512)
                    for h_ in range(2):
                        rows = slice(h_ * 64, (h_ + 1) * 64)
                        nc.tensor.matmul(ps[rows, :], C[m1a][rows, :],
                                         cttr[rows, cols], start=True, stop=False)
                        nc.tensor.matmul(ps[rows, :], C[m1b][rows, :],
                                         ctti[rows, cols], start=False, stop=True)
                    nc.scalar.copy(dst[:, cols], ps[:])

            zcr = work.tile([128, 1024], bf, tag="zcr", bufs=1)
            zci = work.tile([128, 1024], bf, tag="zci", bufs=1)
            # main part k1 in [1,128): rhs cols reversed within each g block
            for (dst, ma, mb) in ((zcr, "f1_r", "f1_i"), (zci, "f1_i", "f1_rn")):
                for c in range(2):   # 4 g per chunk
                    psf = pp.tile([128, 512], f32, name="psc", tag="pbig", bufs=3)
                    ps = psf[:, 0:508]
                    for h_ in range(2):
                        rows = slice(h_ * 64, (h_ + 1) * 64)
                        rev_r = cttr[rows, :].rearrange(
                            "p (a b) -> p a b", a=G)[:, c * 4:(c + 1) * 4, 127:0:-1]
                        rev_i = ctti[rows, :].rearrange(
                            "p (a b) -> p a b", a=G)[:, c * 4:(c + 1) * 4, 127:0:-1]
                        nc.tensor.matmul(ps[rows, :].rearrange(
                            "p (a b) -> p a b", a=4), C[ma][rows, :], rev_r,
                            start=True, stop=False)
                        nc.tensor.matmul(ps[rows, :].rearrange(
                            "p (a b) -> p a b", a=4), C[mb][rows, :], rev_i,
                            start=False, stop=True)
                    # scatter 127-col blocks into dst cols g*128+1 ..
                    nc.scalar.copy(
                        dst[:].rearrange("p (a b) -> p a b", a=G)
                           [:, c * 4:(c + 1) * 4, 1:128],
                        ps[:].rearrange("p (a b) -> p a b", a=4))
            # k1 = 0 columns
            for (dst, ma, mb) in ((zcr, "d2m_r", "d2m_i"), (zci, "d2m_i", "d2m_rn")):
                ps = pp.tile([128, 8], f32, name="psk", tag="pk0", bufs=1)
                r0 = cttr[:].rearrange("p (a b) -> p a b", a=G)[:, :, 0:1]
                i0 = ctti[:].rearrange("p (a b) -> p a b", a=G)[:, :, 0:1]
                for h_ in range(2):
                    rows = slice(h_ * 64, (h_ + 1) * 64)
                    nc.tensor.matmul(ps[rows, :].rearrange("p (a b) -> p a b", a=G),
                                     C[ma][rows, :], r0[h_ * 64:(h_ + 1) * 64],
                                     start=True, stop=False)
                    nc.tensor.matmul(ps[rows, :].rearrange("p (a b) -> p a b", a=G),
                                     C[mb][rows, :], i0[h_ * 64:(h_ + 1) * 64],
                                     start=False, stop=True)
                nc.scalar.copy(
                    dst[:].rearrange("p (a b) -> p a b", a=G)[:, :, 0:1],
                    ps[:].rearrange("p (a b) -> p a b", a=G))

            if debug_taps:
                for tp, t in (("t_z_r", zr), ("t_z_i", zi),
                              ("t_zc_r", zcr), ("t_zc_i", zci)):
                    nc.sync.dma_start(taps[tp][:, g0*128:(g0+G)*128], t[:])

            # pointwise: Zv = A*Z + B*Zc
            ar = pw.tile([128, 1024], bf, tag="ar")
            ai = pw.tile([128, 1024], bf, tag="ai")
            br = pw.tile([128, 1024], bf, tag="br")
            bi = pw.tile([128, 1024], bf, tag="bi")
            cols = slice(g0 * 128, (g0 + G) * 128)
            nc.sync.dma_start(ar[:], dram["apl_r"][:, cols])
            nc.sync.dma_start(ai[:], dram["apl_i"][:, cols])
            nc.sync.dma_start(br[:], dram["bpl_r"][:, cols])
            nc.sync.dma_start(bi[:], dram["bpl_i"][:, cols])
            zvr = work.tile([128, 1024], bf, tag="zvr", bufs=1)
            zvi = work.tile([128, 1024], bf, tag="zvi", bufs=1)
            p1 = work.tile([128, 1024], bf, name="p1", tag="s1")
            p2 = work.tile([128, 1024], bf, name="p2", tag="s2")
            p3 = work.tile([128, 1024], bf, name="p3", tag="s3")
            p4 = work.tile([128, 1024], bf, name="p4", tag="s4")
            nc.vector.tensor_mul(p1[:], zr[:], ar[:])
            nc.gpsimd.tensor_mul(p2[:], zi[:], ai[:])
            nc.gpsimd.tensor_mul(p3[:], zcr[:], br[:])
            nc.vector.tensor_mul(p4[:], zci[:], bi[:])
            nc.vector.tensor_sub(p1[:], p1[:], p2[:])
            nc.gpsimd.tensor_sub(p3[:], p3[:], p4[:])
            nc.vector.tensor_add(zvr[:], p1[:], p3[:])
            nc.gpsimd.tensor_mul(p1[:], zi[:], ar[:])
            nc.vector.tensor_mul(p2[:], zr[:], ai[:])
            nc.vector.tensor_mul(p3[:], zci[:], br[:])
            nc.gpsimd.tensor_mul(p4[:], zcr[:], bi[:])
            nc.gpsimd.tensor_add(p1[:], p1[:], p2[:])
            nc.vector.tensor_add(p3[:], p3[:], p4[:])
            nc.gpsimd.tensor_add(zvi[:], p1[:], p3[:])
            if debug_taps:
                nc.sync.dma_start(taps["t_zv_r"][:, cols], zvr[:])
                nc.sync.dma_start(taps["t_zv_i"][:, cols], zvi[:])

            # B'-rev: C3[k1, (h,j2)] per (g, h')
            c3r = work.tile([128, 1024], bf, tag="c3r", bufs=1)
            c3i = work.tile([128, 1024], bf, tag="c3i", bufs=1)
            for gl in range(G):
                for h_ in range(2):
                    rows = slice(h_ * 64, (h_ + 1) * 64)
                    lr = zvr[rows, gl * 128:(gl + 1) * 128]
                    li = zvi[rows, gl * 128:(gl + 1) * 128]
                    ps_re = pp.tile([128, 64], f32, name="psD_re", tag="pd_re", bufs=1)
                    ps_im = pp.tile([128, 64], f32, name="psD_im", tag="pd_im", bufs=1)
                    nc.tensor.matmul(ps_re[:], lr, C["d2i_r"][rows, :], start=True, stop=False)
                    nc.tensor.matmul(ps_im[:], lr, C["d2i_i"][rows, :], start=True, stop=False)
                    nc.tensor.matmul(ps_re[:], li, C["d2i_in"][rows, :], start=False, stop=True)
                    nc.tensor.matmul(ps_im[:], li, C["d2i_r"][rows, :], start=False, stop=True)
                    oc = (2 * gl + h_) * 64
                    nc.scalar.copy(c3r[:, oc:oc + 64], ps_re[:])
                    nc.scalar.copy(c3i[:, oc:oc + 64], ps_im[:])
            if debug_taps:
                nc.sync.dma_start(taps["t_c3_r"][:, cols], c3r[:])
                nc.sync.dma_start(taps["t_c3_i"][:, cols], c3i[:])

            # inverse twiddle (broadcast [128,64] over h=16)
            twic_b = C["twic"][:].unsqueeze(1).broadcast_to((128, 16, 64))
            twis_b = C["twis"][:].unsqueeze(1).broadcast_to((128, 16, 64))
            v3h = lambda t: t[:].rearrange("p (a b) -> p a b", a=16)
            c3tr = work.tile([128, 1024], bf, tag="c3tr", bufs=1)
            c3ti = work.tile([128, 1024], bf, tag="c3ti", bufs=1)
            tC = work.tile([128, 1024], bf, name="tC", tag="s1")
            tD = work.tile([128, 1024], bf, name="tD", tag="s2")
            nc.vector.tensor_mul(v3h(tC), v3h(c3r), twic_b)
            nc.gpsimd.tensor_mul(v3h(tD), v3h(c3i), twis_b)
            nc.vector.tensor_sub(c3tr[:], tC[:], tD[:])
            nc.gpsimd.tensor_mul(v3h(tC), v3h(c3r), twis_b)
            nc.vector.tensor_mul(v3h(tD), v3h(c3i), twic_b)
            nc.gpsimd.tensor_add(c3ti[:], tC[:], tD[:])

            # stage A' -> vE, vO [j1<64, (h, j2)]
            ve = work.tile([64, 1024], bf, tag="ve")
            vo = work.tile([64, 1024], bf, tag="vo")
            for (dst, ma, mb) in ((ve, "d1i_r", "d1i_in"), (vo, "d1i_i", "d1i_r")):
                for c in range(2):
                    cols2 = slice(c * 512, (c + 1) * 512)
                    ps = pp.tile([64, 512], f32, name="pse", tag="pbig", bufs=3)
                    nc.tensor.matmul(ps[:], C[ma][:], c3tr[:, cols2], start=True, stop=False)
                    nc.tensor.matmul(ps[:], C[mb][:], c3ti[:, cols2], start=False, stop=True)
                    nc.scalar.copy(dst[:, cols2], ps[:])

            # scatter into yE/yO rows [16 h-rows of this group]
            for dst, src in ((yE, ve), (yO, vo)):
                for hl in range(16):
                    r = gg * 16 + hl
                    nc.sync.dma_start(
                        dst[r:r + 1, :].rearrange("a (j b) -> a j b", j=64),
                        src[:, hl * 64:(hl + 1) * 64])

        if debug_taps:
            nc.sync.dma_start(taps["t_ye"][:], yE[:])
            nc.sync.dma_start(taps["t_yo"][:], yO[:])

        # ---- gelu + GLU + pool ----------------------------------------
        CG = 0.7978845608028654
        for pl in (yE, yO):
            for c in range(4):
                cols = slice(c * 1024, (c + 1) * 1024)
                xc = pl[:, cols]
                sq = work.tile([128, 1024], bf, name="sq", tag="s1")
                nc.scalar.activation(sq[:], xc, AF.Square)
                rr = work.tile([128, 1024], bf, name="rr", tag="s2")
                nc.scalar.activation(rr[:], sq[:], AF.Identity,
                                     bias=C["ones_c"][:, 0:1], scale=0.044715)
                qq = work.tile([128, 1024], bf, name="qq", tag="s3")
                nc.vector.tensor_mul(qq[:], xc, rr[:])
                tt = work.tile([128, 1024], bf, name="tt", tag="s4")
                nc.scalar.activation(tt[:], qq[:], AF.Tanh, scale=CG)
                uu = work.tile([128, 1024], bf, name="uu", tag="s1")
                nc.scalar.activation(uu[:], tt[:], AF.Identity,
                                     bias=C["half_c"][:, 0:1], scale=0.5)
                nc.vector.tensor_mul(xc, xc, uu[:])

        scratch = work.tile([128, 512], bf, tag="glu_scratch")
        idx = 0
        for plane in (yE, yO):
            for c in range(8):
                cols = slice(c * 512, (c + 1) * 512)
                ps_a = pp.tile([128, 512], f32, tag="pbig", bufs=3)
                ps_g = pp.tile([128, 512], f32, tag="pbig", bufs=3)
                nc.tensor.matmul(ps_a[:], C["glu_lhsT"][:, 0:128], plane[:, cols],
                                 start=True, stop=True)
                nc.tensor.matmul(ps_g[:], C["glu_lhsT"][:, 128:256], plane[:, cols],
                                 start=True, stop=True)
                sig = work.tile([128, 512], bf, tag="glu_sig")
                nc.scalar.activation(sig[:], ps_g[:], AF.Sigmoid,
                                     bias=C["glu_bg"][:, 0:1], scale=1.0)
                nc.vector.scalar_tensor_tensor(
                    scratch[:], ps_a[:], C["glu_ba"][:, 0:1], sig[:],
                    op0=ALU.add, op1=ALU.mult,
                    accum_out=pool_cols[:, idx:idx + 1])
                idx += 1

        pool_t = work.tile([128, 1], f32, tag="pool_t")
        nc.vector.tensor_reduce(pool_t[:], pool_cols[:],
                                axis=mybir.AxisListType.X, op=ALU.add)
        nc.sync.dma_start(pool_out[:], pool_t[:])

        _stack.close()

    nc.compile()
    return nc


_CACHED_NC = None


def kernel(**inputs):
    global _CACHED_NC
    from concourse.bass_utils import run_bass_kernel_spmd

    shared, per_core = host_prep(inputs)
    if _CACHED_NC is None:
        _CACHED_NC = build_program()
    nc = _CACHED_NC

    in_maps = [{**shared, **pc} for pc in per_core]
    res = run_bass_kernel_spmd(nc, in_maps, list(range(B)))
    pool = np.stack([np.asarray(res.results[b]["pool"][:, 0], np.float64)
                     for b in range(B)])                     # (8, 128)
    pooled = pool / float(L)
    dec_w = np.asarray(inputs["dec_w"], np.float64)
    dec_b = np.asarray(inputs["dec_b"], np.float64)
    return (pooled @ dec_w + dec_b).astype(np.float32)


if __name__ == "__main__":
    ins = {
        "x": np.random.randn(B, L, 2).astype(np.float32),
        "enc_w": np.random.randn(2, H).astype(np.float32),
        "enc_b": np.random.randn(H).astype(np.float32),
        "log_dt": np.random.rand(H).astype(np.float32),
        "log_A_real": np.random.randn(H, 32).astype(np.float32),
        "A_imag": np.random.randn(H, 32).astype(np.float32),
        "C_re": np.random.randn(H, 32).astype(np.float32),
        "C_im": np.random.randn(H, 32).astype(np.float32),
        "D": np.random.randn(H).astype(np.float32),
        "out_w": np.random.randn(2 * H, H).astype(np.float32),
        "out_b": np.random.randn(2 * H).astype(np.float32),
        "dec_w": np.random.randn(H, 1).astype(np.float32),
        "dec_b": np.random.randn(1).astype(np.float32),
    }
    print(kernel(**ins).shape)


# revision 19
# speedup vs baseline: 1.4624x; 1.4624x over previous
"""S4D AddingModel — Bass/Tile kernel for 8 Trainium2 NeuronCores.

Strategy (data-parallel over batch B=8, one batch element per core):
  encoder matmul -> packed complex z (even/odd samples) -> four-step
  FFT_8192 (stage A over j1 via reverse-matmul, twiddle, stage B over j2)
  -> fused pointwise  Zv[k] = A[k]*Z[k] + B[k]*conj(Z[8192-k])  where the
  host-precomputed A/B fields absorb the rfft unpack, the S4D kernel
  transfer function (incl. the D skip term), and the repack -> mirrored
  inverse four-step -> gelu -> GLU projection -> mean-pool partial sums.

The S4D kernel construction + its rFFT + the A/B fields are tiny
parameter-only computations done on host (numpy).  All O(B*H*L) work runs
on the NeuronCores in one NEFF.

Shapes hardcoded: B=8, L=8192, H=128, N=32.
"""
import numpy as np
import ml_dtypes

B, L, H = 8, 8192, 128
M = 8192          # packed complex FFT length
M1, M2 = 128, 64  # j = j1*64 + j2 ; k = k2*128 + k1
G = 8             # g-chunks per group
NG = 8            # number of groups (NG*G = 64 chunks of 128 cols)

_BF = ml_dtypes.bfloat16


# ---------------------------------------------------------------------------
# host-side constants
# ---------------------------------------------------------------------------

def _host_fields(log_dt, log_A_real, A_imag, C_re, C_im, D):
    """S4D kernel K, its 2L rfft, and the packed-pointwise A/B fields."""
    dt = np.exp(log_dt.astype(np.float64))
    A = -np.exp(log_A_real.astype(np.float64)) + 1j * A_imag.astype(np.float64)
    C = C_re.astype(np.float64) + 1j * C_im.astype(np.float64)
    dtA = dt[:, None] * A
    K_coef = C * (np.exp(dtA) - 1.0) / A
    w = np.exp(dtA)
    Tb = 128
    J = L // Tb
    v_lo = w[:, :, None] ** np.arange(Tb)
    v_hi = (w ** Tb)[:, :, None] ** np.arange(J)
    K = 2.0 * np.matmul(K_coef[:, None, :] * v_hi.transpose(0, 2, 1),
                        v_lo).real.reshape(H, L)

    Khat = np.fft.rfft(K, 2 * L, axis=-1)              # (H, 8193)
    Khat = Khat + D.astype(np.float64)[:, None]        # fold skip y += D*u
    k = np.arange(M)
    P = Khat[:, :M]
    idx = (M - k) % (2 * L)
    Q = np.conj(Khat[:, idx])
    Q[:, 0] = Khat[:, M]
    th = 2.0 * np.pi * k / (2 * L)
    Afld = 0.5 * (P + Q) - 0.5 * (P - Q) * np.sin(th)[None, :]
    Bfld = 0.5j * (P - Q) * np.cos(th)[None, :]
    return Afld, Bfld                                   # (H, 8192) complex


def _pack_field(F):
    """(H, 8192) field -> device plane [128=(h',k2), 8192=(g,k1)]."""
    Fg = F.reshape(H, M2, M1)                           # [h, k2, k1]
    P = Fg.reshape(64, 2, M2, M1).transpose(1, 2, 0, 3)  # [h', k2, g, k1]
    return np.ascontiguousarray(P.reshape(128, 8192))


def _dup(mat):
    """[64, X] -> [128, X] duplicated halves (for base-partition 0/64 use)."""
    return np.concatenate([mat, mat], axis=0)


def host_prep(inputs):
    """Returns (shared_map, per_core_maps, dec_w, dec_b)."""
    f32 = np.float32
    x = np.asarray(inputs["x"], f32)
    Afld, Bfld = _host_fields(inputs["log_dt"], inputs["log_A_real"],
                              inputs["A_imag"], inputs["C_re"],
                              inputs["C_im"], inputs["D"])

    j1 = np.arange(64)
    k1 = np.arange(M1)
    j2 = np.arange(M2)
    k2 = np.arange(M2)

    def bf(a):
        return np.ascontiguousarray(a, dtype=np.float32).astype(_BF)

    shared = {}
    shared["enc_lhsT"] = bf(inputs["enc_w"])                      # [2, 128]
    shared["enc_bias"] = np.asarray(inputs["enc_b"], f32).reshape(128, 1)

    th = 2 * np.pi * np.outer(j1, k1) / M1                        # [64, 128]
    shared["d1m_r"] = bf(np.cos(th))
    shared["d1m_i"] = bf(-np.sin(th))
    shared["d1m_in"] = bf(np.sin(th))

    p = np.arange(128) % 64
    th = 2 * np.pi * np.outer(p, k1) / M                          # [128, 128]
    shared["twc"] = bf(np.cos(th))
    shared["tws"] = bf(-np.sin(th))

    th = 2 * np.pi * np.outer(j2, k2) / M2                        # [64, 64]
    shared["d2m_r"] = bf(_dup(np.cos(th)))
    shared["d2m_i"] = bf(_dup(-np.sin(th)))
    shared["d2m_in"] = bf(_dup(np.sin(th)))
    shared["d2m_rn"] = bf(_dup(-np.cos(th)))

    th = 2 * np.pi * np.outer(j2, 63 - k2) / M2                   # [64, 64]
    shared["f1_r"] = bf(_dup(np.cos(th)))
    shared["f1_i"] = bf(_dup(np.sin(th)))
    shared["f1_rn"] = bf(_dup(-np.cos(th)))

    th = 2 * np.pi * np.outer(k2, j2) / M2                        # [64, 64]
    shared["d2i_r"] = bf(_dup(np.cos(th)))
    shared["d2i_i"] = bf(_dup(np.sin(th)))
    shared["d2i_in"] = bf(_dup(-np.sin(th)))

    th = 2 * np.pi * np.outer(np.arange(M1), j2) / M              # [128, 64]
    shared["twic"] = bf(np.cos(th))
    shared["twis"] = bf(np.sin(th))

    th = 2 * np.pi * np.outer(np.arange(M1), j1) / M1             # [128, 64]
    shared["d1i_r"] = bf(np.cos(th) / M)
    shared["d1i_i"] = bf(np.sin(th) / M)
    shared["d1i_in"] = bf(-np.sin(th) / M)

    shared["glu_lhsT"] = bf(np.asarray(inputs["out_w"], f32).T)   # [128, 256]
    ob = np.asarray(inputs["out_b"], f32)
    shared["glu_ba"] = ob[:128].reshape(128, 1).astype(f32)
    shared["glu_bg"] = ob[128:].reshape(128, 1).astype(f32)
    shared["ones_c"] = np.ones((128, 1), f32)
    shared["half_c"] = np.full((128, 1), 0.5, f32)

    shared["fields"] = np.concatenate(
        [bf(_pack_field(p)) for p in (Afld.real, Afld.imag,
                                      Bfld.real, Bfld.imag)], axis=1)

    per_core = []
    for b in range(B):
        xb = x[b]                                                 # (8192, 2)
        per_core.append({
            "xe": bf(xb[0::2, :].T),                              # [2, 4096]
            "xo": bf(xb[1::2, :].T),                              # [2, 4096]
        })
    return shared, per_core


# ---------------------------------------------------------------------------
# device program
# ---------------------------------------------------------------------------

_SHARED_SPECS = [
    ("enc_lhsT", (2, 128), "bf"), ("enc_bias", (128, 1), "f32"),
    ("d1m_r", (64, 128), "bf"), ("d1m_i", (64, 128), "bf"),
    ("d1m_in", (64, 128), "bf"),
    ("twc", (128, 128), "bf"), ("tws", (128, 128), "bf"),
    ("d2m_r", (128, 64), "bf"), ("d2m_i", (128, 64), "bf"),
    ("d2m_in", (128, 64), "bf"), ("d2m_rn", (128, 64), "bf"),
    ("f1_r", (128, 64), "bf"), ("f1_i", (128, 64), "bf"),
    ("f1_rn", (128, 64), "bf"),
    ("d2i_r", (128, 64), "bf"), ("d2i_i", (128, 64), "bf"),
    ("d2i_in", (128, 64), "bf"),
    ("twic", (128, 64), "bf"), ("twis", (128, 64), "bf"),
    ("d1i_r", (128, 64), "bf"), ("d1i_i", (128, 64), "bf"),
    ("d1i_in", (128, 64), "bf"),
    ("glu_lhsT", (128, 256), "bf"), ("glu_ba", (128, 1), "f32"),
    ("glu_bg", (128, 1), "f32"), ("ones_c", (128, 1), "f32"),
    ("half_c", (128, 1), "f32"),
    ("fields", (128, 4 * 8192), "bf"),
]


def build_program(debug_taps=False):
    """Build + compile the single-core SPMD bass program."""
    import concourse.bass as bass
    import concourse.tile as tile
    from concourse import bacc, mybir

    bf = mybir.dt.bfloat16
    f32 = mybir.dt.float32
    AF = mybir.ActivationFunctionType
    ALU = mybir.AluOpType

    nc = bacc.Bacc("TRN2", target_bir_lowering=False, debug=False,
                   num_devices=B)

    dram = {}
    for name, shape, dt_ in _SHARED_SPECS:
        dram[name] = nc.dram_tensor(name, list(shape),
                                    bf if dt_ == "bf" else f32,
                                    kind="ExternalInput").ap()
    dram["xe"] = nc.dram_tensor("xe", [2, 4096], bf, kind="ExternalInput").ap()
    dram["xo"] = nc.dram_tensor("xo", [2, 4096], bf, kind="ExternalInput").ap()
    pool_out = nc.dram_tensor("pool", [128, 1], f32, kind="ExternalOutput").ap()
    taps = {}
    if debug_taps:
        for nm in ("t_ct_r", "t_ct_i", "t_z_r", "t_z_i", "t_zc_r", "t_zc_i",
                   "t_zv_r", "t_zv_i", "t_c3_r", "t_c3_i"):
            taps[nm] = nc.dram_tensor(nm, [128, 8192], bf,
                                      kind="ExternalOutput").ap()
        for nm in ("t_ye", "t_yo"):
            taps[nm] = nc.dram_tensor(nm, [128, 4096], bf,
                                      kind="ExternalOutput").ap()

    with tile.TileContext(nc) as tc:
        from contextlib import ExitStack
        _stack = ExitStack()
        cpool = _stack.enter_context(tc.tile_pool(name="consts", bufs=1))
        C = {}
        for name, shape, dt_ in _SHARED_SPECS:
            if name == "fields":
                continue
            C[name] = cpool.tile(list(shape), bf if dt_ == "bf" else f32,
                                 name="c_" + name, tag=name)
            nc.sync.dma_start(C[name][:], dram[name][:])

        persist = _stack.enter_context(tc.tile_pool(name="persist", bufs=1))
        t1r = persist.tile([64, 8192], bf, tag="t1r")
        t1i = persist.tile([64, 8192], bf, tag="t1i")
        yE = persist.tile([128, 4096], bf, tag="yE")
        yO = persist.tile([128, 4096], bf, tag="yO")
        pool_cols = persist.tile([128, 16], f32, tag="pool_cols")



        work = _stack.enter_context(tc.tile_pool(name="work", bufs=2))
        pw = _stack.enter_context(tc.tile_pool(name="pw", bufs=2))
        pp = _stack.enter_context(tc.tile_pool(name="pp", bufs=1, space="PSUM"))

        # ---- encoder -> DRAM bounce -> T1[j1, (h,j2)] ------------------
        from concourse.tile import add_dep_helper
        dz = {0: nc.dram_tensor("dz_r", [128, 4096], bf, kind="Internal").ap(),
              1: nc.dram_tensor("dz_i", [128, 4096], bf, kind="Internal").ap()}
        for pi, (dst, src) in enumerate(((t1r, dram["xe"]), (t1i, dram["xo"]))):
            scat = []
            for c in range(8):
                xch = work.tile([2, 512], bf, tag="xch")
                nc.sync.dma_start(xch[:], src[:, c * 512:(c + 1) * 512])
                ps = pp.tile([128, 512], f32, name="ps", tag="pbig", bufs=4)
                nc.tensor.matmul(ps[:], C["enc_lhsT"][:], xch[:],
                                 start=True, stop=True)
                zch = work.tile([128, 512], bf, tag="zch")
                nc.scalar.activation(zch[:], ps[:], AF.Identity,
                                     bias=C["enc_bias"][:, 0:1], scale=1.0)
                scat.append(nc.sync.dma_start(
                    dz[pi][:, c * 512:(c + 1) * 512], zch[:]))
            gat = nc.sync.dma_start(
                dst[:].rearrange("a (h b) -> a h b", h=128),
                dz[pi][:].rearrange("h (a b) -> h a b", a=64).transpose([1, 0, 2]))
            for s in scat:
                add_dep_helper(gat.ins, s.ins, reason="t1 gather after scatter")

        # ---- main groups ----------------------------------------------
        dv_e = nc.dram_tensor("dv_e", [NG, 64, 1024], bf, kind="Internal").ap()
        dv_o = nc.dram_tensor("dv_o", [NG, 64, 1024], bf, kind="Internal").ap()
        fin_scat = []
        for gg in range(NG):
            g0 = gg * G
            # A-rev: Ct[(h',j2), (g,k1)] chunks
            ctr = work.tile([128, 1024], bf, tag="ctr")
            cti = work.tile([128, 1024], bf, tag="cti")
            for gl in range(G):
                g = g0 + gl
                ps_re = pp.tile([128, 128], f32, name="psA_re", tag="pa_re", bufs=1)
                ps_im = pp.tile([128, 128], f32, name="psA_im", tag="pa_im", bufs=1)
                lr = t1r[:, g * 128:(g + 1) * 128]
                li = t1i[:, g * 128:(g + 1) * 128]
                nc.tensor.matmul(ps_re[:], lr, C["d1m_r"][:], start=True, stop=False)
                nc.tensor.matmul(ps_im[:], lr, C["d1m_i"][:], start=True, stop=False)
                nc.tensor.matmul(ps_re[:], li, C["d1m_in"][:], start=False, stop=True)
                nc.tensor.matmul(ps_im[:], li, C["d1m_r"][:], start=False, stop=True)
                nc.scalar.copy(ctr[:, gl * 128:(gl + 1) * 128], ps_re[:])
                nc.scalar.copy(cti[:, gl * 128:(gl + 1) * 128], ps_im[:])

            # forward twiddle (broadcast [128,128] tile over g)
            twc_b = C["twc"][:].unsqueeze(1).broadcast_to((128, G, 128))
            tws_b = C["tws"][:].unsqueeze(1).broadcast_to((128, G, 128))
            cttr = work.tile([128, 1024], bf, tag="cttr", bufs=2)
            ctti = work.tile([128, 1024], bf, tag="ctti", bufs=2)
            v3 = lambda t: t[:].rearrange("p (a b) -> p a b", a=G)
            tA = work.tile([128, 1024], bf, name="tA", tag="s1")
            tB = work.tile([128, 1024], bf, name="tB", tag="s2")
            nc.vector.tensor_mul(v3(tA), v3(ctr), twc_b)
            nc.gpsimd.tensor_mul(v3(tB), v3(cti), tws_b)
            nc.vector.tensor_sub(cttr[:], tA[:], tB[:])
            nc.gpsimd.tensor_mul(v3(tA), v3(ctr), tws_b)
            nc.vector.tensor_mul(v3(tB), v3(cti), twc_b)
            nc.vector.tensor_add(ctti[:], tA[:], tB[:])
            if debug_taps:
                nc.sync.dma_start(taps["t_ct_r"][:, g0*128:(g0+G)*128], cttr[:])
                nc.sync.dma_start(taps["t_ct_i"][:, g0*128:(g0+G)*128], ctti[:])

            # stage B -> Z ; Zc via F1/k0 path
            zr = work.tile([128, 1024], bf, tag="zr", bufs=2)
            zi = work.tile([128, 1024], bf, tag="zi", bufs=2)
            for (dst, m1a, m1b, m2a, m2b) in (
                    (zr, "d2m_r", "d2m_in", None, None),
                    (zi, "d2m_i", "d2m_r", None, None)):
                for c in range(2):
                    ps = pp.tile([128, 512], f32, name="psb", tag="pbig", bufs=4)
                    cols = slice(c * 512, (c + 1) * 512)
                    for h_ in range(2):
                        rows = slice(h_ * 64, (h_ + 1) * 64)
                        nc.tensor.matmul(ps[rows, :], C[m1a][rows, :],
                                         cttr[rows, cols], start=True, stop=False)
                        nc.tensor.matmul(ps[rows, :], C[m1b][rows, :],
                                         ctti[rows, cols], start=False, stop=True)
                    nc.scalar.copy(dst[:, cols], ps[:])

            zcr = work.tile([128, 1024], bf, tag="zcr", bufs=2)
            zci = work.tile([128, 1024], bf, tag="zci", bufs=2)
            # main part k1 in [1,128): rhs cols reversed within each g block
            for (dst, ma, mb) in ((zcr, "f1_r", "f1_i"), (zci, "f1_i", "f1_rn")):
                for c in range(2):   # 4 g per chunk
                    psf = pp.tile([128, 512], f32, name="psc", tag="pbig", bufs=4)
                    ps = psf[:, 0:508]
                    for h_ in range(2):
                        rows = slice(h_ * 64, (h_ + 1) * 64)
                        rev_r = cttr[rows, :].rearrange(
                            "p (a b) -> p a b", a=G)[:, c * 4:(c + 1) * 4, 127:0:-1]
                        rev_i = ctti[rows, :].rearrange(
                            "p (a b) -> p a b", a=G)[:, c * 4:(c + 1) * 4, 127:0:-1]
                        nc.tensor.matmul(ps[rows, :].rearrange(
                            "p (a b) -> p a b", a=4), C[ma][rows, :], rev_r,
                            start=True, stop=False)
                        nc.tensor.matmul(ps[rows, :].rearrange(
                            "p (a b) -> p a b", a=4), C[mb][rows, :], rev_i,
                            start=False, stop=True)
                    # scatter 127-col blocks into dst cols g*128+1 ..
                    nc.scalar.copy(
                        dst[:].rearrange("p (a b) -> p a b", a=G)
                           [:, c * 4:(c + 1) * 4, 1:128],
                        ps[:].rearrange("p (a b) -> p a b", a=4))
            # k1 = 0 columns
            for (dst, ma, mb) in ((zcr, "d2m_r", "d2m_i"), (zci, "d2m_i", "d2m_rn")):
                ps = pp.tile([128, 8], f32, name="psk", tag="pd_re", bufs=1)
                r0 = cttr[:].rearrange("p (a b) -> p a b", a=G)[:, :, 0:1]
                i0 = ctti[:].rearrange("p (a b) -> p a b", a=G)[:, :, 0:1]
                for h_ in range(2):
                    rows = slice(h_ * 64, (h_ + 1) * 64)
                    nc.tensor.matmul(ps[rows, :].rearrange("p (a b) -> p a b", a=G),
                                     C[ma][rows, :], r0[h_ * 64:(h_ + 1) * 64],
                                     start=True, stop=False)
                    nc.tensor.matmul(ps[rows, :].rearrange("p (a b) -> p a b", a=G),
                                     C[mb][rows, :], i0[h_ * 64:(h_ + 1) * 64],
                                     start=False, stop=True)
                nc.scalar.copy(
                    dst[:].rearrange("p (a b) -> p a b", a=G)[:, :, 0:1],
                    ps[:].rearrange("p (a b) -> p a b", a=G))

            if debug_taps:
                for tp, t in (("t_z_r", zr), ("t_z_i", zi),
                              ("t_zc_r", zcr), ("t_zc_i", zci)):
                    nc.sync.dma_start(taps[tp][:, g0*128:(g0+G)*128], t[:])

            # pointwise: Zv = A*Z + B*Zc
            ab = pw.tile([128, 4, 1024], bf, tag="ab")
            cols = slice(g0 * 128, (g0 + G) * 128)
            nc.sync.dma_start(
                ab[:],
                dram["fields"][:].rearrange("p (f c) -> p f c", f=4)[:, :, cols])
            ar, ai, br, bi = ab[:, 0], ab[:, 1], ab[:, 2], ab[:, 3]
            zvr = work.tile([128, 1024], bf, tag="zvr", bufs=2)
            zvi = work.tile([128, 1024], bf, tag="zvi", bufs=2)
            p1 = work.tile([128, 1024], bf, name="p1", tag="s1")
            p2 = work.tile([128, 1024], bf, name="p2", tag="s2")
            p3 = work.tile([128, 1024], bf, name="p3", tag="s3")
            p4 = work.tile([128, 1024], bf, name="p4", tag="s4")
            nc.vector.tensor_mul(p1[:], zr[:], ar)
            nc.gpsimd.tensor_mul(p2[:], zi[:], ai)
            nc.gpsimd.tensor_mul(p3[:], zcr[:], br)
            nc.vector.tensor_mul(p4[:], zci[:], bi)
            nc.vector.tensor_sub(p1[:], p1[:], p2[:])
            nc.vector.tensor_sub(p3[:], p3[:], p4[:])
            nc.vector.tensor_add(zvr[:], p1[:], p3[:])
            nc.gpsimd.tensor_mul(p1[:], zi[:], ar)
            nc.vector.tensor_mul(p2[:], zr[:], ai)
            nc.vector.tensor_mul(p3[:], zci[:], br)
            nc.gpsimd.tensor_mul(p4[:], zcr[:], bi)
            nc.vector.tensor_add(p1[:], p1[:], p2[:])
            nc.vector.tensor_add(p3[:], p3[:], p4[:])
            nc.vector.tensor_add(zvi[:], p1[:], p3[:])
            if debug_taps:
                nc.sync.dma_start(taps["t_zv_r"][:, cols], zvr[:])
                nc.sync.dma_start(taps["t_zv_i"][:, cols], zvi[:])

            # B'-rev: C3[k1, (h,j2)] per (g, h')
            c3r = work.tile([128, 1024], bf, tag="c3r", bufs=2)
            c3i = work.tile([128, 1024], bf, tag="c3i", bufs=2)
            for gl in range(G):
                for h_ in range(2):
                    rows = slice(h_ * 64, (h_ + 1) * 64)
                    lr = zvr[rows, gl * 128:(gl + 1) * 128]
                    li = zvi[rows, gl * 128:(gl + 1) * 128]
                    ps_re = pp.tile([128, 64], f32, name="psD_re", tag="pd_re", bufs=1)
                    ps_im = pp.tile([128, 64], f32, name="psD_im", tag="pd_im", bufs=1)
                    nc.tensor.matmul(ps_re[:], lr, C["d2i_r"][rows, :], start=True, stop=False)
                    nc.tensor.matmul(ps_im[:], lr, C["d2i_i"][rows, :], start=True, stop=False)
                    nc.tensor.matmul(ps_re[:], li, C["d2i_in"][rows, :], start=False, stop=True)
                    nc.tensor.matmul(ps_im[:], li, C["d2i_r"][rows, :], start=False, stop=True)
                    oc = (2 * gl + h_) * 64
                    nc.scalar.copy(c3r[:, oc:oc + 64], ps_re[:])
                    nc.vector.tensor_copy(c3i[:, oc:oc + 64], ps_im[:])
            if debug_taps:
                nc.sync.dma_start(taps["t_c3_r"][:, cols], c3r[:])
                nc.sync.dma_start(taps["t_c3_i"][:, cols], c3i[:])

            # inverse twiddle (broadcast [128,64] over h=16)
            twic_b = C["twic"][:].unsqueeze(1).broadcast_to((128, 16, 64))
            twis_b = C["twis"][:].unsqueeze(1).broadcast_to((128, 16, 64))
            v3h = lambda t: t[:].rearrange("p (a b) -> p a b", a=16)
            c3tr = work.tile([128, 1024], bf, tag="c3tr", bufs=2)
            c3ti = work.tile([128, 1024], bf, tag="c3ti", bufs=2)
            tC = work.tile([128, 1024], bf, name="tC", tag="s1")
            tD = work.tile([128, 1024], bf, name="tD", tag="s2")
            nc.vector.tensor_mul(v3h(tC), v3h(c3r), twic_b)
            nc.gpsimd.tensor_mul(v3h(tD), v3h(c3i), twis_b)
            nc.vector.tensor_sub(c3tr[:], tC[:], tD[:])
            nc.gpsimd.tensor_mul(v3h(tC), v3h(c3r), twis_b)
            nc.vector.tensor_mul(v3h(tD), v3h(c3i), twic_b)
            nc.vector.tensor_add(c3ti[:], tC[:], tD[:])

            # stage A' -> vE, vO [j1<64, (h, j2)]
            ve = work.tile([64, 1024], bf, tag="ve")
            vo = work.tile([64, 1024], bf, tag="vo")
            ve_acts, vo_acts = [], []
            for (dst, acts, ma, mb) in ((ve, ve_acts, "d1i_r", "d1i_in"),
                                        (vo, vo_acts, "d1i_i", "d1i_r")):
                for c in range(2):
                    cols2 = slice(c * 512, (c + 1) * 512)
                    ps = pp.tile([64, 512], f32, name="pse", tag="pbig", bufs=4)
                    nc.tensor.matmul(ps[:], C[ma][:], c3tr[:, cols2], start=True, stop=False)
                    nc.tensor.matmul(ps[:], C[mb][:], c3ti[:, cols2], start=False, stop=True)
                    acts.append(nc.scalar.copy(dst[:, cols2], ps[:]))

            # scatter into DRAM bounce [8gg][64 j1][16 h][64 j2]
            for dvt, (srct, acts) in ((dv_e, (ve, ve_acts)), (dv_o, (vo, vo_acts))):
                dma = nc.sync.dma_start(dvt[gg], srct[:])
                for a in acts:
                    add_dep_helper(dma.ins, a.ins, reason="scatter after A' evac")
                fin_scat.append(dma)

        for dst, dvt in ((yE, dv_e), (yO, dv_o)):
            for a in range(NG):
                gat = nc.sync.dma_start(
                    dst[a * 16:(a + 1) * 16, :].rearrange("h (j b) -> h j b", j=64),
                    dvt[a].rearrange("j (hl b) -> hl j b", hl=16))
                for s in fin_scat:
                    add_dep_helper(gat.ins, s.ins, reason="y gather after scatters")

        if debug_taps:
            nc.sync.dma_start(taps["t_ye"][:], yE[:])
            nc.sync.dma_start(taps["t_yo"][:], yO[:])

        # ---- gelu + GLU + pool ----------------------------------------
        CG = 0.7978845608028654
        for pl in (yE, yO):
            for c in range(4):
                cols = slice(c * 1024, (c + 1) * 1024)
                xc = pl[:, cols]
                sq = work.tile([128, 1024], bf, name="sq", tag="s1")
                nc.scalar.activation(sq[:], xc, AF.Square)
                rr = work.tile([128, 1024], bf, name="rr", tag="s2")
                nc.scalar.activation(rr[:], sq[:], AF.Identity,
                                     bias=C["ones_c"][:, 0:1], scale=0.044715)
                qq = work.tile([128, 1024], bf, name="qq", tag="s3")
                nc.vector.tensor_mul(qq[:], xc, rr[:])
                tt = work.tile([128, 1024], bf, name="tt", tag="s4")
                nc.scalar.activation(tt[:], qq[:], AF.Tanh, scale=CG)
                uu = work.tile([128, 1024], bf, name="uu", tag="s1")
                nc.scalar.activation(uu[:], tt[:], AF.Identity,
                                     bias=C["half_c"][:, 0:1], scale=0.5)
                nc.vector.tensor_mul(xc, xc, uu[:])

        scratch = work.tile([128, 512], bf, tag="glu_scratch")
        idx = 0
        for plane in (yE, yO):
            for c in range(8):
                cols = slice(c * 512, (c + 1) * 512)
                ps_a = pp.tile([128, 512], f32, tag="pbig", bufs=4)
                ps_g = pp.tile([128, 512], f32, tag="pbig", bufs=4)
                nc.tensor.matmul(ps_a[:], C["glu_lhsT"][:, 0:128], plane[:, cols],
                                 start=True, stop=True)
                nc.tensor.matmul(ps_g[:], C["glu_lhsT"][:, 128:256], plane[:, cols],
                                 start=True, stop=True)
                sig = work.tile([128, 512], bf, tag="glu_sig")
                nc.scalar.activation(sig[:], ps_g[:], AF.Sigmoid,
                                     bias=C["glu_bg"][:, 0:1], scale=1.0)
                nc.vector.scalar_tensor_tensor(
                    scratch[:], ps_a[:], C["glu_ba"][:, 0:1], sig[:],
                    op0=ALU.add, op1=ALU.mult,
                    accum_out=pool_cols[:, idx:idx + 1])
                idx += 1

        pool_t = work.tile([128, 1], f32, tag="pool_t")
        nc.vector.tensor_reduce(pool_t[:], pool_cols[:],
                                axis=mybir.AxisListType.X, op=ALU.add)
        nc.sync.dma_start(pool_out[:], pool_t[:])

        _stack.close()

    nc.compile()
    return nc


_CACHED_NC = None


def kernel(**inputs):
    global _CACHED_NC
    from concourse.bass_utils import run_bass_kernel_spmd

    shared, per_core = host_prep(inputs)
    if _CACHED_NC is None:
        _CACHED_NC = build_program()
    nc = _CACHED_NC

    in_maps = [{**shared, **pc} for pc in per_core]
    res = run_bass_kernel_spmd(nc, in_maps, list(range(B)))
    pool = np.stack([np.asarray(res.results[b]["pool"][:, 0], np.float64)
                     for b in range(B)])                     # (8, 128)
    pooled = pool / float(L)
    dec_w = np.asarray(inputs["dec_w"], np.float64)
    dec_b = np.asarray(inputs["dec_b"], np.float64)
    return (pooled @ dec_w + dec_b).astype(np.float32)


if __name__ == "__main__":
    ins = {
        "x": np.random.randn(B, L, 2).astype(np.float32),
        "enc_w": np.random.randn(2, H).astype(np.float32),
        "enc_b": np.random.randn(H).astype(np.float32),
        "log_dt": np.random.rand(H).astype(np.float32),
        "log_A_real": np.random.randn(H, 32).astype(np.float32),
        "A_imag": np.random.randn(H, 32).astype(np.float32),
        "C_re": np.random.randn(H, 32).astype(np.float32),
        "C_im": np.random.randn(H, 32).astype(np.float32),
        "D": np.random.randn(H).astype(np.float32),
        "out_w": np.random.randn(2 * H, H).astype(np.float32),
        "out_b": np.random.randn(2 * H).astype(np.float32),
        "dec_w": np.random.randn(H, 1).astype(np.float32),
        "dec_b": np.random.randn(1).astype(np.float32),
    }
    print(kernel(**ins).shape)


# revision 32
# speedup vs baseline: 1.6205x; 1.1082x over previous
"""S4D AddingModel — Bass/Tile kernel for 8 Trainium2 NeuronCores.

Strategy (data-parallel over batch B=8, one batch element per core):
  encoder matmul -> packed complex z (even/odd samples) -> four-step
  FFT_8192 (stage A over j1 via reverse-matmul, twiddle, stage B over j2)
  -> fused pointwise  Zv[k] = A[k]*Z[k] + B[k]*conj(Z[8192-k])  where the
  host-precomputed A/B fields absorb the rfft unpack, the S4D kernel
  transfer function (incl. the D skip term), and the repack -> mirrored
  inverse four-step -> gelu -> GLU projection -> mean-pool partial sums.

The S4D kernel construction + its rFFT + the A/B fields are tiny
parameter-only computations done on host (numpy).  All O(B*H*L) work runs
on the NeuronCores in one NEFF.

Shapes hardcoded: B=8, L=8192, H=128, N=32.
"""
import numpy as np
import ml_dtypes

B, L, H = 8, 8192, 128
M = 8192          # packed complex FFT length
M1, M2 = 128, 64  # j = j1*64 + j2 ; k = k2*128 + k1
G = 8             # g-chunks per group
NG = 8            # number of groups (NG*G = 64 chunks of 128 cols)

_BF = ml_dtypes.bfloat16


# ---------------------------------------------------------------------------
# host-side constants
# ---------------------------------------------------------------------------

def _host_fields(log_dt, log_A_real, A_imag, C_re, C_im, D):
    """S4D kernel K, its 2L rfft, and the packed-pointwise A/B fields."""
    dt = np.exp(log_dt.astype(np.float64))
    A = -np.exp(log_A_real.astype(np.float64)) + 1j * A_imag.astype(np.float64)
    C = C_re.astype(np.float64) + 1j * C_im.astype(np.float64)
    dtA = dt[:, None] * A
    K_coef = C * (np.exp(dtA) - 1.0) / A
    w = np.exp(dtA)
    Tb = 128
    J = L // Tb
    v_lo = w[:, :, None] ** np.arange(Tb)
    v_hi = (w ** Tb)[:, :, None] ** np.arange(J)
    K = 2.0 * np.matmul(K_coef[:, None, :] * v_hi.transpose(0, 2, 1),
                        v_lo).real.reshape(H, L)

    Khat = np.fft.rfft(K, 2 * L, axis=-1)              # (H, 8193)
    Khat = Khat + D.astype(np.float64)[:, None]        # fold skip y += D*u
    k = np.arange(M)
    P = Khat[:, :M]
    idx = (M - k) % (2 * L)
    Q = np.conj(Khat[:, idx])
    Q[:, 0] = Khat[:, M]
    th = 2.0 * np.pi * k / (2 * L)
    Afld = 0.5 * (P + Q) - 0.5 * (P - Q) * np.sin(th)[None, :]
    Bfld = 0.5j * (P - Q) * np.cos(th)[None, :]
    return Afld, Bfld                                   # (H, 8192) complex


def _pack_field(F):
    """(H, 8192) field -> device plane [128=(h',k2), 8192=(g,k1)]."""
    Fg = F.reshape(H, M2, M1)                           # [h, k2, k1]
    P = Fg.reshape(64, 2, M2, M1).transpose(1, 2, 0, 3)  # [h', k2, g, k1]
    return np.ascontiguousarray(P.reshape(128, 8192))


def _dup(mat):
    """[64, X] -> [128, X] duplicated halves (for base-partition 0/64 use)."""
    return np.concatenate([mat, mat], axis=0)


def host_prep(inputs):
    """Returns (shared_map, per_core_maps, dec_w, dec_b)."""
    f32 = np.float32
    x = np.asarray(inputs["x"], f32)
    Afld, Bfld = _host_fields(inputs["log_dt"], inputs["log_A_real"],
                              inputs["A_imag"], inputs["C_re"],
                              inputs["C_im"], inputs["D"])

    j1 = np.arange(64)
    k1 = np.arange(M1)
    j2 = np.arange(M2)
    k2 = np.arange(M2)

    def bf(a):
        return np.ascontiguousarray(a, dtype=np.float32).astype(_BF)

    shared = {}
    shared["enc_lhsT"] = bf(inputs["enc_w"])                      # [2, 128]
    shared["enc_bias"] = np.asarray(inputs["enc_b"], f32).reshape(128, 1)

    th = 2 * np.pi * np.outer(j1, k1) / M1                        # [64, 128]
    shared["d1m_r"] = bf(_dup(np.cos(th)))
    shared["d1m_i"] = bf(_dup(-np.sin(th)))
    shared["d1m_in"] = bf(_dup(np.sin(th)))

    p = np.arange(128) % 64
    th = 2 * np.pi * np.outer(p, k1) / M                          # [128, 128]
    shared["twc"] = bf(np.cos(th))
    shared["tws"] = bf(-np.sin(th))

    th = 2 * np.pi * np.outer(j2, k2) / M2                        # [64, 64]
    shared["d2m_r"] = bf(_dup(np.cos(th)))
    shared["d2m_i"] = bf(_dup(-np.sin(th)))
    shared["d2m_in"] = bf(_dup(np.sin(th)))
    shared["d2m_rn"] = bf(_dup(-np.cos(th)))

    th = 2 * np.pi * np.outer(j2, 63 - k2) / M2                   # [64, 64]
    shared["f1_r"] = bf(_dup(np.cos(th)))
    shared["f1_i"] = bf(_dup(np.sin(th)))
    shared["f1_rn"] = bf(_dup(-np.cos(th)))

    th = 2 * np.pi * np.outer(k2, j2) / M2                        # [64, 64]
    shared["d2i_r"] = bf(_dup(np.cos(th)))
    shared["d2i_i"] = bf(_dup(np.sin(th)))
    shared["d2i_in"] = bf(_dup(-np.sin(th)))

    th = 2 * np.pi * np.outer(np.arange(M1), j2) / M              # [128, 64]
    shared["twic"] = bf(np.cos(th))
    shared["twis"] = bf(np.sin(th))

    th = 2 * np.pi * np.outer(np.arange(M1), j1) / M1             # [128, 64]
    shared["d1i_r"] = bf(np.cos(th) / M)
    shared["d1i_i"] = bf(np.sin(th) / M)
    shared["d1i_in"] = bf(-np.sin(th) / M)

    shared["glu_lhsT"] = bf(np.asarray(inputs["out_w"], f32).T)   # [128, 256]
    ob = np.asarray(inputs["out_b"], f32)
    shared["glu_ba"] = ob[:128].reshape(128, 1).astype(f32)
    shared["glu_bg"] = ob[128:].reshape(128, 1).astype(f32)
    shared["ones_c"] = np.ones((128, 1), f32)
    shared["half_c"] = np.full((128, 1), 0.5, f32)

    shared["fields"] = np.concatenate(
        [bf(_pack_field(p)) for p in (Afld.real, Afld.imag,
                                      Bfld.real, Bfld.imag)], axis=1)

    bf_names = ["enc_lhsT", "d1m_r", "d1m_i", "d1m_in", "twc", "tws",
                "d2m_r", "d2m_i", "d2m_in", "d2m_rn", "f1_r", "f1_i", "f1_rn",
                "d2i_r", "d2i_i", "d2i_in", "twic", "twis",
                "d1i_r", "d1i_i", "d1i_in", "glu_lhsT"]
    blocks = []
    for nm in bf_names:
        a = shared.pop(nm)
        if a.shape[0] != 128:
            pad = np.zeros((128 - a.shape[0], a.shape[1]), a.dtype)
            a = np.concatenate([a, pad], axis=0)
        blocks.append(a)
    shared["cpack"] = np.concatenate(blocks, axis=1)
    f32_names = ["enc_bias", "glu_ba", "glu_bg", "ones_c", "half_c"]
    shared["fpack"] = np.concatenate([shared.pop(nm) for nm in f32_names],
                                     axis=1).astype(f32)

    per_core = []
    for b in range(B):
        xb = x[b]                                                 # (8192, 2)
        per_core.append({
            "xe": bf(xb[0::2, :].T),                              # [2, 4096]
            "xo": bf(xb[1::2, :].T),                              # [2, 4096]
        })
    return shared, per_core


# ---------------------------------------------------------------------------
# device program
# ---------------------------------------------------------------------------

_SHARED_SPECS = [
    ("enc_lhsT", (2, 128), "bf"), ("enc_bias", (128, 1), "f32"),
    ("d1m_r", (128, 128), "bf"), ("d1m_i", (128, 128), "bf"),
    ("d1m_in", (128, 128), "bf"),
    ("twc", (128, 128), "bf"), ("tws", (128, 128), "bf"),
    ("d2m_r", (128, 64), "bf"), ("d2m_i", (128, 64), "bf"),
    ("d2m_in", (128, 64), "bf"), ("d2m_rn", (128, 64), "bf"),
    ("f1_r", (128, 64), "bf"), ("f1_i", (128, 64), "bf"),
    ("f1_rn", (128, 64), "bf"),
    ("d2i_r", (128, 64), "bf"), ("d2i_i", (128, 64), "bf"),
    ("d2i_in", (128, 64), "bf"),
    ("twic", (128, 64), "bf"), ("twis", (128, 64), "bf"),
    ("d1i_r", (128, 64), "bf"), ("d1i_i", (128, 64), "bf"),
    ("d1i_in", (128, 64), "bf"),
    ("glu_lhsT", (128, 256), "bf"), ("glu_ba", (128, 1), "f32"),
    ("glu_bg", (128, 1), "f32"), ("ones_c", (128, 1), "f32"),
    ("half_c", (128, 1), "f32"),
    ("fields", (128, 4 * 8192), "bf"),
]


def build_program(debug_taps=False):
    """Build + compile the single-core SPMD bass program."""
    import concourse.bass as bass
    import concourse.tile as tile
    from concourse import bacc, mybir

    bf = mybir.dt.bfloat16
    f32 = mybir.dt.float32
    AF = mybir.ActivationFunctionType
    ALU = mybir.AluOpType

    nc = bacc.Bacc("TRN2", target_bir_lowering=False, debug=False,
                   num_devices=B)

    dram = {}
    for name, shape, dt_ in _SHARED_SPECS:
        dram[name] = nc.dram_tensor(name, list(shape),
                                    bf if dt_ == "bf" else f32,
                                    kind="ExternalInput").ap()
    dram["xe"] = nc.dram_tensor("xe", [2, 4096], bf, kind="ExternalInput").ap()
    dram["xo"] = nc.dram_tensor("xo", [2, 4096], bf, kind="ExternalInput").ap()
    pool_out = nc.dram_tensor("pool", [128, 1], f32, kind="ExternalOutput").ap()
    taps = {}
    if debug_taps:
        for nm in ("t_ct_r", "t_ct_i", "t_z_r", "t_z_i", "t_zc_r", "t_zc_i",
                   "t_zv_r", "t_zv_i", "t_c3_r", "t_c3_i"):
            taps[nm] = nc.dram_tensor(nm, [128, 8192], bf,
                                      kind="ExternalOutput").ap()
        for nm in ("t_ye", "t_yo"):
            taps[nm] = nc.dram_tensor(nm, [128, 4096], bf,
                                      kind="ExternalOutput").ap()

    with tile.TileContext(nc) as tc:
        from contextlib import ExitStack
        _stack = ExitStack()
        cpool = _stack.enter_context(tc.tile_pool(name="consts", bufs=1))
        C = {}
        for name, shape, dt_ in _SHARED_SPECS:
            if name == "fields":
                continue
            ct = cpool.tile(list(shape), bf if dt_ == "bf" else f32,
                            name="c_" + name, tag=name)
            nc.sync.dma_start(ct[:], dram[name][:])
            C[name] = ct[:]

        persist = _stack.enter_context(tc.tile_pool(name="persist", bufs=1))
        t1 = persist.tile([128, 8192], bf, tag="t1")
        yE = persist.tile([128, 4096], bf, tag="yE")
        yO = persist.tile([128, 4096], bf, tag="yO")
        pool_cols = persist.tile([128, 16], f32, tag="pool_cols")



        work = _stack.enter_context(tc.tile_pool(name="work", bufs=2))
        pw = _stack.enter_context(tc.tile_pool(name="pw", bufs=2))
        pp = _stack.enter_context(tc.tile_pool(name="pp", bufs=1, space="PSUM"))

        # ---- encoder -> DRAM bounce -> T1[j1, (h,j2)] ------------------
        from concourse.tile import add_dep_helper
        dz = {0: nc.dram_tensor("dz_r", [128, 4096], bf, kind="Internal").ap(),
              1: nc.dram_tensor("dz_i", [128, 4096], bf, kind="Internal").ap()}
        for pi, (dst, src) in enumerate(((t1[0:64, :], dram["xe"]),
                                         (t1[64:128, :], dram["xo"]))):
            scat = []
            for c in range(8):
                xch = work.tile([2, 512], bf, tag="xch")
                nc.sync.dma_start(xch[:], src[:, c * 512:(c + 1) * 512])
                pse2 = pp.tile([128, 512], f32, name="pse2", tag="pbig", bufs=4)
                nc.tensor.matmul(pse2[:], C["enc_lhsT"][:], xch[:],
                                 start=True, stop=True)
                zch = work.tile([128, 512], bf, tag="zch")
                nc.scalar.activation(zch[:], pse2[:], AF.Identity,
                                     bias=C["enc_bias"], scale=1.0)
                scat.append(nc.sync.dma_start(
                    dz[pi][:, c * 512:(c + 1) * 512], zch[:]))
            gat = nc.sync.dma_start(
                dst.rearrange("a (h b) -> a h b", h=128),
                dz[pi][:].rearrange("h (a b) -> h a b", a=64).transpose([1, 0, 2]))
            for s in scat:
                add_dep_helper(gat.ins, s.ins, reason="t1 gather after scatter")

        # ---- main groups ----------------------------------------------
        dv_e = nc.dram_tensor("dv_e", [NG, 64, 1024], bf, kind="Internal").ap()
        dv_o = nc.dram_tensor("dv_o", [NG, 64, 1024], bf, kind="Internal").ap()
        fin_scat = []
        for gg in range(NG):
            g0 = gg * G
            # A-rev: Ct[(h',j2), (g,k1)] chunks
            ctr = work.tile([128, 1024], bf, tag="ctr", bufs=3)
            cti = work.tile([128, 1024], bf, tag="cti", bufs=3)
            for gl in range(G):
                g = g0 + gl
                lr = t1[0:64, g * 128:(g + 1) * 128]
                li = t1[64:128, g * 128:(g + 1) * 128]
                ps_re = pp.tile([128, 128], f32, name="psA_re", tag="pa", bufs=2)
                nc.tensor.matmul(ps_re[:], lr, C["d1m_r"][0:64, :], start=True, stop=False)
                nc.tensor.matmul(ps_re[:], li, C["d1m_in"][64:128, :], start=False, stop=True)
                nc.scalar.copy(ctr[:, gl * 128:(gl + 1) * 128], ps_re[:])
                ps_im = pp.tile([128, 128], f32, name="psA_im", tag="pa", bufs=2)
                nc.tensor.matmul(ps_im[:], lr, C["d1m_i"][0:64, :], start=True, stop=False)
                nc.tensor.matmul(ps_im[:], li, C["d1m_r"][64:128, :], start=False, stop=True)
                nc.vector.tensor_copy(cti[:, gl * 128:(gl + 1) * 128], ps_im[:])

            # forward twiddle (broadcast [128,128] tile over g)
            twc_b = C["twc"][:].unsqueeze(1).broadcast_to((128, G, 128))
            tws_b = C["tws"][:].unsqueeze(1).broadcast_to((128, G, 128))
            cttr = work.tile([128, 1024], bf, tag="cttr", bufs=3)
            ctti = work.tile([128, 1024], bf, tag="ctti", bufs=3)
            v3 = lambda t: t[:].rearrange("p (a b) -> p a b", a=G)
            tA = work.tile([128, 1024], bf, name="tA", tag="s1")
            tB = work.tile([128, 1024], bf, name="tB", tag="s2")
            nc.vector.tensor_mul(v3(tA), v3(ctr), twc_b)
            nc.gpsimd.tensor_mul(v3(tB), v3(cti), tws_b)
            nc.vector.tensor_sub(cttr[:], tA[:], tB[:])
            nc.gpsimd.tensor_mul(v3(tA), v3(ctr), tws_b)
            nc.vector.tensor_mul(v3(tB), v3(cti), twc_b)
            nc.vector.tensor_add(ctti[:], tA[:], tB[:])
            if debug_taps:
                nc.sync.dma_start(taps["t_ct_r"][:, g0*128:(g0+G)*128], cttr[:])
                nc.sync.dma_start(taps["t_ct_i"][:, g0*128:(g0+G)*128], ctti[:])

            # stage B -> Z ; Zc via F1/k0 path
            zr = work.tile([128, 1024], bf, tag="zr", bufs=3)
            zi = work.tile([128, 1024], bf, tag="zi", bufs=3)
            for (dst, m1a, m1b) in ((zr, "d2m_r", "d2m_in"),
                                    (zi, "d2m_i", "d2m_r")):
                for c in range(2):
                    ps = pp.tile([128, 512], f32, name="psb", tag="pbig", bufs=4)
                    cols = slice(c * 512, (c + 1) * 512)
                    for h_ in range(2):
                        rows = slice(h_ * 64, (h_ + 1) * 64)
                        nc.tensor.matmul(ps[rows, :], C[m1a][rows, :],
                                         cttr[rows, cols], start=True, stop=False)
                        nc.tensor.matmul(ps[rows, :], C[m1b][rows, :],
                                         ctti[rows, cols], start=False, stop=True)
                    nc.scalar.copy(dst[:, cols], ps[:])

            zcr = work.tile([128, 1024], bf, tag="zcr", bufs=3)
            zci = work.tile([128, 1024], bf, tag="zci", bufs=3)
            # main part k1 in [1,128): rhs cols reversed within each g block
            for (dst, ma, mb) in ((zcr, "f1_r", "f1_i"), (zci, "f1_i", "f1_rn")):
                for c in range(2):   # 4 g per chunk
                    psz = pp.tile([128, 512], f32, name="psc", tag="pbig", bufs=4)
                    ps = psz[:, 0:508]
                    for h_ in range(2):
                        rows = slice(h_ * 64, (h_ + 1) * 64)
                        rev_r = cttr[rows, :].rearrange(
                            "p (a b) -> p a b", a=G)[:, c * 4:(c + 1) * 4, 127:0:-1]
                        rev_i = ctti[rows, :].rearrange(
                            "p (a b) -> p a b", a=G)[:, c * 4:(c + 1) * 4, 127:0:-1]
                        nc.tensor.matmul(ps[rows, :].rearrange(
                            "p (a b) -> p a b", a=4), C[ma][rows, :], rev_r,
                            start=True, stop=False)
                        nc.tensor.matmul(ps[rows, :].rearrange(
                            "p (a b) -> p a b", a=4), C[mb][rows, :], rev_i,
                            start=False, stop=True)
                    nc.scalar.copy(
                        dst[:].rearrange("p (a b) -> p a b", a=G)
                           [:, c * 4:(c + 1) * 4, 1:128],
                        ps.rearrange("p (c b) -> p c b", c=4))
            # k1 = 0 columns
            for (dst, ma, mb) in ((zcr, "d2m_r", "d2m_i"), (zci, "d2m_i", "d2m_rn")):
                psk = pp.tile([128, 128], f32, name="psk", tag="pa", bufs=2)
                ps = psk[:, 0:8]
                r0 = cttr[:].rearrange("p (a b) -> p a b", a=G)[:, :, 0:1]
                i0 = ctti[:].rearrange("p (a b) -> p a b", a=G)[:, :, 0:1]
                for h_ in range(2):
                    rows = slice(h_ * 64, (h_ + 1) * 64)
                    nc.tensor.matmul(ps[rows, :].rearrange("p (a b) -> p a b", a=G),
                                     C[ma][rows, :], r0[h_ * 64:(h_ + 1) * 64],
                                     start=True, stop=False)
                    nc.tensor.matmul(ps[rows, :].rearrange("p (a b) -> p a b", a=G),
                                     C[mb][rows, :], i0[h_ * 64:(h_ + 1) * 64],
                                     start=False, stop=True)
                nc.vector.tensor_copy(
                    dst[:].rearrange("p (a b) -> p a b", a=G)[:, :, 0:1],
                    ps.rearrange("p (a b) -> p a b", a=G))

            if debug_taps:
                for tp, t in (("t_z_r", zr), ("t_z_i", zi),
                              ("t_zc_r", zcr), ("t_zc_i", zci)):
                    nc.sync.dma_start(taps[tp][:, g0*128:(g0+G)*128], t[:])

            # pointwise: Zv = A*Z + B*Zc
            ab = pw.tile([128, 4, 1024], bf, tag="ab")
            cols = slice(g0 * 128, (g0 + G) * 128)
            nc.sync.dma_start(
                ab[:],
                dram["fields"][:].rearrange("p (f c) -> p f c", f=4)[:, :, cols])
            ar, ai, br, bi = ab[:, 0], ab[:, 1], ab[:, 2], ab[:, 3]
            zvr = work.tile([128, 1024], bf, tag="zvr", bufs=3)
            zvi = work.tile([128, 1024], bf, tag="zvi", bufs=3)
            p1 = work.tile([128, 1024], bf, name="p1", tag="s1")
            p2 = work.tile([128, 1024], bf, name="p2", tag="s2")
            p3 = work.tile([128, 1024], bf, name="p3", tag="s3")
            p4 = work.tile([128, 1024], bf, name="p4", tag="s4")
            nc.vector.tensor_mul(p1[:], zr[:], ar)
            nc.gpsimd.tensor_mul(p2[:], zi[:], ai)
            nc.gpsimd.tensor_mul(p3[:], zcr[:], br)
            nc.vector.tensor_mul(p4[:], zci[:], bi)
            nc.vector.tensor_sub(p1[:], p1[:], p2[:])
            nc.vector.tensor_sub(p3[:], p3[:], p4[:])
            nc.vector.tensor_add(zvr[:], p1[:], p3[:])
            nc.vector.tensor_mul(p1[:], zi[:], ar)
            nc.vector.tensor_mul(p2[:], zr[:], ai)
            nc.vector.tensor_mul(p3[:], zci[:], br)
            nc.vector.tensor_mul(p4[:], zcr[:], bi)
            nc.vector.tensor_add(p1[:], p1[:], p2[:])
            nc.vector.tensor_add(p3[:], p3[:], p4[:])
            nc.vector.tensor_add(zvi[:], p1[:], p3[:])
            if debug_taps:
                nc.sync.dma_start(taps["t_zv_r"][:, cols], zvr[:])
                nc.sync.dma_start(taps["t_zv_i"][:, cols], zvi[:])

            # B'-rev: C3[k1, (h,j2)] per (g, h')
            c3r = work.tile([128, 1024], bf, tag="c3r", bufs=3)
            c3i = work.tile([128, 1024], bf, tag="c3i", bufs=3)
            for gl in range(G):
                for h_ in range(2):
                    rows = slice(h_ * 64, (h_ + 1) * 64)
                    lr = zvr[rows, gl * 128:(gl + 1) * 128]
                    li = zvi[rows, gl * 128:(gl + 1) * 128]
                    oc = (2 * gl + h_) * 64
                    ps_re = pp.tile([128, 64], f32, name="psD_re", tag="pd", bufs=2)
                    nc.tensor.matmul(ps_re[:], lr, C["d2i_r"][rows, :], start=True, stop=False)
                    nc.tensor.matmul(ps_re[:], li, C["d2i_in"][rows, :], start=False, stop=True)
                    nc.scalar.copy(c3r[:, oc:oc + 64], ps_re[:])
                    ps_im = pp.tile([128, 64], f32, name="psD_im", tag="pd", bufs=2)
                    nc.tensor.matmul(ps_im[:], lr, C["d2i_i"][rows, :], start=True, stop=False)
                    nc.tensor.matmul(ps_im[:], li, C["d2i_r"][rows, :], start=False, stop=True)
                    nc.vector.tensor_copy(c3i[:, oc:oc + 64], ps_im[:])
            if debug_taps:
                nc.sync.dma_start(taps["t_c3_r"][:, cols], c3r[:])
                nc.sync.dma_start(taps["t_c3_i"][:, cols], c3i[:])

            # inverse twiddle (broadcast [128,64] over h=16)
            twic_b = C["twic"][:].unsqueeze(1).broadcast_to((128, 16, 64))
            twis_b = C["twis"][:].unsqueeze(1).broadcast_to((128, 16, 64))
            v3h = lambda t: t[:].rearrange("p (a b) -> p a b", a=16)
            c3tr = work.tile([128, 1024], bf, tag="c3tr", bufs=2)
            c3ti = work.tile([128, 1024], bf, tag="c3ti", bufs=2)
            tC = work.tile([128, 1024], bf, name="tC", tag="s1")
            tD = work.tile([128, 1024], bf, name="tD", tag="s2")
            nc.vector.tensor_mul(v3h(tC), v3h(c3r), twic_b)
            nc.gpsimd.tensor_mul(v3h(tD), v3h(c3i), twis_b)
            nc.vector.tensor_sub(c3tr[:], tC[:], tD[:])
            nc.gpsimd.tensor_mul(v3h(tC), v3h(c3r), twis_b)
            nc.vector.tensor_mul(v3h(tD), v3h(c3i), twic_b)
            nc.vector.tensor_add(c3ti[:], tC[:], tD[:])

            # stage A' -> vE, vO [j1<64, (h, j2)]
            ve = work.tile([64, 1024], bf, tag="ve")
            vo = work.tile([64, 1024], bf, tag="vo")
            ve_acts, vo_acts = [], []
            for (dst, acts, ma, mb) in ((ve, ve_acts, "d1i_r", "d1i_in"),
                                        (vo, vo_acts, "d1i_i", "d1i_r")):
                for c in range(2):
                    cols2 = slice(c * 512, (c + 1) * 512)
                    ps = pp.tile([64, 512], f32, name="pse", tag="pbig", bufs=4)
                    nc.tensor.matmul(ps[:], C[ma][:], c3tr[:, cols2], start=True, stop=False)
                    nc.tensor.matmul(ps[:], C[mb][:], c3ti[:, cols2], start=False, stop=True)
                    acts.append(nc.scalar.copy(dst[:, cols2], ps[:]))

            # scatter into DRAM bounce then gather this group's 16 y-rows
            for dvt, dst, (srct, acts) in ((dv_e, yE, (ve, ve_acts)),
                                           (dv_o, yO, (vo, vo_acts))):
                dma = nc.sync.dma_start(dvt[gg], srct[:])
                for a in acts:
                    add_dep_helper(dma.ins, a.ins, reason="scatter after A' evac")
                gat = nc.sync.dma_start(
                    yE[gg * 16:(gg + 1) * 16, :].rearrange("h (j b) -> h j b", j=64)
                    if dst is yE else
                    yO[gg * 16:(gg + 1) * 16, :].rearrange("h (j b) -> h j b", j=64),
                    dvt[gg].rearrange("j (hl b) -> hl j b", hl=16))
                add_dep_helper(gat.ins, dma.ins, reason="y gather after scatter")

        if debug_taps:
            nc.sync.dma_start(taps["t_ye"][:], yE[:])
            nc.sync.dma_start(taps["t_yo"][:], yO[:])

        # ---- gelu + GLU + pool ----------------------------------------
        CG = 0.7978845608028654
        gtiles = {}
        for i, pl in enumerate((yE, yO)):
            gtiles[i] = pw.tile([128, 4096], bf, name=f"gel{i}", tag="ab")
        for i, pl in enumerate((yE, yO)):   # sq = x*x
            nc.vector.tensor_mul(gtiles[i][:], pl[:], pl[:])
        for i, pl in enumerate((yE, yO)):   # rr = 0.044715*sq + 1
            nc.scalar.activation(gtiles[i][:], gtiles[i][:], AF.Identity,
                                 bias=C["ones_c"], scale=0.044715)
        for i, pl in enumerate((yE, yO)):   # qq = x*rr
            nc.vector.tensor_mul(gtiles[i][:], pl[:], gtiles[i][:])
        for i, pl in enumerate((yE, yO)):   # tt = tanh(CG*qq)
            nc.scalar.activation(gtiles[i][:], gtiles[i][:], AF.Tanh, scale=CG)
        for i, pl in enumerate((yE, yO)):   # uu = 0.5*tt + 0.5
            nc.scalar.activation(gtiles[i][:], gtiles[i][:], AF.Identity,
                                 bias=C["half_c"], scale=0.5)
        for i, pl in enumerate((yE, yO)):   # y = x*uu
            nc.vector.tensor_mul(pl[:], pl[:], gtiles[i][:])

        scratch = work.tile([128, 512], bf, tag="glu_scratch")
        idx = 0
        for plane in (yE, yO):
            for c in range(8):
                cols = slice(c * 512, (c + 1) * 512)
                ps_a = pp.tile([128, 512], f32, name="ps_a", tag="pbig", bufs=4)
                ps_g = pp.tile([128, 512], f32, name="ps_g", tag="pbig", bufs=4)
                nc.tensor.matmul(ps_a, C["glu_lhsT"][:, 0:128], plane[:, cols],
                                 start=True, stop=True)
                nc.tensor.matmul(ps_g, C["glu_lhsT"][:, 128:256], plane[:, cols],
                                 start=True, stop=True)
                sig = work.tile([128, 512], bf, tag="glu_sig")
                nc.scalar.activation(sig[:], ps_g, AF.Sigmoid,
                                     bias=C["glu_bg"], scale=1.0)
                nc.vector.scalar_tensor_tensor(
                    scratch[:], ps_a, C["glu_ba"], sig[:],
                    op0=ALU.add, op1=ALU.mult,
                    accum_out=pool_cols[:, idx:idx + 1])
                idx += 1

        pool_t = work.tile([128, 1], f32, tag="pool_t")
        nc.vector.tensor_reduce(pool_t[:], pool_cols[:],
                                axis=mybir.AxisListType.X, op=ALU.add)
        nc.sync.dma_start(pool_out[:], pool_t[:])

        _stack.close()

    nc.compile()
    return nc


_CACHED_NC = None


def kernel(**inputs):
    global _CACHED_NC
    from concourse.bass_utils import run_bass_kernel_spmd

    shared, per_core = host_prep(inputs)
    if _CACHED_NC is None:
        _CACHED_NC = build_program()
    nc = _CACHED_NC

    in_maps = [{**shared, **pc} for pc in per_core]
    res = run_bass_kernel_spmd(nc, in_maps, list(range(B)))
    pool = np.stack([np.asarray(res.results[b]["pool"][:, 0], np.float64)
                     for b in range(B)])                     # (8, 128)
    pooled = pool / float(L)
    dec_w = np.asarray(inputs["dec_w"], np.float64)
    dec_b = np.asarray(inputs["dec_b"], np.float64)
    return (pooled @ dec_w + dec_b).astype(np.float32)


if __name__ == "__main__":
    ins = {
        "x": np.random.randn(B, L, 2).astype(np.float32),
        "enc_w": np.random.randn(2, H).astype(np.float32),
        "enc_b": np.random.randn(H).astype(np.float32),
        "log_dt": np.random.rand(H).astype(np.float32),
        "log_A_real": np.random.randn(H, 32).astype(np.float32),
        "A_imag": np.random.randn(H, 32).astype(np.float32),
        "C_re": np.random.randn(H, 32).astype(np.float32),
        "C_im": np.random.randn(H, 32).astype(np.float32),
        "D": np.random.randn(H).astype(np.float32),
        "out_w": np.random.randn(2 * H, H).astype(np.float32),
        "out_b": np.random.randn(2 * H).astype(np.float32),
        "dec_w": np.random.randn(H, 1).astype(np.float32),
        "dec_b": np.random.randn(1).astype(np.float32),
    }
    print(kernel(**ins).shape)


# revision 39
# speedup vs baseline: 2.2488x; 1.3877x over previous
"""S4D AddingModel — Bass/Tile kernel for 8 Trainium2 NeuronCores.

Strategy (data-parallel over batch B=8, one batch element per core):
  encoder matmul -> packed complex z (even/odd samples) -> four-step
  FFT_8192 (stage A over j1 via reverse-matmul, twiddle, stage B over j2)
  -> fused pointwise  Zv[k] = A[k]*Z[k] + B[k]*conj(Z[8192-k])  where the
  host-precomputed A/B fields absorb the rfft unpack, the S4D kernel
  transfer function (incl. the D skip term), and the repack -> mirrored
  inverse four-step -> gelu -> GLU projection -> mean-pool partial sums.

The S4D kernel construction + its rFFT + the A/B fields are tiny
parameter-only computations done on host (numpy).  All O(B*H*L) work runs
on the NeuronCores in one NEFF.

Shapes hardcoded: B=8, L=8192, H=128, N=32.
"""
import numpy as np
import ml_dtypes

B, L, H = 8, 8192, 128
M = 8192          # packed complex FFT length
M1, M2 = 128, 64  # j = j1*64 + j2 ; k = k2*128 + k1
G = 8             # g-chunks per group
NG = 8            # number of groups (NG*G = 64 chunks of 128 cols)

_BF = ml_dtypes.bfloat16


# ---------------------------------------------------------------------------
# host-side constants
# ---------------------------------------------------------------------------

def _host_fields(log_dt, log_A_real, A_imag, C_re, C_im, D):
    """S4D kernel K, its 2L rfft, and the packed-pointwise A/B fields."""
    dt = np.exp(log_dt.astype(np.float64))
    A = -np.exp(log_A_real.astype(np.float64)) + 1j * A_imag.astype(np.float64)
    C = C_re.astype(np.float64) + 1j * C_im.astype(np.float64)
    dtA = dt[:, None] * A
    K_coef = C * (np.exp(dtA) - 1.0) / A
    w = np.exp(dtA)
    Tb = 128
    J = L // Tb
    v_lo = w[:, :, None] ** np.arange(Tb)
    v_hi = (w ** Tb)[:, :, None] ** np.arange(J)
    K = 2.0 * np.matmul(K_coef[:, None, :] * v_hi.transpose(0, 2, 1),
                        v_lo).real.reshape(H, L)

    Khat = np.fft.rfft(K, 2 * L, axis=-1)              # (H, 8193)
    Khat = Khat + D.astype(np.float64)[:, None]        # fold skip y += D*u
    k = np.arange(M)
    P = Khat[:, :M]
    idx = (M - k) % (2 * L)
    Q = np.conj(Khat[:, idx])
    Q[:, 0] = Khat[:, M]
    th = 2.0 * np.pi * k / (2 * L)
    Afld = 0.5 * (P + Q) - 0.5 * (P - Q) * np.sin(th)[None, :]
    Bfld = 0.5j * (P - Q) * np.cos(th)[None, :]
    return Afld, Bfld                                   # (H, 8192) complex


def _pack_field(F):
    """(H, 8192) field -> device plane [128=(h',k2), 8192=(g,k1)]."""
    Fg = F.reshape(H, M2, M1)                           # [h, k2, k1]
    P = Fg.reshape(64, 2, M2, M1).transpose(1, 2, 0, 3)  # [h', k2, g, k1]
    return np.ascontiguousarray(P.reshape(128, 8192))


def _dup(mat):
    """[64, X] -> [128, X] duplicated halves (for base-partition 0/64 use)."""
    return np.concatenate([mat, mat], axis=0)


def host_prep(inputs):
    """Returns (shared_map, per_core_maps, dec_w, dec_b)."""
    f32 = np.float32
    x = np.asarray(inputs["x"], f32)
    Afld, Bfld = _host_fields(inputs["log_dt"], inputs["log_A_real"],
                              inputs["A_imag"], inputs["C_re"],
                              inputs["C_im"], inputs["D"])

    j1 = np.arange(64)
    k1 = np.arange(M1)
    j2 = np.arange(M2)
    k2 = np.arange(M2)

    def bf(a):
        return np.ascontiguousarray(a, dtype=np.float32).astype(_BF)

    shared = {}
    shared["enc_lhsT"] = bf(inputs["enc_w"])                      # [2, 128]
    shared["enc_bias"] = np.asarray(inputs["enc_b"], f32).reshape(128, 1)

    th = 2 * np.pi * np.outer(j1, k1) / M1                        # [64, 128]
    shared["d1m_r"] = bf(_dup(np.cos(th)))
    shared["d1m_i"] = bf(_dup(-np.sin(th)))
    shared["d1m_in"] = bf(_dup(np.sin(th)))

    p = np.arange(128) % 64
    th = 2 * np.pi * np.outer(p, k1) / M                          # [128, 128]
    shared["twc"] = bf(np.cos(th))
    shared["tws"] = bf(-np.sin(th))

    th = 2 * np.pi * np.outer(j2, k2) / M2                        # [64, 64]
    shared["d2m_r"] = bf(_dup(np.cos(th)))
    shared["d2m_i"] = bf(_dup(-np.sin(th)))
    shared["d2m_in"] = bf(_dup(np.sin(th)))
    shared["d2m_rn"] = bf(_dup(-np.cos(th)))

    th = 2 * np.pi * np.outer(j2, 63 - k2) / M2                   # [64, 64]
    shared["f1_r"] = bf(_dup(np.cos(th)))
    shared["f1_i"] = bf(_dup(np.sin(th)))
    shared["f1_rn"] = bf(_dup(-np.cos(th)))

    th = 2 * np.pi * np.outer(k2, j2) / M2                        # [64, 64]
    shared["d2i_r"] = bf(_dup(np.cos(th)))
    shared["d2i_i"] = bf(_dup(np.sin(th)))
    shared["d2i_in"] = bf(_dup(-np.sin(th)))

    th = 2 * np.pi * np.outer(np.arange(M1), j2) / M              # [128, 64]
    shared["twic"] = bf(np.cos(th))
    shared["twis"] = bf(np.sin(th))

    th = 2 * np.pi * np.outer(np.arange(M1), j1) / M1             # [128, 64]
    shared["d1i_r"] = bf(np.cos(th) / M)
    shared["d1i_i"] = bf(np.sin(th) / M)
    shared["d1i_in"] = bf(-np.sin(th) / M)

    shared["glu_lhsT"] = bf(np.asarray(inputs["out_w"], f32).T)   # [128, 256]
    ob = np.asarray(inputs["out_b"], f32)
    shared["glu_ba"] = ob[:128].reshape(128, 1).astype(f32)
    shared["glu_bg"] = ob[128:].reshape(128, 1).astype(f32)
    shared["ones_c"] = np.ones((128, 1), f32)
    shared["half_c"] = np.full((128, 1), 0.5, f32)

    shared["fields"] = np.concatenate(
        [bf(_pack_field(p)) for p in (Afld.real, Afld.imag,
                                      Bfld.real, Bfld.imag)], axis=1)

    bf_names = ["enc_lhsT", "d1m_r", "d1m_i", "d1m_in", "twc", "tws",
                "d2m_r", "d2m_i", "d2m_in", "d2m_rn", "f1_r", "f1_i", "f1_rn",
                "d2i_r", "d2i_i", "d2i_in", "twic", "twis",
                "d1i_r", "d1i_i", "d1i_in", "glu_lhsT"]
    blocks = []
    for nm in bf_names:
        a = shared.pop(nm)
        if a.shape[0] != 128:
            pad = np.zeros((128 - a.shape[0], a.shape[1]), a.dtype)
            a = np.concatenate([a, pad], axis=0)
        blocks.append(a)
    shared["cpack"] = np.concatenate(blocks, axis=1)
    f32_names = ["enc_bias", "glu_ba", "glu_bg", "ones_c", "half_c"]
    shared["fpack"] = np.concatenate([shared.pop(nm) for nm in f32_names],
                                     axis=1).astype(f32)

    per_core = []
    for b in range(B):
        xb = x[b]                                                 # (8192, 2)
        per_core.append({
            "xe": bf(xb[0::2, :].T),                              # [2, 4096]
            "xo": bf(xb[1::2, :].T),                              # [2, 4096]
        })
    return shared, per_core


# ---------------------------------------------------------------------------
# device program
# ---------------------------------------------------------------------------

_SHARED_SPECS = [
    ("enc_lhsT", (2, 128), "bf"), ("enc_bias", (128, 1), "f32"),
    ("d1m_r", (128, 128), "bf"), ("d1m_i", (128, 128), "bf"),
    ("d1m_in", (128, 128), "bf"),
    ("twc", (128, 128), "bf"), ("tws", (128, 128), "bf"),
    ("d2m_r", (128, 64), "bf"), ("d2m_i", (128, 64), "bf"),
    ("d2m_in", (128, 64), "bf"), ("d2m_rn", (128, 64), "bf"),
    ("f1_r", (128, 64), "bf"), ("f1_i", (128, 64), "bf"),
    ("f1_rn", (128, 64), "bf"),
    ("d2i_r", (128, 64), "bf"), ("d2i_i", (128, 64), "bf"),
    ("d2i_in", (128, 64), "bf"),
    ("twic", (128, 64), "bf"), ("twis", (128, 64), "bf"),
    ("d1i_r", (128, 64), "bf"), ("d1i_i", (128, 64), "bf"),
    ("d1i_in", (128, 64), "bf"),
    ("glu_lhsT", (128, 256), "bf"), ("glu_ba", (128, 1), "f32"),
    ("glu_bg", (128, 1), "f32"), ("ones_c", (128, 1), "f32"),
    ("half_c", (128, 1), "f32"),
    ("fields", (128, 4 * 8192), "bf"),
]


def build_program(debug_taps=False):
    """Build + compile the single-core SPMD bass program."""
    import concourse.bass as bass
    import concourse.tile as tile
    from concourse import bacc, mybir

    bf = mybir.dt.bfloat16
    f32 = mybir.dt.float32
    AF = mybir.ActivationFunctionType
    ALU = mybir.AluOpType

    nc = bacc.Bacc("TRN2", target_bir_lowering=False, debug=False,
                   num_devices=B)

    dram = {}
    for name, shape, dt_ in _SHARED_SPECS:
        dram[name] = nc.dram_tensor(name, list(shape),
                                    bf if dt_ == "bf" else f32,
                                    kind="ExternalInput").ap()
    dram["xe"] = nc.dram_tensor("xe", [2, 4096], bf, kind="ExternalInput").ap()
    dram["xo"] = nc.dram_tensor("xo", [2, 4096], bf, kind="ExternalInput").ap()
    pool_out = nc.dram_tensor("pool", [128, 1], f32, kind="ExternalOutput").ap()
    taps = {}
    if debug_taps:
        for nm in ("t_ct_r", "t_ct_i", "t_z_r", "t_z_i", "t_zc_r", "t_zc_i",
                   "t_zv_r", "t_zv_i", "t_c3_r", "t_c3_i"):
            taps[nm] = nc.dram_tensor(nm, [128, 8192], bf,
                                      kind="ExternalOutput").ap()
        for nm in ("t_ye", "t_yo"):
            taps[nm] = nc.dram_tensor(nm, [128, 4096], bf,
                                      kind="ExternalOutput").ap()

    with tile.TileContext(nc) as tc:
        from contextlib import ExitStack
        _stack = ExitStack()
        cpool = _stack.enter_context(tc.tile_pool(name="consts", bufs=1))
        C = {}
        for name, shape, dt_ in _SHARED_SPECS:
            if name == "fields":
                continue
            ct = cpool.tile(list(shape), bf if dt_ == "bf" else f32,
                            name="c_" + name, tag=name)
            nc.sync.dma_start(ct[:], dram[name][:])
            C[name] = ct[:]

        persist = _stack.enter_context(tc.tile_pool(name="persist", bufs=1))
        t1 = persist.tile([128, 8192], bf, tag="t1")
        yE = persist.tile([128, 4096], bf, tag="yE")
        yO = persist.tile([128, 4096], bf, tag="yO")
        pool_cols = persist.tile([128, 16], f32, tag="pool_cols")



        work = _stack.enter_context(tc.tile_pool(name="work", bufs=2))
        pw = _stack.enter_context(tc.tile_pool(name="pw", bufs=2))
        pp = _stack.enter_context(tc.tile_pool(name="pp", bufs=1, space="PSUM"))

        # ---- encoder -> DRAM bounce -> T1[j1, (h,j2)] ------------------
        from concourse.tile import add_dep_helper
        dz = {0: nc.dram_tensor("dz_r", [128, 4096], bf, kind="Internal").ap(),
              1: nc.dram_tensor("dz_i", [128, 4096], bf, kind="Internal").ap()}
        for pi, (dst, src) in enumerate(((t1[0:64, :], dram["xe"]),
                                         (t1[64:128, :], dram["xo"]))):
            scat = []
            for c in range(8):
                xch = work.tile([2, 512], bf, tag="xch")
                nc.sync.dma_start(xch[:], src[:, c * 512:(c + 1) * 512])
                pse2 = pp.tile([128, 512], f32, name="pse2", tag="pbig", bufs=3)
                nc.tensor.matmul(pse2[:], C["enc_lhsT"][:], xch[:],
                                 start=True, stop=True)
                zch = work.tile([128, 512], bf, tag="zch")
                nc.scalar.activation(zch[:], pse2[:], AF.Identity,
                                     bias=C["enc_bias"], scale=1.0)
                scat.append(nc.sync.dma_start(
                    dz[pi][:, c * 512:(c + 1) * 512], zch[:]))
            gat = nc.sync.dma_start(
                dst.rearrange("a (h b) -> a h b", h=128),
                dz[pi][:].rearrange("h (a b) -> h a b", a=64).transpose([1, 0, 2]))
            for s in scat:
                add_dep_helper(gat.ins, s.ins, reason="t1 gather after scatter")

        # ---- main groups ----------------------------------------------
        dv_e = nc.dram_tensor("dv_e", [NG, 64, 1024], bf, kind="Internal").ap()
        dv_o = nc.dram_tensor("dv_o", [NG, 64, 1024], bf, kind="Internal").ap()
        fin_scat = []
        for gg in range(NG):
            g0 = gg * G
            # A-rev: Ct[(h',j2), (g,k1)] chunks
            ctr = work.tile([128, 1024], bf, tag="ctr", bufs=3)
            cti = work.tile([128, 1024], bf, tag="cti", bufs=3)
            for gl in range(G):
                g = g0 + gl
                lr = t1[0:64, g * 128:(g + 1) * 128]
                li = t1[64:128, g * 128:(g + 1) * 128]
                ps_re = pp.tile([128, 128], f32, name="psA_re", tag="pa", bufs=2)
                nc.tensor.matmul(ps_re[:], lr, C["d1m_r"][0:64, :], start=True, stop=False)
                nc.tensor.matmul(ps_re[:], li, C["d1m_in"][64:128, :], start=False, stop=True)
                nc.scalar.copy(ctr[:, gl * 128:(gl + 1) * 128], ps_re[:])
                ps_im = pp.tile([128, 128], f32, name="psA_im", tag="pa", bufs=2)
                nc.tensor.matmul(ps_im[:], lr, C["d1m_i"][0:64, :], start=True, stop=False)
                nc.tensor.matmul(ps_im[:], li, C["d1m_r"][64:128, :], start=False, stop=True)
                nc.vector.tensor_copy(cti[:, gl * 128:(gl + 1) * 128], ps_im[:])

            # forward twiddle (broadcast [128,128] tile over g)
            twc_b = C["twc"][:].unsqueeze(1).broadcast_to((128, G, 128))
            tws_b = C["tws"][:].unsqueeze(1).broadcast_to((128, G, 128))
            cttr = work.tile([128, 1024], bf, tag="cttr", bufs=3)
            ctti = work.tile([128, 1024], bf, tag="ctti", bufs=3)
            v3 = lambda t: t[:].rearrange("p (a b) -> p a b", a=G)
            tA = work.tile([128, 1024], bf, name="tA", tag="s1")
            tB = work.tile([128, 1024], bf, name="tB", tag="s2")
            nc.vector.tensor_mul(v3(tA), v3(ctr), twc_b)
            nc.gpsimd.tensor_mul(v3(tB), v3(cti), tws_b)
            nc.vector.tensor_sub(cttr[:], tA[:], tB[:])
            nc.gpsimd.tensor_mul(v3(tA), v3(ctr), tws_b)
            nc.vector.tensor_mul(v3(tB), v3(cti), twc_b)
            nc.vector.tensor_add(ctti[:], tA[:], tB[:])
            if debug_taps:
                nc.sync.dma_start(taps["t_ct_r"][:, g0*128:(g0+G)*128], cttr[:])
                nc.sync.dma_start(taps["t_ct_i"][:, g0*128:(g0+G)*128], ctti[:])

            # stage B -> Z ; Zc via F1/k0 path
            zr = work.tile([128, 1024], bf, tag="zr", bufs=3)
            zi = work.tile([128, 1024], bf, tag="zi", bufs=3)
            for (dst, m1a, m1b) in ((zr, "d2m_r", "d2m_in"),
                                    (zi, "d2m_i", "d2m_r")):
                for c in range(2):
                    ps = pp.tile([128, 512], f32, name="psb", tag="pbig", bufs=3)
                    cols = slice(c * 512, (c + 1) * 512)
                    for h_ in range(2):
                        rows = slice(h_ * 64, (h_ + 1) * 64)
                        nc.tensor.matmul(ps[rows, :], C[m1a][rows, :],
                                         cttr[rows, cols], start=True, stop=False)
                        nc.tensor.matmul(ps[rows, :], C[m1b][rows, :],
                                         ctti[rows, cols], start=False, stop=True)
                    nc.scalar.copy(dst[:, cols], ps[:])

            zcr = work.tile([128, 1024], bf, tag="zcr", bufs=3)
            zci = work.tile([128, 1024], bf, tag="zci", bufs=3)
            # main part k1 in [1,128): rhs cols reversed within each g block
            for (dst, ma, mb) in ((zcr, "f1_r", "f1_i"), (zci, "f1_i", "f1_rn")):
                for c in range(2):   # 4 g per chunk
                    psz = pp.tile([128, 512], f32, name="psc", tag="pbig", bufs=3)
                    ps = psz[:, 0:508]
                    for h_ in range(2):
                        rows = slice(h_ * 64, (h_ + 1) * 64)
                        rev_r = cttr[rows, :].rearrange(
                            "p (a b) -> p a b", a=G)[:, c * 4:(c + 1) * 4, 127:0:-1]
                        rev_i = ctti[rows, :].rearrange(
                            "p (a b) -> p a b", a=G)[:, c * 4:(c + 1) * 4, 127:0:-1]
                        nc.tensor.matmul(ps[rows, :].rearrange(
                            "p (a b) -> p a b", a=4), C[ma][rows, :], rev_r,
                            start=True, stop=False)
                        nc.tensor.matmul(ps[rows, :].rearrange(
                            "p (a b) -> p a b", a=4), C[mb][rows, :], rev_i,
                            start=False, stop=True)
                    nc.scalar.copy(
                        dst[:].rearrange("p (a b) -> p a b", a=G)
                           [:, c * 4:(c + 1) * 4, 1:128],
                        ps.rearrange("p (c b) -> p c b", c=4))
            # k1 = 0 columns
            for (dst, ma, mb) in ((zcr, "d2m_r", "d2m_i"), (zci, "d2m_i", "d2m_rn")):
                psk = pp.tile([128, 128], f32, name="psk", tag="pa", bufs=2)
                ps = psk[:, 0:8]
                r0 = cttr[:].rearrange("p (a b) -> p a b", a=G)[:, :, 0:1]
                i0 = ctti[:].rearrange("p (a b) -> p a b", a=G)[:, :, 0:1]
                for h_ in range(2):
                    rows = slice(h_ * 64, (h_ + 1) * 64)
                    nc.tensor.matmul(ps[rows, :].rearrange("p (a b) -> p a b", a=G),
                                     C[ma][rows, :], r0[h_ * 64:(h_ + 1) * 64],
                                     start=True, stop=False)
                    nc.tensor.matmul(ps[rows, :].rearrange("p (a b) -> p a b", a=G),
                                     C[mb][rows, :], i0[h_ * 64:(h_ + 1) * 64],
                                     start=False, stop=True)
                nc.vector.tensor_copy(
                    dst[:].rearrange("p (a b) -> p a b", a=G)[:, :, 0:1],
                    ps.rearrange("p (a b) -> p a b", a=G))

            if debug_taps:
                for tp, t in (("t_z_r", zr), ("t_z_i", zi),
                              ("t_zc_r", zcr), ("t_zc_i", zci)):
                    nc.sync.dma_start(taps[tp][:, g0*128:(g0+G)*128], t[:])

            # pointwise: Zv = A*Z + B*Zc
            ab = pw.tile([128, 4, 1024], bf, tag="ab")
            cols = slice(g0 * 128, (g0 + G) * 128)
            nc.sync.dma_start(
                ab[:],
                dram["fields"][:].rearrange("p (f c) -> p f c", f=4)[:, :, cols])
            ar, ai, br, bi = ab[:, 0], ab[:, 1], ab[:, 2], ab[:, 3]
            zvr = work.tile([128, 1024], bf, tag="zvr", bufs=3)
            zvi = work.tile([128, 1024], bf, tag="zvi", bufs=3)
            p1 = work.tile([128, 1024], bf, name="p1", tag="s1")
            p2 = work.tile([128, 1024], bf, name="p2", tag="s2")
            p3 = work.tile([128, 1024], bf, name="p3", tag="s3")
            p4 = work.tile([128, 1024], bf, name="p4", tag="s4")
            nc.vector.tensor_mul(p1[:], zr[:], ar)
            nc.gpsimd.tensor_mul(p2[:], zi[:], ai)
            nc.gpsimd.tensor_mul(p3[:], zcr[:], br)
            nc.vector.tensor_mul(p4[:], zci[:], bi)
            nc.vector.tensor_sub(p1[:], p1[:], p2[:])
            nc.vector.tensor_sub(p3[:], p3[:], p4[:])
            nc.vector.tensor_add(zvr[:], p1[:], p3[:])
            nc.vector.tensor_mul(p1[:], zi[:], ar)
            nc.vector.tensor_mul(p2[:], zr[:], ai)
            nc.vector.tensor_mul(p3[:], zci[:], br)
            nc.vector.tensor_mul(p4[:], zcr[:], bi)
            nc.vector.tensor_add(p1[:], p1[:], p2[:])
            nc.vector.tensor_add(p3[:], p3[:], p4[:])
            nc.vector.tensor_add(zvi[:], p1[:], p3[:])
            if debug_taps:
                nc.sync.dma_start(taps["t_zv_r"][:, cols], zvr[:])
                nc.sync.dma_start(taps["t_zv_i"][:, cols], zvi[:])

            # B'-rev: C3[k1, (h,j2)] per (g, h')
            c3r = work.tile([128, 1024], bf, tag="c3r", bufs=3)
            c3i = work.tile([128, 1024], bf, tag="c3i", bufs=3)
            for gl in range(G):
                for h_ in range(2):
                    rows = slice(h_ * 64, (h_ + 1) * 64)
                    lr = zvr[rows, gl * 128:(gl + 1) * 128]
                    li = zvi[rows, gl * 128:(gl + 1) * 128]
                    oc = (2 * gl + h_) * 64
                    ps_re = pp.tile([128, 64], f32, name="psD_re", tag="pd", bufs=3)
                    nc.tensor.matmul(ps_re[:], lr, C["d2i_r"][rows, :], start=True, stop=False)
                    nc.tensor.matmul(ps_re[:], li, C["d2i_in"][rows, :], start=False, stop=True)
                    nc.scalar.copy(c3r[:, oc:oc + 64], ps_re[:])
                    ps_im = pp.tile([128, 64], f32, name="psD_im", tag="pd", bufs=3)
                    nc.tensor.matmul(ps_im[:], lr, C["d2i_i"][rows, :], start=True, stop=False)
                    nc.tensor.matmul(ps_im[:], li, C["d2i_r"][rows, :], start=False, stop=True)
                    nc.vector.tensor_copy(c3i[:, oc:oc + 64], ps_im[:])
            if debug_taps:
                nc.sync.dma_start(taps["t_c3_r"][:, cols], c3r[:])
                nc.sync.dma_start(taps["t_c3_i"][:, cols], c3i[:])

            # inverse twiddle (broadcast [128,64] over h=16)
            twic_b = C["twic"][:].unsqueeze(1).broadcast_to((128, 16, 64))
            twis_b = C["twis"][:].unsqueeze(1).broadcast_to((128, 16, 64))
            v3h = lambda t: t[:].rearrange("p (a b) -> p a b", a=16)
            c3tr = work.tile([128, 1024], bf, tag="c3tr", bufs=2)
            c3ti = work.tile([128, 1024], bf, tag="c3ti", bufs=2)
            tC = work.tile([128, 1024], bf, name="tC", tag="s1")
            tD = work.tile([128, 1024], bf, name="tD", tag="s2")
            nc.vector.tensor_mul(v3h(tC), v3h(c3r), twic_b)
            nc.gpsimd.tensor_mul(v3h(tD), v3h(c3i), twis_b)
            nc.vector.tensor_sub(c3tr[:], tC[:], tD[:])
            nc.gpsimd.tensor_mul(v3h(tC), v3h(c3r), twis_b)
            nc.vector.tensor_mul(v3h(tD), v3h(c3i), twic_b)
            nc.vector.tensor_add(c3ti[:], tC[:], tD[:])

            # stage A' -> vE, vO [j1<64, (h, j2)]
            ve = work.tile([64, 1024], bf, tag="ve")
            vo = work.tile([64, 1024], bf, tag="vo")
            ve_acts, vo_acts = [], []
            for (dst, acts, ma, mb) in ((ve, ve_acts, "d1i_r", "d1i_in"),
                                        (vo, vo_acts, "d1i_i", "d1i_r")):
                for c in range(2):
                    cols2 = slice(c * 512, (c + 1) * 512)
                    ps = pp.tile([64, 512], f32, name="pse", tag="pbig", bufs=3)
                    nc.tensor.matmul(ps[:], C[ma][:], c3tr[:, cols2], start=True, stop=False)
                    nc.tensor.matmul(ps[:], C[mb][:], c3ti[:, cols2], start=False, stop=True)
                    acts.append(nc.scalar.copy(dst[:, cols2], ps[:]))

            # scatter into DRAM bounce then gather this group's 16 y-rows
            for dvt, dst, (srct, acts) in ((dv_e, yE, (ve, ve_acts)),
                                           (dv_o, yO, (vo, vo_acts))):
                dma = nc.sync.dma_start(dvt[gg], srct[:])
                for a in acts:
                    add_dep_helper(dma.ins, a.ins, reason="scatter after A' evac")
                gat = nc.sync.dma_start(
                    yE[gg * 16:(gg + 1) * 16, :].rearrange("h (j b) -> h j b", j=64)
                    if dst is yE else
                    yO[gg * 16:(gg + 1) * 16, :].rearrange("h (j b) -> h j b", j=64),
                    dvt[gg].rearrange("j (hl b) -> hl j b", hl=16))
                add_dep_helper(gat.ins, dma.ins, reason="y gather after scatter")

        if debug_taps:
            nc.sync.dma_start(taps["t_ye"][:], yE[:])
            nc.sync.dma_start(taps["t_yo"][:], yO[:])

        # ---- gelu + GLU + pool ----------------------------------------
        CG = 0.7978845608028654
        gtiles = {}
        for i, pl in enumerate((yE, yO)):
            gtiles[i] = pw.tile([128, 4096], bf, name=f"gel{i}", tag="ab")
        for i, pl in enumerate((yE, yO)):   # sq = x*x
            nc.vector.tensor_mul(gtiles[i][:], pl[:], pl[:])
        for i, pl in enumerate((yE, yO)):   # rr = 0.044715*sq + 1
            nc.scalar.activation(gtiles[i][:], gtiles[i][:], AF.Identity,
                                 bias=C["ones_c"], scale=0.044715)
        for i, pl in enumerate((yE, yO)):   # qq = x*rr
            nc.vector.tensor_mul(gtiles[i][:], pl[:], gtiles[i][:])
        for i, pl in enumerate((yE, yO)):   # tt = tanh(CG*qq)
            nc.scalar.activation(gtiles[i][:], gtiles[i][:], AF.Tanh, scale=CG)
        for i, pl in enumerate((yE, yO)):   # uu = 0.5*tt + 0.5
            nc.scalar.activation(gtiles[i][:], gtiles[i][:], AF.Identity,
                                 bias=C["half_c"], scale=0.5)
        for i, pl in enumerate((yE, yO)):   # y = x*uu
            nc.vector.tensor_mul(pl[:], pl[:], gtiles[i][:])

        scratch = work.tile([128, 512], bf, tag="glu_scratch")
        idx = 0
        for plane in (yE, yO):
            for c in range(8):
                cols = slice(c * 512, (c + 1) * 512)
                ps_a = pp.tile([128, 512], f32, name="ps_a", tag="pbig", bufs=3)
                ps_g = pp.tile([128, 512], f32, name="ps_g", tag="pbig", bufs=3)
                nc.tensor.matmul(ps_a, C["glu_lhsT"][:, 0:128], plane[:, cols],
                                 start=True, stop=True)
                nc.tensor.matmul(ps_g, C["glu_lhsT"][:, 128:256], plane[:, cols],
                                 start=True, stop=True)
                sig = work.tile([128, 512], bf, tag="glu_sig")
                nc.scalar.activation(sig[:], ps_g, AF.Sigmoid,
                                     bias=C["glu_bg"], scale=1.0)
                nc.vector.scalar_tensor_tensor(
                    scratch[:], ps_a, C["glu_ba"], sig[:],
                    op0=ALU.add, op1=ALU.mult,
                    accum_out=pool_cols[:, idx:idx + 1])
                idx += 1

        pool_t = work.tile([128, 1], f32, tag="pool_t")
        nc.vector.tensor_reduce(pool_t[:], pool_cols[:],
                                axis=mybir.AxisListType.X, op=ALU.add)
        nc.sync.dma_start(pool_out[:], pool_t[:])

        _stack.close()

    nc.compile()
    return nc


_CACHED_NC = None


def kernel(**inputs):
    global _CACHED_NC
    from concourse.bass_utils import run_bass_kernel_spmd

    shared, per_core = host_prep(inputs)
    if _CACHED_NC is None:
        _CACHED_NC = build_program()
    nc = _CACHED_NC

    in_maps = [{**shared, **pc} for pc in per_core]
    res = run_bass_kernel_spmd(nc, in_maps, list(range(B)))
    pool = np.stack([np.asarray(res.results[b]["pool"][:, 0], np.float64)
                     for b in range(B)])                     # (8, 128)
    pooled = pool / float(L)
    dec_w = np.asarray(inputs["dec_w"], np.float64)
    dec_b = np.asarray(inputs["dec_b"], np.float64)
    return (pooled @ dec_w + dec_b).astype(np.float32)


if __name__ == "__main__":
    ins = {
        "x": np.random.randn(B, L, 2).astype(np.float32),
        "enc_w": np.random.randn(2, H).astype(np.float32),
        "enc_b": np.random.randn(H).astype(np.float32),
        "log_dt": np.random.rand(H).astype(np.float32),
        "log_A_real": np.random.randn(H, 32).astype(np.float32),
        "A_imag": np.random.randn(H, 32).astype(np.float32),
        "C_re": np.random.randn(H, 32).astype(np.float32),
        "C_im": np.random.randn(H, 32).astype(np.float32),
        "D": np.random.randn(H).astype(np.float32),
        "out_w": np.random.randn(2 * H, H).astype(np.float32),
        "out_b": np.random.randn(2 * H).astype(np.float32),
        "dec_w": np.random.randn(H, 1).astype(np.float32),
        "dec_b": np.random.randn(1).astype(np.float32),
    }
    print(kernel(**ins).shape)


# revision 47
# speedup vs baseline: 2.2503x; 1.0007x over previous
"""S4D AddingModel — Bass/Tile kernel for 8 Trainium2 NeuronCores.

Strategy (data-parallel over batch B=8, one batch element per core):
  encoder matmul -> packed complex z (even/odd samples) -> four-step
  FFT_8192 (stage A over j1 via reverse-matmul, twiddle, stage B over j2)
  -> fused pointwise  Zv[k] = A[k]*Z[k] + B[k]*conj(Z[8192-k])  where the
  host-precomputed A/B fields absorb the rfft unpack, the S4D kernel
  transfer function (incl. the D skip term), and the repack -> mirrored
  inverse four-step -> gelu -> GLU projection -> mean-pool partial sums.

The S4D kernel construction + its rFFT + the A/B fields are tiny
parameter-only computations done on host (numpy).  All O(B*H*L) work runs
on the NeuronCores in one NEFF.

Shapes hardcoded: B=8, L=8192, H=128, N=32.
"""
import numpy as np
import ml_dtypes

B, L, H = 8, 8192, 128
M = 8192          # packed complex FFT length
M1, M2 = 128, 64  # j = j1*64 + j2 ; k = k2*128 + k1
G = 8             # g-chunks per group
NG = 8            # number of groups (NG*G = 64 chunks of 128 cols)

_BF = ml_dtypes.bfloat16


# ---------------------------------------------------------------------------
# host-side constants
# ---------------------------------------------------------------------------

def _host_fields(log_dt, log_A_real, A_imag, C_re, C_im, D):
    """S4D kernel K, its 2L rfft, and the packed-pointwise A/B fields."""
    dt = np.exp(log_dt.astype(np.float64))
    A = -np.exp(log_A_real.astype(np.float64)) + 1j * A_imag.astype(np.float64)
    C = C_re.astype(np.float64) + 1j * C_im.astype(np.float64)
    dtA = dt[:, None] * A
    K_coef = C * (np.exp(dtA) - 1.0) / A
    w = np.exp(dtA)
    Tb = 128
    J = L // Tb
    v_lo = w[:, :, None] ** np.arange(Tb)
    v_hi = (w ** Tb)[:, :, None] ** np.arange(J)
    K = 2.0 * np.matmul(K_coef[:, None, :] * v_hi.transpose(0, 2, 1),
                        v_lo).real.reshape(H, L)

    Khat = np.fft.rfft(K, 2 * L, axis=-1)              # (H, 8193)
    Khat = Khat + D.astype(np.float64)[:, None]        # fold skip y += D*u
    k = np.arange(M)
    P = Khat[:, :M]
    idx = (M - k) % (2 * L)
    Q = np.conj(Khat[:, idx])
    Q[:, 0] = Khat[:, M]
    th = 2.0 * np.pi * k / (2 * L)
    Afld = 0.5 * (P + Q) - 0.5 * (P - Q) * np.sin(th)[None, :]
    Bfld = 0.5j * (P - Q) * np.cos(th)[None, :]
    return Afld, Bfld                                   # (H, 8192) complex


def _pack_field(F):
    """(H, 8192) field -> device plane [128=(h',k2), 8192=(g,k1)]."""
    Fg = F.reshape(H, M2, M1)                           # [h, k2, k1]
    P = Fg.reshape(64, 2, M2, M1).transpose(1, 2, 0, 3)  # [h', k2, g, k1]
    return np.ascontiguousarray(P.reshape(128, 8192))


def _dup(mat):
    """[64, X] -> [128, X] duplicated halves (for base-partition 0/64 use)."""
    return np.concatenate([mat, mat], axis=0)


def host_prep(inputs):
    """Returns (shared_map, per_core_maps, dec_w, dec_b)."""
    f32 = np.float32
    x = np.asarray(inputs["x"], f32)
    Afld, Bfld = _host_fields(inputs["log_dt"], inputs["log_A_real"],
                              inputs["A_imag"], inputs["C_re"],
                              inputs["C_im"], inputs["D"])

    j1 = np.arange(64)
    k1 = np.arange(M1)
    j2 = np.arange(M2)
    k2 = np.arange(M2)

    def bf(a):
        return np.ascontiguousarray(a, dtype=np.float32).astype(_BF)

    shared = {}
    shared["enc_lhsT"] = bf(inputs["enc_w"])                      # [2, 128]
    shared["enc_bias"] = np.asarray(inputs["enc_b"], f32).reshape(128, 1)

    th = 2 * np.pi * np.outer(j1, k1) / M1                        # [64, 128]
    shared["d1m_r"] = bf(_dup(np.cos(th)))
    shared["d1m_i"] = bf(_dup(-np.sin(th)))
    shared["d1m_in"] = bf(_dup(np.sin(th)))

    p = np.arange(128) % 64
    th = 2 * np.pi * np.outer(p, k1) / M                          # [128, 128]
    shared["twc"] = bf(np.cos(th))
    shared["tws"] = bf(-np.sin(th))

    th = 2 * np.pi * np.outer(j2, k2) / M2                        # [64, 64]
    shared["d2m_r"] = bf(_dup(np.cos(th)))
    shared["d2m_i"] = bf(_dup(-np.sin(th)))
    shared["d2m_in"] = bf(_dup(np.sin(th)))
    shared["d2m_rn"] = bf(_dup(-np.cos(th)))

    th = 2 * np.pi * np.outer(j2, 63 - k2) / M2                   # [64, 64]
    shared["f1_r"] = bf(_dup(np.cos(th)))
    shared["f1_i"] = bf(_dup(np.sin(th)))
    shared["f1_rn"] = bf(_dup(-np.cos(th)))

    th = 2 * np.pi * np.outer(k2, j2) / M2                        # [64, 64]
    shared["d2i_r"] = bf(_dup(np.cos(th)))
    shared["d2i_i"] = bf(_dup(np.sin(th)))
    shared["d2i_in"] = bf(_dup(-np.sin(th)))

    th = 2 * np.pi * np.outer(np.arange(M1), j2) / M              # [128, 64]
    shared["twic"] = bf(np.cos(th))
    shared["twis"] = bf(np.sin(th))

    th = 2 * np.pi * np.outer(np.arange(M1), j1) / M1             # [128, 64]
    shared["d1i_r"] = bf(np.cos(th) / M)
    shared["d1i_i"] = bf(np.sin(th) / M)
    shared["d1i_in"] = bf(-np.sin(th) / M)

    shared["glu_lhsT"] = bf(np.asarray(inputs["out_w"], f32).T)   # [128, 256]
    ob = np.asarray(inputs["out_b"], f32)
    shared["glu_ba"] = ob[:128].reshape(128, 1).astype(f32)
    shared["glu_bg"] = ob[128:].reshape(128, 1).astype(f32)
    shared["ones_c"] = np.ones((128, 1), f32)
    shared["half_c"] = np.full((128, 1), 0.5, f32)

    shared["fields"] = np.concatenate(
        [bf(_pack_field(p)) for p in (Afld.real, Afld.imag,
                                      Bfld.real, Bfld.imag)], axis=1)

    bf_names = ["enc_lhsT", "d1m_r", "d1m_i", "d1m_in", "twc", "tws",
                "d2m_r", "d2m_i", "d2m_in", "d2m_rn", "f1_r", "f1_i", "f1_rn",
                "d2i_r", "d2i_i", "d2i_in", "twic", "twis",
                "d1i_r", "d1i_i", "d1i_in", "glu_lhsT"]
    blocks = []
    for nm in bf_names:
        a = shared.pop(nm)
        if a.shape[0] != 128:
            pad = np.zeros((128 - a.shape[0], a.shape[1]), a.dtype)
            a = np.concatenate([a, pad], axis=0)
        blocks.append(a)
    shared["cpack"] = np.concatenate(blocks, axis=1)
    f32_names = ["enc_bias", "glu_ba", "glu_bg", "ones_c", "half_c"]
    shared["fpack"] = np.concatenate([shared.pop(nm) for nm in f32_names],
                                     axis=1).astype(f32)

    per_core = []
    for b in range(B):
        xb = x[b]                                                 # (8192, 2)
        per_core.append({
            "xe": bf(xb[0::2, :].T),                              # [2, 4096]
            "xo": bf(xb[1::2, :].T),                              # [2, 4096]
        })
    return shared, per_core


# ---------------------------------------------------------------------------
# device program
# ---------------------------------------------------------------------------

_SHARED_SPECS = [
    ("enc_lhsT", (2, 128), "bf"), ("enc_bias", (128, 1), "f32"),
    ("d1m_r", (128, 128), "bf"), ("d1m_i", (128, 128), "bf"),
    ("d1m_in", (128, 128), "bf"),
    ("twc", (128, 128), "bf"), ("tws", (128, 128), "bf"),
    ("d2m_r", (128, 64), "bf"), ("d2m_i", (128, 64), "bf"),
    ("d2m_in", (128, 64), "bf"), ("d2m_rn", (128, 64), "bf"),
    ("f1_r", (128, 64), "bf"), ("f1_i", (128, 64), "bf"),
    ("f1_rn", (128, 64), "bf"),
    ("d2i_r", (128, 64), "bf"), ("d2i_i", (128, 64), "bf"),
    ("d2i_in", (128, 64), "bf"),
    ("twic", (128, 64), "bf"), ("twis", (128, 64), "bf"),
    ("d1i_r", (128, 64), "bf"), ("d1i_i", (128, 64), "bf"),
    ("d1i_in", (128, 64), "bf"),
    ("glu_lhsT", (128, 256), "bf"), ("glu_ba", (128, 1), "f32"),
    ("glu_bg", (128, 1), "f32"), ("ones_c", (128, 1), "f32"),
    ("half_c", (128, 1), "f32"),
    ("fields", (128, 4 * 8192), "bf"),
]


def build_program(debug_taps=False):
    """Build + compile the single-core SPMD bass program."""
    import concourse.bass as bass
    import concourse.tile as tile
    from concourse import bacc, mybir

    bf = mybir.dt.bfloat16
    f32 = mybir.dt.float32
    AF = mybir.ActivationFunctionType
    ALU = mybir.AluOpType

    nc = bacc.Bacc("TRN2", target_bir_lowering=False, debug=False,
                   num_devices=B)

    dram = {}
    for name, shape, dt_ in _SHARED_SPECS:
        dram[name] = nc.dram_tensor(name, list(shape),
                                    bf if dt_ == "bf" else f32,
                                    kind="ExternalInput").ap()
    dram["xe"] = nc.dram_tensor("xe", [2, 4096], bf, kind="ExternalInput").ap()
    dram["xo"] = nc.dram_tensor("xo", [2, 4096], bf, kind="ExternalInput").ap()
    pool_out = nc.dram_tensor("pool", [128, 1], f32, kind="ExternalOutput").ap()
    taps = {}
    if debug_taps:
        for nm in ("t_ct_r", "t_ct_i", "t_z_r", "t_z_i", "t_zc_r", "t_zc_i",
                   "t_zv_r", "t_zv_i", "t_c3_r", "t_c3_i"):
            taps[nm] = nc.dram_tensor(nm, [128, 8192], bf,
                                      kind="ExternalOutput").ap()
        for nm in ("t_ye", "t_yo"):
            taps[nm] = nc.dram_tensor(nm, [128, 4096], bf,
                                      kind="ExternalOutput").ap()

    with tile.TileContext(nc) as tc:
        from contextlib import ExitStack
        _stack = ExitStack()
        cpool = _stack.enter_context(tc.tile_pool(name="consts", bufs=1))
        C = {}
        for name, shape, dt_ in _SHARED_SPECS:
            if name == "fields":
                continue
            ct = cpool.tile(list(shape), bf if dt_ == "bf" else f32,
                            name="c_" + name, tag=name)
            nc.sync.dma_start(ct[:], dram[name][:])
            C[name] = ct[:]

        persist = _stack.enter_context(tc.tile_pool(name="persist", bufs=1))
        t1 = persist.tile([128, 8192], bf, tag="t1")
        yE = persist.tile([128, 4096], bf, tag="yE")
        yO = persist.tile([128, 4096], bf, tag="yO")
        pool_cols = persist.tile([128, 16], f32, tag="pool_cols")



        work = _stack.enter_context(tc.tile_pool(name="work", bufs=2))
        pw = _stack.enter_context(tc.tile_pool(name="pw", bufs=2))
        pp = _stack.enter_context(tc.tile_pool(name="pp", bufs=1, space="PSUM"))

        # ---- encoder -> DRAM bounce -> T1[j1, (h,j2)] ------------------
        from concourse.tile import add_dep_helper
        dz = {0: nc.dram_tensor("dz_r", [128, 4096], bf, kind="Internal").ap(),
              1: nc.dram_tensor("dz_i", [128, 4096], bf, kind="Internal").ap()}
        for pi, (dst, src) in enumerate(((t1[0:64, :], dram["xe"]),
                                         (t1[64:128, :], dram["xo"]))):
            scat = []
            for c in range(8):
                xch = work.tile([2, 512], bf, tag="xch")
                nc.sync.dma_start(xch[:], src[:, c * 512:(c + 1) * 512])
                pse2 = pp.tile([128, 512], f32, name="pse2", tag="pbig", bufs=3)
                nc.tensor.matmul(pse2[:], C["enc_lhsT"][:], xch[:],
                                 start=True, stop=True)
                zch = work.tile([128, 512], bf, tag="zch")
                nc.scalar.activation(zch[:], pse2[:], AF.Identity,
                                     bias=C["enc_bias"], scale=1.0)
                scat.append(nc.sync.dma_start(
                    dz[pi][:, c * 512:(c + 1) * 512], zch[:]))
            gat = nc.sync.dma_start(
                dst.rearrange("a (h b) -> a h b", h=128),
                dz[pi][:].rearrange("h (a b) -> h a b", a=64).transpose([1, 0, 2]))
            for s in scat:
                add_dep_helper(gat.ins, s.ins, reason="t1 gather after scatter")

        # ---- main groups ----------------------------------------------
        dv_e = nc.dram_tensor("dv_e", [NG, 64, 1024], bf, kind="Internal").ap()
        dv_o = nc.dram_tensor("dv_o", [NG, 64, 1024], bf, kind="Internal").ap()
        fin_scat = []
        for gg in range(NG):
            g0 = gg * G
            # A-rev: Ct[(h',j2), (g,k1)] chunks
            ctr = work.tile([128, 1024], bf, tag="ctr", bufs=3)
            cti = work.tile([128, 1024], bf, tag="cti", bufs=3)
            for gl in range(G):
                g = g0 + gl
                lr = t1[0:64, g * 128:(g + 1) * 128]
                li = t1[64:128, g * 128:(g + 1) * 128]
                ps_re = pp.tile([128, 128], f32, name="psA_re", tag="pa", bufs=2)
                nc.tensor.matmul(ps_re[:], lr, C["d1m_r"][0:64, :], start=True, stop=False)
                nc.tensor.matmul(ps_re[:], li, C["d1m_in"][64:128, :], start=False, stop=True)
                nc.scalar.copy(ctr[:, gl * 128:(gl + 1) * 128], ps_re[:])
                ps_im = pp.tile([128, 128], f32, name="psA_im", tag="pa", bufs=2)
                nc.tensor.matmul(ps_im[:], lr, C["d1m_i"][0:64, :], start=True, stop=False)
                nc.tensor.matmul(ps_im[:], li, C["d1m_r"][64:128, :], start=False, stop=True)
                nc.vector.tensor_copy(cti[:, gl * 128:(gl + 1) * 128], ps_im[:])

            # forward twiddle (broadcast [128,128] tile over g)
            twc_b = C["twc"][:].unsqueeze(1).broadcast_to((128, G, 128))
            tws_b = C["tws"][:].unsqueeze(1).broadcast_to((128, G, 128))
            cttr = work.tile([128, 1024], bf, tag="cttr", bufs=3)
            ctti = work.tile([128, 1024], bf, tag="ctti", bufs=3)
            v3 = lambda t: t[:].rearrange("p (a b) -> p a b", a=G)
            tA = work.tile([128, 1024], bf, name="tA", tag="s1")
            tB = work.tile([128, 1024], bf, name="tB", tag="s2")
            nc.vector.tensor_mul(v3(tA), v3(ctr), twc_b)
            nc.gpsimd.tensor_mul(v3(tB), v3(cti), tws_b)
            nc.vector.tensor_sub(cttr[:], tA[:], tB[:])
            nc.gpsimd.tensor_mul(v3(tA), v3(ctr), tws_b)
            nc.vector.tensor_mul(v3(tB), v3(cti), twc_b)
            nc.vector.tensor_add(ctti[:], tA[:], tB[:])
            if debug_taps:
                nc.sync.dma_start(taps["t_ct_r"][:, g0*128:(g0+G)*128], cttr[:])
                nc.sync.dma_start(taps["t_ct_i"][:, g0*128:(g0+G)*128], ctti[:])

            # stage B -> Z ; Zc via F1/k0 path
            zr = work.tile([128, 1024], bf, tag="zr", bufs=3)
            zi = work.tile([128, 1024], bf, tag="zi", bufs=3)
            for (dst, m1a, m1b) in ((zr, "d2m_r", "d2m_in"),
                                    (zi, "d2m_i", "d2m_r")):
                for c in range(2):
                    ps = pp.tile([128, 512], f32, name="psb", tag="pbig", bufs=3)
                    cols = slice(c * 512, (c + 1) * 512)
                    for h_ in range(2):
                        rows = slice(h_ * 64, (h_ + 1) * 64)
                        nc.tensor.matmul(ps[rows, :], C[m1a][rows, :],
                                         cttr[rows, cols], start=True, stop=False)
                        nc.tensor.matmul(ps[rows, :], C[m1b][rows, :],
                                         ctti[rows, cols], start=False, stop=True)
                    nc.scalar.copy(dst[:, cols], ps[:])

            zcr = work.tile([128, 1024], bf, tag="zcr", bufs=3)
            zci = work.tile([128, 1024], bf, tag="zci", bufs=3)
            # main part k1 in [1,128): rhs cols reversed within each g block
            for (dst, ma, mb) in ((zcr, "f1_r", "f1_i"), (zci, "f1_i", "f1_rn")):
                for c in range(2):   # 4 g per chunk
                    psz = pp.tile([128, 512], f32, name="psc", tag="pbig", bufs=3)
                    ps = psz[:, 0:508]
                    for h_ in range(2):
                        rows = slice(h_ * 64, (h_ + 1) * 64)
                        rev_r = cttr[rows, :].rearrange(
                            "p (a b) -> p a b", a=G)[:, c * 4:(c + 1) * 4, 127:0:-1]
                        rev_i = ctti[rows, :].rearrange(
                            "p (a b) -> p a b", a=G)[:, c * 4:(c + 1) * 4, 127:0:-1]
                        nc.tensor.matmul(ps[rows, :].rearrange(
                            "p (a b) -> p a b", a=4), C[ma][rows, :], rev_r,
                            start=True, stop=False)
                        nc.tensor.matmul(ps[rows, :].rearrange(
                            "p (a b) -> p a b", a=4), C[mb][rows, :], rev_i,
                            start=False, stop=True)
                    nc.scalar.copy(
                        dst[:].rearrange("p (a b) -> p a b", a=G)
                           [:, c * 4:(c + 1) * 4, 1:128],
                        ps.rearrange("p (c b) -> p c b", c=4))
            # k1 = 0 columns
            for (dst, ma, mb) in ((zcr, "d2m_r", "d2m_i"), (zci, "d2m_i", "d2m_rn")):
                psk = pp.tile([128, 128], f32, name="psk", tag="pa", bufs=2)
                ps = psk[:, 0:8]
                r0 = cttr[:].rearrange("p (a b) -> p a b", a=G)[:, :, 0:1]
                i0 = ctti[:].rearrange("p (a b) -> p a b", a=G)[:, :, 0:1]
                for h_ in range(2):
                    rows = slice(h_ * 64, (h_ + 1) * 64)
                    nc.tensor.matmul(ps[rows, :].rearrange("p (a b) -> p a b", a=G),
                                     C[ma][rows, :], r0[h_ * 64:(h_ + 1) * 64],
                                     start=True, stop=False)
                    nc.tensor.matmul(ps[rows, :].rearrange("p (a b) -> p a b", a=G),
                                     C[mb][rows, :], i0[h_ * 64:(h_ + 1) * 64],
                                     start=False, stop=True)
                nc.vector.tensor_copy(
                    dst[:].rearrange("p (a b) -> p a b", a=G)[:, :, 0:1],
                    ps.rearrange("p (a b) -> p a b", a=G))

            if debug_taps:
                for tp, t in (("t_z_r", zr), ("t_z_i", zi),
                              ("t_zc_r", zcr), ("t_zc_i", zci)):
                    nc.sync.dma_start(taps[tp][:, g0*128:(g0+G)*128], t[:])

            # pointwise: Zv = A*Z + B*Zc
            ab = pw.tile([128, 4, 1024], bf, tag="ab")
            cols = slice(g0 * 128, (g0 + G) * 128)
            nc.sync.dma_start(
                ab[:],
                dram["fields"][:].rearrange("p (f c) -> p f c", f=4)[:, :, cols])
            ar, ai, br, bi = ab[:, 0], ab[:, 1], ab[:, 2], ab[:, 3]
            zvr = work.tile([128, 1024], bf, tag="zvr", bufs=2)
            zvi = work.tile([128, 1024], bf, tag="zvi", bufs=2)
            p1 = work.tile([128, 1024], bf, name="p1", tag="s1")
            p2 = work.tile([128, 1024], bf, name="p2", tag="s2")
            p3 = work.tile([128, 1024], bf, name="p3", tag="s3")
            p4 = work.tile([128, 1024], bf, name="p4", tag="s4")
            nc.vector.tensor_mul(p1[:], zr[:], ar)
            nc.gpsimd.tensor_mul(p2[:], zi[:], ai)
            nc.gpsimd.tensor_mul(p3[:], zcr[:], br)
            nc.vector.tensor_mul(p4[:], zci[:], bi)
            nc.vector.tensor_sub(p1[:], p1[:], p2[:])
            nc.vector.tensor_sub(p3[:], p3[:], p4[:])
            nc.vector.tensor_add(zvr[:], p1[:], p3[:])
            nc.vector.tensor_mul(p1[:], zi[:], ar)
            nc.vector.tensor_mul(p2[:], zr[:], ai)
            nc.vector.tensor_mul(p3[:], zci[:], br)
            nc.vector.tensor_mul(p4[:], zcr[:], bi)
            nc.vector.tensor_add(p1[:], p1[:], p2[:])
            nc.vector.tensor_add(p3[:], p3[:], p4[:])
            nc.vector.tensor_add(zvi[:], p1[:], p3[:])
            if debug_taps:
                nc.sync.dma_start(taps["t_zv_r"][:, cols], zvr[:])
                nc.sync.dma_start(taps["t_zv_i"][:, cols], zvi[:])

            # B'-rev: C3[k1, (h,j2)] per (g, h')
            c3r = work.tile([128, 1024], bf, tag="c3r", bufs=3)
            c3i = work.tile([128, 1024], bf, tag="c3i", bufs=3)
            for gl in range(G):
                for h_ in range(2):
                    rows = slice(h_ * 64, (h_ + 1) * 64)
                    lr = zvr[rows, gl * 128:(gl + 1) * 128]
                    li = zvi[rows, gl * 128:(gl + 1) * 128]
                    oc = (2 * gl + h_) * 64
                    ps_re = pp.tile([128, 64], f32, name="psD_re", tag="pd", bufs=3)
                    nc.tensor.matmul(ps_re[:], lr, C["d2i_r"][rows, :], start=True, stop=False)
                    nc.tensor.matmul(ps_re[:], li, C["d2i_in"][rows, :], start=False, stop=True)
                    nc.scalar.copy(c3r[:, oc:oc + 64], ps_re[:])
                    ps_im = pp.tile([128, 64], f32, name="psD_im", tag="pd", bufs=3)
                    nc.tensor.matmul(ps_im[:], lr, C["d2i_i"][rows, :], start=True, stop=False)
                    nc.tensor.matmul(ps_im[:], li, C["d2i_r"][rows, :], start=False, stop=True)
                    nc.vector.tensor_copy(c3i[:, oc:oc + 64], ps_im[:])
            if debug_taps:
                nc.sync.dma_start(taps["t_c3_r"][:, cols], c3r[:])
                nc.sync.dma_start(taps["t_c3_i"][:, cols], c3i[:])

            # inverse twiddle (broadcast [128,64] over h=16)
            twic_b = C["twic"][:].unsqueeze(1).broadcast_to((128, 16, 64))
            twis_b = C["twis"][:].unsqueeze(1).broadcast_to((128, 16, 64))
            v3h = lambda t: t[:].rearrange("p (a b) -> p a b", a=16)
            c3tr = work.tile([128, 1024], bf, tag="c3tr", bufs=2)
            c3ti = work.tile([128, 1024], bf, tag="c3ti", bufs=2)
            tC = work.tile([128, 1024], bf, name="tC", tag="s1")
            tD = work.tile([128, 1024], bf, name="tD", tag="s2")
            nc.vector.tensor_mul(v3h(tC), v3h(c3r), twic_b)
            nc.gpsimd.tensor_mul(v3h(tD), v3h(c3i), twis_b)
            nc.vector.tensor_sub(c3tr[:], tC[:], tD[:])
            nc.gpsimd.tensor_mul(v3h(tC), v3h(c3r), twis_b)
            nc.vector.tensor_mul(v3h(tD), v3h(c3i), twic_b)
            nc.vector.tensor_add(c3ti[:], tC[:], tD[:])

            # stage A' -> vE, vO [j1<64, (h, j2)]
            ve = work.tile([64, 1024], bf, tag="ve")
            vo = work.tile([64, 1024], bf, tag="vo")
            ve_acts, vo_acts = [], []
            for (dst, acts, ma, mb) in ((ve, ve_acts, "d1i_r", "d1i_in"),
                                        (vo, vo_acts, "d1i_i", "d1i_r")):
                for c in range(2):
                    cols2 = slice(c * 512, (c + 1) * 512)
                    ps = pp.tile([64, 512], f32, name="pse", tag="pbig", bufs=3)
                    nc.tensor.matmul(ps[:], C[ma][:], c3tr[:, cols2], start=True, stop=False)
                    nc.tensor.matmul(ps[:], C[mb][:], c3ti[:, cols2], start=False, stop=True)
                    acts.append(nc.scalar.copy(dst[:, cols2], ps[:]))

            # scatter into DRAM bounce then gather this group's 16 y-rows
            for dvt, dst, (srct, acts) in ((dv_e, yE, (ve, ve_acts)),
                                           (dv_o, yO, (vo, vo_acts))):
                dma = nc.sync.dma_start(dvt[gg], srct[:])
                for a in acts:
                    add_dep_helper(dma.ins, a.ins, reason="scatter after A' evac")
                gat = nc.sync.dma_start(
                    yE[gg * 16:(gg + 1) * 16, :].rearrange("h (j b) -> h j b", j=64)
                    if dst is yE else
                    yO[gg * 16:(gg + 1) * 16, :].rearrange("h (j b) -> h j b", j=64),
                    dvt[gg].rearrange("j (hl b) -> hl j b", hl=16))
                add_dep_helper(gat.ins, dma.ins, reason="y gather after scatter")

        if debug_taps:
            nc.sync.dma_start(taps["t_ye"][:], yE[:])
            nc.sync.dma_start(taps["t_yo"][:], yO[:])

        # ---- gelu + GLU + pool ----------------------------------------
        CG = 0.7978845608028654
        gtiles = {}
        for i, pl in enumerate((yE, yO)):
            gtiles[i] = pw.tile([128, 4096], bf, name=f"gel{i}", tag="ab")
        for i, pl in enumerate((yE, yO)):   # sq = x*x
            nc.vector.tensor_mul(gtiles[i][:], pl[:], pl[:])
        for i, pl in enumerate((yE, yO)):   # rr = 0.044715*sq + 1
            nc.scalar.activation(gtiles[i][:], gtiles[i][:], AF.Identity,
                                 bias=C["ones_c"], scale=0.044715)
        for i, pl in enumerate((yE, yO)):   # qq = x*rr
            nc.vector.tensor_mul(gtiles[i][:], pl[:], gtiles[i][:])
        for i, pl in enumerate((yE, yO)):   # tt = tanh(CG*qq)
            nc.scalar.activation(gtiles[i][:], gtiles[i][:], AF.Tanh, scale=CG)
        for i, pl in enumerate((yE, yO)):   # uu = 0.5*tt + 0.5
            nc.scalar.activation(gtiles[i][:], gtiles[i][:], AF.Identity,
                                 bias=C["half_c"], scale=0.5)
        for i, pl in enumerate((yE, yO)):   # y = x*uu
            nc.vector.tensor_mul(pl[:], pl[:], gtiles[i][:])

        scratch = work.tile([128, 512], bf, tag="glu_scratch")
        idx = 0
        for plane in (yE, yO):
            for c in range(8):
                cols = slice(c * 512, (c + 1) * 512)
                ps_a = pp.tile([128, 512], f32, name="ps_a", tag="pbig", bufs=3)
                ps_g = pp.tile([128, 512], f32, name="ps_g", tag="pbig", bufs=3)
                nc.tensor.matmul(ps_a, C["glu_lhsT"][:, 0:128], plane[:, cols],
                                 start=True, stop=True)
                nc.tensor.matmul(ps_g, C["glu_lhsT"][:, 128:256], plane[:, cols],
                                 start=True, stop=True)
                sig = work.tile([128, 512], bf, tag="glu_sig")
                nc.scalar.activation(sig[:], ps_g, AF.Sigmoid,
                                     bias=C["glu_bg"], scale=1.0)
                nc.vector.scalar_tensor_tensor(
                    scratch[:], ps_a, C["glu_ba"], sig[:],
                    op0=ALU.add, op1=ALU.mult,
                    accum_out=pool_cols[:, idx:idx + 1])
                idx += 1

        pool_t = work.tile([128, 1], f32, tag="pool_t")
        nc.vector.tensor_reduce(pool_t[:], pool_cols[:],
                                axis=mybir.AxisListType.X, op=ALU.add)
        nc.sync.dma_start(pool_out[:], pool_t[:])

        _stack.close()

    nc.compile()
    return nc


_CACHED_NC = None


def kernel(**inputs):
    global _CACHED_NC
    from concourse.bass_utils import run_bass_kernel_spmd

    shared, per_core = host_prep(inputs)
    if _CACHED_NC is None:
        _CACHED_NC = build_program()
    nc = _CACHED_NC

    in_maps = [{**shared, **pc} for pc in per_core]
    res = run_bass_kernel_spmd(nc, in_maps, list(range(B)))
    pool = np.stack([np.asarray(res.results[b]["pool"][:, 0], np.float64)
                     for b in range(B)])                     # (8, 128)
    pooled = pool / float(L)
    dec_w = np.asarray(inputs["dec_w"], np.float64)
    dec_b = np.asarray(inputs["dec_b"], np.float64)
    return (pooled @ dec_w + dec_b).astype(np.float32)


if __name__ == "__main__":
    ins = {
        "x": np.random.randn(B, L, 2).astype(np.float32),
        "enc_w": np.random.randn(2, H).astype(np.float32),
        "enc_b": np.random.randn(H).astype(np.float32),
        "log_dt": np.random.rand(H).astype(np.float32),
        "log_A_real": np.random.randn(H, 32).astype(np.float32),
        "A_imag": np.random.randn(H, 32).astype(np.float32),
        "C_re": np.random.randn(H, 32).astype(np.float32),
        "C_im": np.random.randn(H, 32).astype(np.float32),
        "D": np.random.randn(H).astype(np.float32),
        "out_w": np.random.randn(2 * H, H).astype(np.float32),
        "out_b": np.random.randn(2 * H).astype(np.float32),
        "dec_w": np.random.randn(H, 1).astype(np.float32),
        "dec_b": np.random.randn(1).astype(np.float32),
    }
    print(kernel(**ins).shape)


# revision 50
# speedup vs baseline: 2.2763x; 1.0115x over previous
"""S4D AddingModel — Bass/Tile kernel for 8 Trainium2 NeuronCores.

Strategy (data-parallel over batch B=8, one batch element per core):
  encoder matmul -> packed complex z (even/odd samples) -> four-step
  FFT_8192 (stage A over j1 via reverse-matmul, twiddle, stage B over j2)
  -> fused pointwise  Zv[k] = A[k]*Z[k] + B[k]*conj(Z[8192-k])  where the
  host-precomputed A/B fields absorb the rfft unpack, the S4D kernel
  transfer function (incl. the D skip term), and the repack -> mirrored
  inverse four-step -> gelu -> GLU projection -> mean-pool partial sums.

The S4D kernel construction + its rFFT + the A/B fields are tiny
parameter-only computations done on host (numpy).  All O(B*H*L) work runs
on the NeuronCores in one NEFF.

Shapes hardcoded: B=8, L=8192, H=128, N=32.
"""
import numpy as np
import ml_dtypes

B, L, H = 8, 8192, 128
M = 8192          # packed complex FFT length
M1, M2 = 128, 64  # j = j1*64 + j2 ; k = k2*128 + k1
G = 8             # g-chunks per group
NG = 8            # number of groups (NG*G = 64 chunks of 128 cols)

_BF = ml_dtypes.bfloat16


# ---------------------------------------------------------------------------
# host-side constants
# ---------------------------------------------------------------------------

def _host_fields(log_dt, log_A_real, A_imag, C_re, C_im, D):
    """S4D kernel K, its 2L rfft, and the packed-pointwise A/B fields."""
    dt = np.exp(log_dt.astype(np.float64))
    A = -np.exp(log_A_real.astype(np.float64)) + 1j * A_imag.astype(np.float64)
    C = C_re.astype(np.float64) + 1j * C_im.astype(np.float64)
    dtA = dt[:, None] * A
    K_coef = C * (np.exp(dtA) - 1.0) / A
    w = np.exp(dtA)
    Tb = 128
    J = L // Tb
    v_lo = w[:, :, None] ** np.arange(Tb)
    v_hi = (w ** Tb)[:, :, None] ** np.arange(J)
    K = 2.0 * np.matmul(K_coef[:, None, :] * v_hi.transpose(0, 2, 1),
                        v_lo).real.reshape(H, L)

    Khat = np.fft.rfft(K, 2 * L, axis=-1)              # (H, 8193)
    Khat = Khat + D.astype(np.float64)[:, None]        # fold skip y += D*u
    k = np.arange(M)
    P = Khat[:, :M]
    idx = (M - k) % (2 * L)
    Q = np.conj(Khat[:, idx])
    Q[:, 0] = Khat[:, M]
    th = 2.0 * np.pi * k / (2 * L)
    Afld = 0.5 * (P + Q) - 0.5 * (P - Q) * np.sin(th)[None, :]
    Bfld = 0.5j * (P - Q) * np.cos(th)[None, :]
    return Afld, Bfld                                   # (H, 8192) complex


def _pack_field(F):
    """(H, 8192) field -> device plane [128=(h',k2), 8192=(g,k1)]."""
    Fg = F.reshape(H, M2, M1)                           # [h, k2, k1]
    P = Fg.reshape(64, 2, M2, M1).transpose(1, 2, 0, 3)  # [h', k2, g, k1]
    return np.ascontiguousarray(P.reshape(128, 8192))


def _dup(mat):
    """[64, X] -> [128, X] duplicated halves (for base-partition 0/64 use)."""
    return np.concatenate([mat, mat], axis=0)


def host_prep(inputs):
    """Returns (shared_map, per_core_maps, dec_w, dec_b)."""
    f32 = np.float32
    x = np.asarray(inputs["x"], f32)
    Afld, Bfld = _host_fields(inputs["log_dt"], inputs["log_A_real"],
                              inputs["A_imag"], inputs["C_re"],
                              inputs["C_im"], inputs["D"])

    j1 = np.arange(64)
    k1 = np.arange(M1)
    j2 = np.arange(M2)
    k2 = np.arange(M2)

    def bf(a):
        return np.ascontiguousarray(a, dtype=np.float32).astype(_BF)

    shared = {}
    shared["enc_lhsT"] = bf(inputs["enc_w"])                      # [2, 128]
    shared["enc_bias"] = np.asarray(inputs["enc_b"], f32).reshape(128, 1)

    th = 2 * np.pi * np.outer(j1, k1) / M1                        # [64, 128]
    shared["d1m_r"] = bf(_dup(np.cos(th)))
    shared["d1m_i"] = bf(_dup(-np.sin(th)))
    shared["d1m_in"] = bf(_dup(np.sin(th)))

    p = np.arange(128) % 64
    th = 2 * np.pi * np.outer(p, k1) / M                          # [128, 128]
    shared["twc"] = bf(np.cos(th))
    shared["tws"] = bf(-np.sin(th))

    th = 2 * np.pi * np.outer(j2, k2) / M2                        # [64, 64]
    shared["d2m_r"] = bf(_dup(np.cos(th)))
    shared["d2m_i"] = bf(_dup(-np.sin(th)))
    shared["d2m_in"] = bf(_dup(np.sin(th)))
    shared["d2m_rn"] = bf(_dup(-np.cos(th)))

    th = 2 * np.pi * np.outer(j2, 63 - k2) / M2                   # [64, 64]
    shared["f1_r"] = bf(_dup(np.cos(th)))
    shared["f1_i"] = bf(_dup(np.sin(th)))
    shared["f1_rn"] = bf(_dup(-np.cos(th)))

    th = 2 * np.pi * np.outer(k2, j2) / M2                        # [64, 64]
    shared["d2i_r"] = bf(_dup(np.cos(th)))
    shared["d2i_i"] = bf(_dup(np.sin(th)))
    shared["d2i_in"] = bf(_dup(-np.sin(th)))

    th = 2 * np.pi * np.outer(np.arange(M1), j2) / M              # [128, 64]
    shared["twic"] = bf(np.cos(th))
    shared["twis"] = bf(np.sin(th))

    th = 2 * np.pi * np.outer(np.arange(M1), j1) / M1             # [128, 64]
    shared["d1i_r"] = bf(np.cos(th) / M)
    shared["d1i_i"] = bf(np.sin(th) / M)
    shared["d1i_in"] = bf(-np.sin(th) / M)

    shared["glu_lhsT"] = bf(np.asarray(inputs["out_w"], f32).T)   # [128, 256]
    ob = np.asarray(inputs["out_b"], f32)
    shared["glu_ba"] = ob[:128].reshape(128, 1).astype(f32)
    shared["glu_bg"] = ob[128:].reshape(128, 1).astype(f32)
    shared["ones_c"] = np.ones((128, 1), f32)
    shared["half_c"] = np.full((128, 1), 0.5, f32)

    shared["fields"] = np.concatenate(
        [bf(_pack_field(p)) for p in (Afld.real, Afld.imag,
                                      Bfld.real, Bfld.imag)], axis=1)

    bf_names = ["enc_lhsT", "d1m_r", "d1m_i", "d1m_in", "twc", "tws",
                "d2m_r", "d2m_i", "d2m_in", "d2m_rn", "f1_r", "f1_i", "f1_rn",
                "d2i_r", "d2i_i", "d2i_in", "twic", "twis",
                "d1i_r", "d1i_i", "d1i_in", "glu_lhsT"]
    blocks = []
    for nm in bf_names:
        a = shared.pop(nm)
        if a.shape[0] != 128:
            pad = np.zeros((128 - a.shape[0], a.shape[1]), a.dtype)
            a = np.concatenate([a, pad], axis=0)
        blocks.append(a)
    shared["cpack"] = np.concatenate(blocks, axis=1)
    f32_names = ["enc_bias", "glu_ba", "glu_bg", "ones_c", "half_c"]
    shared["fpack"] = np.concatenate([shared.pop(nm) for nm in f32_names],
                                     axis=1).astype(f32)

    per_core = []
    for b in range(B):
        xb = x[b]                                                 # (8192, 2)
        per_core.append({
            "xe": bf(xb[0::2, :].T),                              # [2, 4096]
            "xo": bf(xb[1::2, :].T),                              # [2, 4096]
        })
    return shared, per_core


# ---------------------------------------------------------------------------
# device program
# ---------------------------------------------------------------------------

_SHARED_SPECS = [
    ("enc_lhsT", (2, 128), "bf"), ("enc_bias", (128, 1), "f32"),
    ("d1m_r", (128, 128), "bf"), ("d1m_i", (128, 128), "bf"),
    ("d1m_in", (128, 128), "bf"),
    ("twc", (128, 128), "bf"), ("tws", (128, 128), "bf"),
    ("d2m_r", (128, 64), "bf"), ("d2m_i", (128, 64), "bf"),
    ("d2m_in", (128, 64), "bf"), ("d2m_rn", (128, 64), "bf"),
    ("f1_r", (128, 64), "bf"), ("f1_i", (128, 64), "bf"),
    ("f1_rn", (128, 64), "bf"),
    ("d2i_r", (128, 64), "bf"), ("d2i_i", (128, 64), "bf"),
    ("d2i_in", (128, 64), "bf"),
    ("twic", (128, 64), "bf"), ("twis", (128, 64), "bf"),
    ("d1i_r", (128, 64), "bf"), ("d1i_i", (128, 64), "bf"),
    ("d1i_in", (128, 64), "bf"),
    ("glu_lhsT", (128, 256), "bf"), ("glu_ba", (128, 1), "f32"),
    ("glu_bg", (128, 1), "f32"), ("ones_c", (128, 1), "f32"),
    ("half_c", (128, 1), "f32"),
    ("fields", (128, 4 * 8192), "bf"),
]


def build_program(debug_taps=False):
    """Build + compile the single-core SPMD bass program."""
    import concourse.bass as bass
    import concourse.tile as tile
    from concourse import bacc, mybir

    bf = mybir.dt.bfloat16
    f32 = mybir.dt.float32
    AF = mybir.ActivationFunctionType
    ALU = mybir.AluOpType

    nc = bacc.Bacc("TRN2", target_bir_lowering=False, debug=False,
                   num_devices=B)

    dram = {}
    for name, shape, dt_ in _SHARED_SPECS:
        dram[name] = nc.dram_tensor(name, list(shape),
                                    bf if dt_ == "bf" else f32,
                                    kind="ExternalInput").ap()
    dram["xe"] = nc.dram_tensor("xe", [2, 4096], bf, kind="ExternalInput").ap()
    dram["xo"] = nc.dram_tensor("xo", [2, 4096], bf, kind="ExternalInput").ap()
    pool_out = nc.dram_tensor("pool", [128, 1], f32, kind="ExternalOutput").ap()
    taps = {}
    if debug_taps:
        for nm in ("t_ct_r", "t_ct_i", "t_z_r", "t_z_i", "t_zc_r", "t_zc_i",
                   "t_zv_r", "t_zv_i", "t_c3_r", "t_c3_i"):
            taps[nm] = nc.dram_tensor(nm, [128, 8192], bf,
                                      kind="ExternalOutput").ap()
        for nm in ("t_ye", "t_yo"):
            taps[nm] = nc.dram_tensor(nm, [128, 4096], bf,
                                      kind="ExternalOutput").ap()

    with tile.TileContext(nc) as tc:
        from contextlib import ExitStack
        _stack = ExitStack()
        cpool = _stack.enter_context(tc.tile_pool(name="consts", bufs=1))
        C = {}
        for name, shape, dt_ in _SHARED_SPECS:
            if name == "fields":
                continue
            ct = cpool.tile(list(shape), bf if dt_ == "bf" else f32,
                            name="c_" + name, tag=name)
            nc.sync.dma_start(ct[:], dram[name][:])
            C[name] = ct[:]

        persist = _stack.enter_context(tc.tile_pool(name="persist", bufs=1))
        t1 = persist.tile([128, 8192], bf, tag="t1")
        yE = persist.tile([128, 4096], bf, tag="yE")
        yO = persist.tile([128, 4096], bf, tag="yO")
        pool_cols = persist.tile([128, 16], f32, tag="pool_cols")



        work = _stack.enter_context(tc.tile_pool(name="work", bufs=2))
        pw = _stack.enter_context(tc.tile_pool(name="pw", bufs=2))
        pp = _stack.enter_context(tc.tile_pool(name="pp", bufs=1, space="PSUM"))

        # ---- encoder -> DRAM bounce -> T1[j1, (h,j2)] ------------------
        from concourse.tile import add_dep_helper
        dz = {0: nc.dram_tensor("dz_r", [128, 4096], bf, kind="Internal").ap(),
              1: nc.dram_tensor("dz_i", [128, 4096], bf, kind="Internal").ap()}
        for pi, (dst, src) in enumerate(((t1[0:64, :], dram["xe"]),
                                         (t1[64:128, :], dram["xo"]))):
            scat = []
            for c in range(8):
                xch = work.tile([2, 512], bf, tag="xch")
                nc.sync.dma_start(xch[:], src[:, c * 512:(c + 1) * 512])
                pse2 = pp.tile([128, 512], f32, name="pse2", tag="pbig", bufs=3)
                nc.tensor.matmul(pse2[:], C["enc_lhsT"][:], xch[:],
                                 start=True, stop=True)
                zch = work.tile([128, 512], bf, tag="zch")
                nc.scalar.activation(zch[:], pse2[:], AF.Identity,
                                     bias=C["enc_bias"], scale=1.0)
                scat.append(nc.sync.dma_start(
                    dz[pi][:, c * 512:(c + 1) * 512], zch[:]))
            gat = nc.sync.dma_start(
                dst.rearrange("a (h b) -> a h b", h=128),
                dz[pi][:].rearrange("h (a b) -> h a b", a=64).transpose([1, 0, 2]))
            for s in scat:
                add_dep_helper(gat.ins, s.ins, reason="t1 gather after scatter")

        # ---- main groups ----------------------------------------------
        dv_e = nc.dram_tensor("dv_e", [NG, 64, 1024], bf, kind="Internal").ap()
        dv_o = nc.dram_tensor("dv_o", [NG, 64, 1024], bf, kind="Internal").ap()
        fin_scat = []
        for gg in range(NG):
            g0 = gg * G
            # A-rev: Ct[(h',j2), (g,k1)] chunks
            ctr = work.tile([128, 1024], bf, tag="ctr", bufs=3)
            cti = work.tile([128, 1024], bf, tag="cti", bufs=3)
            for gl in range(G):
                g = g0 + gl
                lr = t1[0:64, g * 128:(g + 1) * 128]
                li = t1[64:128, g * 128:(g + 1) * 128]
                ps_re = pp.tile([128, 128], f32, name="psA_re", tag="pa", bufs=2)
                nc.tensor.matmul(ps_re[:], lr, C["d1m_r"][0:64, :], start=True, stop=False)
                nc.tensor.matmul(ps_re[:], li, C["d1m_in"][64:128, :], start=False, stop=True)
                nc.scalar.copy(ctr[:, gl * 128:(gl + 1) * 128], ps_re[:])
                ps_im = pp.tile([128, 128], f32, name="psA_im", tag="pa", bufs=2)
                nc.tensor.matmul(ps_im[:], lr, C["d1m_i"][0:64, :], start=True, stop=False)
                nc.tensor.matmul(ps_im[:], li, C["d1m_r"][64:128, :], start=False, stop=True)
                nc.vector.tensor_copy(cti[:, gl * 128:(gl + 1) * 128], ps_im[:])

            # forward twiddle (broadcast [128,128] tile over g)
            twc_b = C["twc"][:].unsqueeze(1).broadcast_to((128, G, 128))
            tws_b = C["tws"][:].unsqueeze(1).broadcast_to((128, G, 128))
            cttr = work.tile([128, 1024], bf, tag="cttr", bufs=3)
            ctti = work.tile([128, 1024], bf, tag="ctti", bufs=3)
            v3 = lambda t: t[:].rearrange("p (a b) -> p a b", a=G)
            tA = work.tile([128, 1024], bf, name="tA", tag="s1")
            tB = work.tile([128, 1024], bf, name="tB", tag="s2")
            nc.vector.tensor_mul(v3(tA), v3(ctr), twc_b)
            nc.gpsimd.tensor_mul(v3(tB), v3(cti), tws_b)
            nc.vector.tensor_sub(cttr[:], tA[:], tB[:])
            nc.gpsimd.tensor_mul(v3(tA), v3(ctr), tws_b)
            nc.vector.tensor_mul(v3(tB), v3(cti), twc_b)
            nc.vector.tensor_add(ctti[:], tA[:], tB[:])
            if debug_taps:
                nc.sync.dma_start(taps["t_ct_r"][:, g0*128:(g0+G)*128], cttr[:])
                nc.sync.dma_start(taps["t_ct_i"][:, g0*128:(g0+G)*128], ctti[:])

            # stage B -> Z ; Zc via F1/k0 path
            zr = work.tile([128, 1024], bf, tag="zr", bufs=3)
            zi = work.tile([128, 1024], bf, tag="zi", bufs=3)
            for (dst, m1a, m1b) in ((zr, "d2m_r", "d2m_in"),
                                    (zi, "d2m_i", "d2m_r")):
                for c in range(2):
                    ps = pp.tile([128, 512], f32, name="psb", tag="pbig", bufs=3)
                    cols = slice(c * 512, (c + 1) * 512)
                    for h_ in range(2):
                        rows = slice(h_ * 64, (h_ + 1) * 64)
                        nc.tensor.matmul(ps[rows, :], C[m1a][rows, :],
                                         cttr[rows, cols], start=True, stop=False)
                        nc.tensor.matmul(ps[rows, :], C[m1b][rows, :],
                                         ctti[rows, cols], start=False, stop=True)
                    nc.scalar.copy(dst[:, cols], ps[:])

            zcr = work.tile([128, 1024], bf, tag="zcr", bufs=3)
            zci = work.tile([128, 1024], bf, tag="zci", bufs=3)
            # main part k1 in [1,128): rhs cols reversed within each g block
            for (dst, ma, mb) in ((zcr, "f1_r", "f1_i"), (zci, "f1_i", "f1_rn")):
                for c in range(2):   # 4 g per chunk
                    psz = pp.tile([128, 512], f32, name="psc", tag="pbig", bufs=3)
                    ps = psz[:, 0:508]
                    for h_ in range(2):
                        rows = slice(h_ * 64, (h_ + 1) * 64)
                        rev_r = cttr[rows, :].rearrange(
                            "p (a b) -> p a b", a=G)[:, c * 4:(c + 1) * 4, 127:0:-1]
                        rev_i = ctti[rows, :].rearrange(
                            "p (a b) -> p a b", a=G)[:, c * 4:(c + 1) * 4, 127:0:-1]
                        nc.tensor.matmul(ps[rows, :].rearrange(
                            "p (a b) -> p a b", a=4), C[ma][rows, :], rev_r,
                            start=True, stop=False)
                        nc.tensor.matmul(ps[rows, :].rearrange(
                            "p (a b) -> p a b", a=4), C[mb][rows, :], rev_i,
                            start=False, stop=True)
                    nc.scalar.copy(
                        dst[:].rearrange("p (a b) -> p a b", a=G)
                           [:, c * 4:(c + 1) * 4, 1:128],
                        ps.rearrange("p (c b) -> p c b", c=4))
            # k1 = 0 columns
            for (dst, ma, mb) in ((zcr, "d2m_r", "d2m_i"), (zci, "d2m_i", "d2m_rn")):
                psk = pp.tile([128, 128], f32, name="psk", tag="pa", bufs=2)
                ps = psk[:, 0:8]
                r0 = cttr[:].rearrange("p (a b) -> p a b", a=G)[:, :, 0:1]
                i0 = ctti[:].rearrange("p (a b) -> p a b", a=G)[:, :, 0:1]
                for h_ in range(2):
                    rows = slice(h_ * 64, (h_ + 1) * 64)
                    nc.tensor.matmul(ps[rows, :].rearrange("p (a b) -> p a b", a=G),
                                     C[ma][rows, :], r0[h_ * 64:(h_ + 1) * 64],
                                     start=True, stop=False)
                    nc.tensor.matmul(ps[rows, :].rearrange("p (a b) -> p a b", a=G),
                                     C[mb][rows, :], i0[h_ * 64:(h_ + 1) * 64],
                                     start=False, stop=True)
                nc.vector.tensor_copy(
                    dst[:].rearrange("p (a b) -> p a b", a=G)[:, :, 0:1],
                    ps.rearrange("p (a b) -> p a b", a=G))

            if debug_taps:
                for tp, t in (("t_z_r", zr), ("t_z_i", zi),
                              ("t_zc_r", zcr), ("t_zc_i", zci)):
                    nc.sync.dma_start(taps[tp][:, g0*128:(g0+G)*128], t[:])

            # pointwise: Zv = A*Z + B*Zc
            ab = pw.tile([128, 4, 1024], bf, tag="ab")
            cols = slice(g0 * 128, (g0 + G) * 128)
            nc.sync.dma_start(
                ab[:],
                dram["fields"][:].rearrange("p (f c) -> p f c", f=4)[:, :, cols])
            ar, ai, br, bi = ab[:, 0], ab[:, 1], ab[:, 2], ab[:, 3]
            zvr = work.tile([128, 1024], bf, tag="zvr", bufs=2)
            zvi = work.tile([128, 1024], bf, tag="zvi", bufs=2)
            p1 = work.tile([128, 1024], bf, name="p1", tag="s1")
            p2 = work.tile([128, 1024], bf, name="p2", tag="s2")
            p3 = work.tile([128, 1024], bf, name="p3", tag="s3")
            p4 = work.tile([128, 1024], bf, name="p4", tag="s4")
            nc.vector.tensor_mul(p1[:], zr[:], ar)
            nc.gpsimd.tensor_mul(p2[:], zi[:], ai)
            nc.gpsimd.tensor_mul(p3[:], zcr[:], br)
            nc.vector.tensor_mul(p4[:], zci[:], bi)
            nc.vector.tensor_sub(p1[:], p1[:], p2[:])
            nc.vector.tensor_sub(p3[:], p3[:], p4[:])
            nc.vector.tensor_add(zvr[:], p1[:], p3[:])
            nc.vector.tensor_mul(p1[:], zi[:], ar)
            nc.vector.tensor_mul(p2[:], zr[:], ai)
            nc.vector.tensor_mul(p3[:], zci[:], br)
            nc.vector.tensor_mul(p4[:], zcr[:], bi)
            nc.vector.tensor_add(p1[:], p1[:], p2[:])
            nc.vector.tensor_add(p3[:], p3[:], p4[:])
            nc.vector.tensor_add(zvi[:], p1[:], p3[:])
            if debug_taps:
                nc.sync.dma_start(taps["t_zv_r"][:, cols], zvr[:])
                nc.sync.dma_start(taps["t_zv_i"][:, cols], zvi[:])

            # B'-rev: C3[k1, (h,j2)] per (g, h')
            c3r = work.tile([128, 1024], bf, tag="c3r", bufs=3)
            c3i = work.tile([128, 1024], bf, tag="c3i", bufs=3)
            for gl in range(G):
                for h_ in range(2):
                    rows = slice(h_ * 64, (h_ + 1) * 64)
                    lr = zvr[rows, gl * 128:(gl + 1) * 128]
                    li = zvi[rows, gl * 128:(gl + 1) * 128]
                    oc = (2 * gl + h_) * 64
                    ps_re = pp.tile([128, 64], f32, name="psD_re", tag="pd", bufs=3)
                    nc.tensor.matmul(ps_re[:], lr, C["d2i_r"][rows, :], start=True, stop=False)
                    nc.tensor.matmul(ps_re[:], li, C["d2i_in"][rows, :], start=False, stop=True)
                    nc.scalar.copy(c3r[:, oc:oc + 64], ps_re[:])
                    ps_im = pp.tile([128, 64], f32, name="psD_im", tag="pd", bufs=3)
                    nc.tensor.matmul(ps_im[:], lr, C["d2i_i"][rows, :], start=True, stop=False)
                    nc.tensor.matmul(ps_im[:], li, C["d2i_r"][rows, :], start=False, stop=True)
                    nc.vector.tensor_copy(c3i[:, oc:oc + 64], ps_im[:])
            if debug_taps:
                nc.sync.dma_start(taps["t_c3_r"][:, cols], c3r[:])
                nc.sync.dma_start(taps["t_c3_i"][:, cols], c3i[:])

            # inverse twiddle (broadcast [128,64] over h=16)
            twic_b = C["twic"][:].unsqueeze(1).broadcast_to((128, 16, 64))
            twis_b = C["twis"][:].unsqueeze(1).broadcast_to((128, 16, 64))
            v3h = lambda t: t[:].rearrange("p (a b) -> p a b", a=16)
            c3tr = work.tile([128, 1024], bf, tag="c3tr", bufs=2)
            c3ti = work.tile([128, 1024], bf, tag="c3ti", bufs=2)
            tC = work.tile([128, 1024], bf, name="tC", tag="s1")
            tD = work.tile([128, 1024], bf, name="tD", tag="s2")
            nc.vector.tensor_mul(v3h(tC), v3h(c3r), twic_b)
            nc.gpsimd.tensor_mul(v3h(tD), v3h(c3i), twis_b)
            nc.vector.tensor_sub(c3tr[:], tC[:], tD[:])
            nc.gpsimd.tensor_mul(v3h(tC), v3h(c3r), twis_b)
            nc.vector.tensor_mul(v3h(tD), v3h(c3i), twic_b)
            nc.vector.tensor_add(c3ti[:], tC[:], tD[:])

            # stage A' -> vE, vO [j1<64, (h, j2)]
            ve = work.tile([64, 1024], bf, tag="ve")
            vo = work.tile([64, 1024], bf, tag="vo")
            ve_acts, vo_acts = [], []
            for (dst, acts, ma, mb) in ((ve, ve_acts, "d1i_r", "d1i_in"),
                                        (vo, vo_acts, "d1i_i", "d1i_r")):
                for c in range(2):
                    cols2 = slice(c * 512, (c + 1) * 512)
                    ps = pp.tile([64, 512], f32, name="pse", tag="pbig", bufs=3)
                    nc.tensor.matmul(ps[:], C[ma][:], c3tr[:, cols2], start=True, stop=False)
                    nc.tensor.matmul(ps[:], C[mb][:], c3ti[:, cols2], start=False, stop=True)
                    acts.append(nc.scalar.copy(dst[:, cols2], ps[:]))

            # scatter into DRAM bounce then gather this group's 16 y-rows
            for dvt, dst, (srct, acts) in ((dv_e, yE, (ve, ve_acts)),
                                           (dv_o, yO, (vo, vo_acts))):
                dma = nc.sync.dma_start(dvt[gg], srct[:])
                for a in acts:
                    add_dep_helper(dma.ins, a.ins, reason="scatter after A' evac")
                gat = nc.sync.dma_start(
                    yE[gg * 16:(gg + 1) * 16, :].rearrange("h (j b) -> h j b", j=64)
                    if dst is yE else
                    yO[gg * 16:(gg + 1) * 16, :].rearrange("h (j b) -> h j b", j=64),
                    dvt[gg].rearrange("j (hl b) -> hl j b", hl=16))
                add_dep_helper(gat.ins, dma.ins, reason="y gather after scatter")

        if debug_taps:
            nc.sync.dma_start(taps["t_ye"][:], yE[:])
            nc.sync.dma_start(taps["t_yo"][:], yO[:])

        # ---- gelu + GLU + pool ----------------------------------------
        CG = 0.7978845608028654
        planes = (yE, yO)
        idx = 0
        scratch = work.tile([128, 512], bf, tag="glu_scratch")
        for ch in range(2):
            h0 = ch * 2048
            hc = slice(h0, h0 + 2048)
            SPl = h0 + 1472
            lo = slice(h0, SPl)
            hi = slice(SPl, h0 + 2048)
            gt = {}
            for i in range(2):
                gt[i] = pw.tile([128, 2048], bf, name=f"gel{ch}_{i}", tag="ab")
            gl_ = lambda t: t[:, 0:1472]
            gh_ = lambda t: t[:, 1472:2048]
            for i, pl in enumerate(planes):   # sq = x*x
                nc.vector.tensor_mul(gl_(gt[i]), pl[:, lo], pl[:, lo])
                nc.gpsimd.tensor_mul(gh_(gt[i]), pl[:, hi], pl[:, hi])
            for i in range(2):                # rr = 0.044715*sq + 1
                nc.vector.tensor_scalar(gt[i][:], gt[i][:], 0.044715, 1.0,
                                        op0=ALU.mult, op1=ALU.add)
            for i, pl in enumerate(planes):   # qq = x*rr
                nc.vector.tensor_mul(gl_(gt[i]), pl[:, lo], gl_(gt[i]))
                nc.gpsimd.tensor_mul(gh_(gt[i]), pl[:, hi], gh_(gt[i]))
            for i in range(2):                # tt = tanh(CG*qq)
                nc.scalar.activation(gt[i][:], gt[i][:], AF.Tanh, scale=CG)
            for i in range(2):                # uu = 0.5*tt + 0.5
                nc.vector.tensor_scalar(gt[i][:], gt[i][:], 0.5, 0.5,
                                        op0=ALU.mult, op1=ALU.add)
            for i, pl in enumerate(planes):   # y = x*uu
                nc.vector.tensor_mul(pl[:, lo], pl[:, lo], gl_(gt[i]))
                nc.gpsimd.tensor_mul(pl[:, hi], pl[:, hi], gh_(gt[i]))

            for plane in planes:              # GLU on this column half
                for c in range(4):
                    cols = slice(h0 + c * 512, h0 + (c + 1) * 512)
                    ps_a = pp.tile([128, 512], f32, tag="pbig", bufs=3)
                    ps_g = pp.tile([128, 512], f32, tag="pbig", bufs=3)
                    nc.tensor.matmul(ps_a[:], C["glu_lhsT"][:, 0:128],
                                     plane[:, cols], start=True, stop=True)
                    nc.tensor.matmul(ps_g[:], C["glu_lhsT"][:, 128:256],
                                     plane[:, cols], start=True, stop=True)
                    sig = work.tile([128, 512], bf, tag="glu_sig")
                    nc.scalar.activation(sig[:], ps_g[:], AF.Sigmoid,
                                         bias=C["glu_bg"], scale=1.0)
                    nc.vector.scalar_tensor_tensor(
                        scratch[:], ps_a[:], C["glu_ba"], sig[:],
                        op0=ALU.add, op1=ALU.mult,
                        accum_out=pool_cols[:, idx:idx + 1])
                    idx += 1

        pool_t = work.tile([128, 1], f32, tag="pool_t")
        nc.vector.tensor_reduce(pool_t[:], pool_cols[:],
                                axis=mybir.AxisListType.X, op=ALU.add)
        nc.sync.dma_start(pool_out[:], pool_t[:])

        _stack.close()

    nc.compile()
    return nc


_CACHED_NC = None


def kernel(**inputs):
    global _CACHED_NC
    from concourse.bass_utils import run_bass_kernel_spmd

    shared, per_core = host_prep(inputs)
    if _CACHED_NC is None:
        _CACHED_NC = build_program()
    nc = _CACHED_NC

    in_maps = [{**shared, **pc} for pc in per_core]
    res = run_bass_kernel_spmd(nc, in_maps, list(range(B)))
    pool = np.stack([np.asarray(res.results[b]["pool"][:, 0], np.float64)
                     for b in range(B)])                     # (8, 128)
    pooled = pool / float(L)
    dec_w = np.asarray(inputs["dec_w"], np.float64)
    dec_b = np.asarray(inputs["dec_b"], np.float64)
    return (pooled @ dec_w + dec_b).astype(np.float32)


if __name__ == "__main__":
    ins = {
        "x": np.random.randn(B, L, 2).astype(np.float32),
        "enc_w": np.random.randn(2, H).astype(np.float32),
        "enc_b": np.random.randn(H).astype(np.float32),
        "log_dt": np.random.rand(H).astype(np.float32),
        "log_A_real": np.random.randn(H, 32).astype(np.float32),
        "A_imag": np.random.randn(H, 32).astype(np.float32),
        "C_re": np.random.randn(H, 32).astype(np.float32),
        "C_im": np.random.randn(H, 32).astype(np.float32),
        "D": np.random.randn(H).astype(np.float32),
        "out_w": np.random.randn(2 * H, H).astype(np.float32),
        "out_b": np.random.randn(2 * H).astype(np.float32),
        "dec_w": np.random.randn(H, 1).astype(np.float32),
        "dec_b": np.random.randn(1).astype(np.float32),
    }
    print(kernel(**ins).shape)


# revision 51
# speedup vs baseline: 2.3059x; 1.0130x over previous
"""S4D AddingModel — Bass/Tile kernel for 8 Trainium2 NeuronCores.

Strategy (data-parallel over batch B=8, one batch element per core):
  encoder matmul -> packed complex z (even/odd samples) -> four-step
  FFT_8192 (stage A over j1 via reverse-matmul, twiddle, stage B over j2)
  -> fused pointwise  Zv[k] = A[k]*Z[k] + B[k]*conj(Z[8192-k])  where the
  host-precomputed A/B fields absorb the rfft unpack, the S4D kernel
  transfer function (incl. the D skip term), and the repack -> mirrored
  inverse four-step -> gelu -> GLU projection -> mean-pool partial sums.

The S4D kernel construction + its rFFT + the A/B fields are tiny
parameter-only computations done on host (numpy).  All O(B*H*L) work runs
on the NeuronCores in one NEFF.

Shapes hardcoded: B=8, L=8192, H=128, N=32.
"""
import numpy as np
import ml_dtypes

B, L, H = 8, 8192, 128
M = 8192          # packed complex FFT length
M1, M2 = 128, 64  # j = j1*64 + j2 ; k = k2*128 + k1
G = 8             # g-chunks per group
NG = 8            # number of groups (NG*G = 64 chunks of 128 cols)

_BF = ml_dtypes.bfloat16


# ---------------------------------------------------------------------------
# host-side constants
# ---------------------------------------------------------------------------

def _host_fields(log_dt, log_A_real, A_imag, C_re, C_im, D):
    """S4D kernel K, its 2L rfft, and the packed-pointwise A/B fields."""
    dt = np.exp(log_dt.astype(np.float64))
    A = -np.exp(log_A_real.astype(np.float64)) + 1j * A_imag.astype(np.float64)
    C = C_re.astype(np.float64) + 1j * C_im.astype(np.float64)
    dtA = dt[:, None] * A
    K_coef = C * (np.exp(dtA) - 1.0) / A
    w = np.exp(dtA)
    Tb = 128
    J = L // Tb
    v_lo = w[:, :, None] ** np.arange(Tb)
    v_hi = (w ** Tb)[:, :, None] ** np.arange(J)
    K = 2.0 * np.matmul(K_coef[:, None, :] * v_hi.transpose(0, 2, 1),
                        v_lo).real.reshape(H, L)

    Khat = np.fft.rfft(K, 2 * L, axis=-1)              # (H, 8193)
    Khat = Khat + D.astype(np.float64)[:, None]        # fold skip y += D*u
    k = np.arange(M)
    P = Khat[:, :M]
    idx = (M - k) % (2 * L)
    Q = np.conj(Khat[:, idx])
    Q[:, 0] = Khat[:, M]
    th = 2.0 * np.pi * k / (2 * L)
    Afld = 0.5 * (P + Q) - 0.5 * (P - Q) * np.sin(th)[None, :]
    Bfld = 0.5j * (P - Q) * np.cos(th)[None, :]
    return Afld, Bfld                                   # (H, 8192) complex


def _pack_field(F):
    """(H, 8192) field -> device plane [128=(h',k2), 8192=(g,k1)]."""
    Fg = F.reshape(H, M2, M1)                           # [h, k2, k1]
    P = Fg.reshape(64, 2, M2, M1).transpose(1, 2, 0, 3)  # [h', k2, g, k1]
    return np.ascontiguousarray(P.reshape(128, 8192))


def _dup(mat):
    """[64, X] -> [128, X] duplicated halves (for base-partition 0/64 use)."""
    return np.concatenate([mat, mat], axis=0)


def host_prep(inputs):
    """Returns (shared_map, per_core_maps, dec_w, dec_b)."""
    f32 = np.float32
    x = np.asarray(inputs["x"], f32)
    Afld, Bfld = _host_fields(inputs["log_dt"], inputs["log_A_real"],
                              inputs["A_imag"], inputs["C_re"],
                              inputs["C_im"], inputs["D"])

    j1 = np.arange(64)
    k1 = np.arange(M1)
    j2 = np.arange(M2)
    k2 = np.arange(M2)

    def bf(a):
        return np.ascontiguousarray(a, dtype=np.float32).astype(_BF)

    shared = {}
    shared["enc_lhsT"] = bf(inputs["enc_w"])                      # [2, 128]
    shared["enc_bias"] = np.asarray(inputs["enc_b"], f32).reshape(128, 1)

    th = 2 * np.pi * np.outer(j1, k1) / M1                        # [64, 128]
    shared["d1m_r"] = bf(_dup(np.cos(th)))
    shared["d1m_i"] = bf(_dup(-np.sin(th)))
    shared["d1m_in"] = bf(_dup(np.sin(th)))

    p = np.arange(128) % 64
    th = 2 * np.pi * np.outer(p, k1) / M                          # [128, 128]
    shared["twc"] = bf(np.cos(th))
    shared["tws"] = bf(-np.sin(th))

    th = 2 * np.pi * np.outer(j2, k2) / M2                        # [64, 64]
    shared["d2m_r"] = bf(_dup(np.cos(th)))
    shared["d2m_i"] = bf(_dup(-np.sin(th)))
    shared["d2m_in"] = bf(_dup(np.sin(th)))
    shared["d2m_rn"] = bf(_dup(-np.cos(th)))

    th = 2 * np.pi * np.outer(j2, 63 - k2) / M2                   # [64, 64]
    shared["f1_r"] = bf(_dup(np.cos(th)))
    shared["f1_i"] = bf(_dup(np.sin(th)))
    shared["f1_rn"] = bf(_dup(-np.cos(th)))

    th = 2 * np.pi * np.outer(k2, j2) / M2                        # [64, 64]
    shared["d2i_r"] = bf(_dup(np.cos(th)))
    shared["d2i_i"] = bf(_dup(np.sin(th)))
    shared["d2i_in"] = bf(_dup(-np.sin(th)))

    th = 2 * np.pi * np.outer(np.arange(M1), j2) / M              # [128, 64]
    shared["twic"] = bf(np.cos(th))
    shared["twis"] = bf(np.sin(th))

    th = 2 * np.pi * np.outer(np.arange(M1), j1) / M1             # [128, 64]
    shared["d1i_r"] = bf(np.cos(th) / M)
    shared["d1i_i"] = bf(np.sin(th) / M)
    shared["d1i_in"] = bf(-np.sin(th) / M)

    shared["glu_lhsT"] = bf(np.asarray(inputs["out_w"], f32).T)   # [128, 256]
    ob = np.asarray(inputs["out_b"], f32)
    shared["glu_ba"] = ob[:128].reshape(128, 1).astype(f32)
    shared["glu_bg"] = ob[128:].reshape(128, 1).astype(f32)
    shared["ones_c"] = np.ones((128, 1), f32)
    shared["half_c"] = np.full((128, 1), 0.5, f32)

    shared["fields"] = np.concatenate(
        [bf(_pack_field(p)) for p in (Afld.real, Afld.imag,
                                      Bfld.real, Bfld.imag)], axis=1)

    bf_names = ["enc_lhsT", "d1m_r", "d1m_i", "d1m_in", "twc", "tws",
                "d2m_r", "d2m_i", "d2m_in", "d2m_rn", "f1_r", "f1_i", "f1_rn",
                "d2i_r", "d2i_i", "d2i_in", "twic", "twis",
                "d1i_r", "d1i_i", "d1i_in", "glu_lhsT"]
    blocks = []
    for nm in bf_names:
        a = shared.pop(nm)
        if a.shape[0] != 128:
            pad = np.zeros((128 - a.shape[0], a.shape[1]), a.dtype)
            a = np.concatenate([a, pad], axis=0)
        blocks.append(a)
    shared["cpack"] = np.concatenate(blocks, axis=1)
    f32_names = ["enc_bias", "glu_ba", "glu_bg", "ones_c", "half_c"]
    shared["fpack"] = np.concatenate([shared.pop(nm) for nm in f32_names],
                                     axis=1).astype(f32)

    per_core = []
    for b in range(B):
        xb = x[b]                                                 # (8192, 2)
        per_core.append({
            "xe": bf(xb[0::2, :].T),                              # [2, 4096]
            "xo": bf(xb[1::2, :].T),                              # [2, 4096]
        })
    return shared, per_core


# ---------------------------------------------------------------------------
# device program
# ---------------------------------------------------------------------------

_SHARED_SPECS = [
    ("enc_lhsT", (2, 128), "bf"), ("enc_bias", (128, 1), "f32"),
    ("d1m_r", (128, 128), "bf"), ("d1m_i", (128, 128), "bf"),
    ("d1m_in", (128, 128), "bf"),
    ("twc", (128, 128), "bf"), ("tws", (128, 128), "bf"),
    ("d2m_r", (128, 64), "bf"), ("d2m_i", (128, 64), "bf"),
    ("d2m_in", (128, 64), "bf"), ("d2m_rn", (128, 64), "bf"),
    ("f1_r", (128, 64), "bf"), ("f1_i", (128, 64), "bf"),
    ("f1_rn", (128, 64), "bf"),
    ("d2i_r", (128, 64), "bf"), ("d2i_i", (128, 64), "bf"),
    ("d2i_in", (128, 64), "bf"),
    ("twic", (128, 64), "bf"), ("twis", (128, 64), "bf"),
    ("d1i_r", (128, 64), "bf"), ("d1i_i", (128, 64), "bf"),
    ("d1i_in", (128, 64), "bf"),
    ("glu_lhsT", (128, 256), "bf"), ("glu_ba", (128, 1), "f32"),
    ("glu_bg", (128, 1), "f32"), ("ones_c", (128, 1), "f32"),
    ("half_c", (128, 1), "f32"),
    ("fields", (128, 4 * 8192), "bf"),
]


def build_program(debug_taps=False):
    """Build + compile the single-core SPMD bass program."""
    import concourse.bass as bass
    import concourse.tile as tile
    from concourse import bacc, mybir

    bf = mybir.dt.bfloat16
    f32 = mybir.dt.float32
    AF = mybir.ActivationFunctionType
    ALU = mybir.AluOpType

    nc = bacc.Bacc("TRN2", target_bir_lowering=False, debug=False,
                   num_devices=B)

    dram = {}
    for name, shape, dt_ in _SHARED_SPECS:
        dram[name] = nc.dram_tensor(name, list(shape),
                                    bf if dt_ == "bf" else f32,
                                    kind="ExternalInput").ap()
    dram["xe"] = nc.dram_tensor("xe", [2, 4096], bf, kind="ExternalInput").ap()
    dram["xo"] = nc.dram_tensor("xo", [2, 4096], bf, kind="ExternalInput").ap()
    pool_out = nc.dram_tensor("pool", [128, 1], f32, kind="ExternalOutput").ap()
    taps = {}
    if debug_taps:
        for nm in ("t_ct_r", "t_ct_i", "t_z_r", "t_z_i", "t_zc_r", "t_zc_i",
                   "t_zv_r", "t_zv_i", "t_c3_r", "t_c3_i"):
            taps[nm] = nc.dram_tensor(nm, [128, 8192], bf,
                                      kind="ExternalOutput").ap()
        for nm in ("t_ye", "t_yo"):
            taps[nm] = nc.dram_tensor(nm, [128, 4096], bf,
                                      kind="ExternalOutput").ap()

    with tile.TileContext(nc) as tc:
        from contextlib import ExitStack
        _stack = ExitStack()
        cpool = _stack.enter_context(tc.tile_pool(name="consts", bufs=1))
        C = {}
        for name, shape, dt_ in _SHARED_SPECS:
            if name == "fields":
                continue
            ct = cpool.tile(list(shape), bf if dt_ == "bf" else f32,
                            name="c_" + name, tag=name)
            nc.sync.dma_start(ct[:], dram[name][:])
            C[name] = ct[:]

        persist = _stack.enter_context(tc.tile_pool(name="persist", bufs=1))
        t1 = persist.tile([128, 8192], bf, tag="t1")
        yE = persist.tile([128, 4096], bf, tag="yE")
        yO = persist.tile([128, 4096], bf, tag="yO")
        pool_cols = persist.tile([128, 16], f32, tag="pool_cols")



        work = _stack.enter_context(tc.tile_pool(name="work", bufs=2))
        pw = _stack.enter_context(tc.tile_pool(name="pw", bufs=2))
        pp = _stack.enter_context(tc.tile_pool(name="pp", bufs=1, space="PSUM"))

        # ---- encoder -> DRAM bounce -> T1[j1, (h,j2)] ------------------
        from concourse.tile import add_dep_helper
        dz = {0: nc.dram_tensor("dz_r", [128, 4096], bf, kind="Internal").ap(),
              1: nc.dram_tensor("dz_i", [128, 4096], bf, kind="Internal").ap()}
        for pi, (dst, src) in enumerate(((t1[0:64, :], dram["xe"]),
                                         (t1[64:128, :], dram["xo"]))):
            scat = []
            for c in range(8):
                xch = work.tile([2, 512], bf, tag="xch")
                nc.sync.dma_start(xch[:], src[:, c * 512:(c + 1) * 512])
                pse2 = pp.tile([128, 512], f32, name="pse2", tag="pbig", bufs=2)
                nc.tensor.matmul(pse2[:], C["enc_lhsT"][:], xch[:],
                                 start=True, stop=True)
                zch = work.tile([128, 512], bf, tag="zch")
                nc.scalar.activation(zch[:], pse2[:], AF.Identity,
                                     bias=C["enc_bias"], scale=1.0)
                scat.append(nc.sync.dma_start(
                    dz[pi][:, c * 512:(c + 1) * 512], zch[:]))
            gat = nc.sync.dma_start(
                dst.rearrange("a (h b) -> a h b", h=128),
                dz[pi][:].rearrange("h (a b) -> h a b", a=64).transpose([1, 0, 2]))
            for s in scat:
                add_dep_helper(gat.ins, s.ins, reason="t1 gather after scatter")

        # ---- main groups ----------------------------------------------
        dv_e = nc.dram_tensor("dv_e", [NG, 64, 1024], bf, kind="Internal").ap()
        dv_o = nc.dram_tensor("dv_o", [NG, 64, 1024], bf, kind="Internal").ap()
        fin_scat = []
        for gg in range(NG):
            g0 = gg * G
            # A-rev: Ct[(h',j2), (g,k1)] chunks
            ctr = work.tile([128, 1024], bf, tag="ctr", bufs=3)
            cti = work.tile([128, 1024], bf, tag="cti", bufs=3)
            for gl in range(G):
                g = g0 + gl
                lr = t1[0:64, g * 128:(g + 1) * 128]
                li = t1[64:128, g * 128:(g + 1) * 128]
                ps_re = pp.tile([128, 128], f32, name="psA_re", tag="pa", bufs=3)
                nc.tensor.matmul(ps_re[:], lr, C["d1m_r"][0:64, :], start=True, stop=False)
                nc.tensor.matmul(ps_re[:], li, C["d1m_in"][64:128, :], start=False, stop=True)
                nc.scalar.copy(ctr[:, gl * 128:(gl + 1) * 128], ps_re[:])
                ps_im = pp.tile([128, 128], f32, name="psA_im", tag="pa", bufs=3)
                nc.tensor.matmul(ps_im[:], lr, C["d1m_i"][0:64, :], start=True, stop=False)
                nc.tensor.matmul(ps_im[:], li, C["d1m_r"][64:128, :], start=False, stop=True)
                nc.vector.tensor_copy(cti[:, gl * 128:(gl + 1) * 128], ps_im[:])

            # forward twiddle (broadcast [128,128] tile over g)
            twc_b = C["twc"][:].unsqueeze(1).broadcast_to((128, G, 128))
            tws_b = C["tws"][:].unsqueeze(1).broadcast_to((128, G, 128))
            cttr = work.tile([128, 1024], bf, tag="cttr", bufs=3)
            ctti = work.tile([128, 1024], bf, tag="ctti", bufs=3)
            v3 = lambda t: t[:].rearrange("p (a b) -> p a b", a=G)
            tA = work.tile([128, 1024], bf, name="tA", tag="s1")
            tB = work.tile([128, 1024], bf, name="tB", tag="s2")
            nc.vector.tensor_mul(v3(tA), v3(ctr), twc_b)
            nc.gpsimd.tensor_mul(v3(tB), v3(cti), tws_b)
            nc.vector.tensor_sub(cttr[:], tA[:], tB[:])
            nc.gpsimd.tensor_mul(v3(tA), v3(ctr), tws_b)
            nc.vector.tensor_mul(v3(tB), v3(cti), twc_b)
            nc.vector.tensor_add(ctti[:], tA[:], tB[:])
            if debug_taps:
                nc.sync.dma_start(taps["t_ct_r"][:, g0*128:(g0+G)*128], cttr[:])
                nc.sync.dma_start(taps["t_ct_i"][:, g0*128:(g0+G)*128], ctti[:])

            # stage B -> Z ; Zc via F1/k0 path
            zr = work.tile([128, 1024], bf, tag="zr", bufs=3)
            zi = work.tile([128, 1024], bf, tag="zi", bufs=3)
            for (dst, m1a, m1b) in ((zr, "d2m_r", "d2m_in"),
                                    (zi, "d2m_i", "d2m_r")):
                for c in range(2):
                    ps = pp.tile([128, 512], f32, name="psb", tag="pbig", bufs=2)
                    cols = slice(c * 512, (c + 1) * 512)
                    for h_ in range(2):
                        rows = slice(h_ * 64, (h_ + 1) * 64)
                        nc.tensor.matmul(ps[rows, :], C[m1a][rows, :],
                                         cttr[rows, cols], start=True, stop=False)
                        nc.tensor.matmul(ps[rows, :], C[m1b][rows, :],
                                         ctti[rows, cols], start=False, stop=True)
                    nc.scalar.copy(dst[:, cols], ps[:])

            zcr = work.tile([128, 1024], bf, tag="zcr", bufs=3)
            zci = work.tile([128, 1024], bf, tag="zci", bufs=3)
            # main part k1 in [1,128): rhs cols reversed within each g block
            for (dst, ma, mb) in ((zcr, "f1_r", "f1_i"), (zci, "f1_i", "f1_rn")):
                for c in range(2):   # 4 g per chunk
                    psz = pp.tile([128, 512], f32, name="psc", tag="pbig", bufs=2)
                    ps = psz[:, 0:508]
                    for h_ in range(2):
                        rows = slice(h_ * 64, (h_ + 1) * 64)
                        rev_r = cttr[rows, :].rearrange(
                            "p (a b) -> p a b", a=G)[:, c * 4:(c + 1) * 4, 127:0:-1]
                        rev_i = ctti[rows, :].rearrange(
                            "p (a b) -> p a b", a=G)[:, c * 4:(c + 1) * 4, 127:0:-1]
                        nc.tensor.matmul(ps[rows, :].rearrange(
                            "p (a b) -> p a b", a=4), C[ma][rows, :], rev_r,
                            start=True, stop=False)
                        nc.tensor.matmul(ps[rows, :].rearrange(
                            "p (a b) -> p a b", a=4), C[mb][rows, :], rev_i,
                            start=False, stop=True)
                    nc.scalar.copy(
                        dst[:].rearrange("p (a b) -> p a b", a=G)
                           [:, c * 4:(c + 1) * 4, 1:128],
                        ps.rearrange("p (c b) -> p c b", c=4))
            # k1 = 0 columns
            for (dst, ma, mb) in ((zcr, "d2m_r", "d2m_i"), (zci, "d2m_i", "d2m_rn")):
                psk = pp.tile([128, 128], f32, name="psk", tag="pa", bufs=3)
                ps = psk[:, 0:8]
                r0 = cttr[:].rearrange("p (a b) -> p a b", a=G)[:, :, 0:1]
                i0 = ctti[:].rearrange("p (a b) -> p a b", a=G)[:, :, 0:1]
                for h_ in range(2):
                    rows = slice(h_ * 64, (h_ + 1) * 64)
                    nc.tensor.matmul(ps[rows, :].rearrange("p (a b) -> p a b", a=G),
                                     C[ma][rows, :], r0[h_ * 64:(h_ + 1) * 64],
                                     start=True, stop=False)
                    nc.tensor.matmul(ps[rows, :].rearrange("p (a b) -> p a b", a=G),
                                     C[mb][rows, :], i0[h_ * 64:(h_ + 1) * 64],
                                     start=False, stop=True)
                nc.vector.tensor_copy(
                    dst[:].rearrange("p (a b) -> p a b", a=G)[:, :, 0:1],
                    ps.rearrange("p (a b) -> p a b", a=G))

            if debug_taps:
                for tp, t in (("t_z_r", zr), ("t_z_i", zi),
                              ("t_zc_r", zcr), ("t_zc_i", zci)):
                    nc.sync.dma_start(taps[tp][:, g0*128:(g0+G)*128], t[:])

            # pointwise: Zv = A*Z + B*Zc
            ab = pw.tile([128, 4, 1024], bf, tag="ab")
            cols = slice(g0 * 128, (g0 + G) * 128)
            nc.sync.dma_start(
                ab[:],
                dram["fields"][:].rearrange("p (f c) -> p f c", f=4)[:, :, cols])
            ar, ai, br, bi = ab[:, 0], ab[:, 1], ab[:, 2], ab[:, 3]
            zvr = work.tile([128, 1024], bf, tag="zvr", bufs=2)
            zvi = work.tile([128, 1024], bf, tag="zvi", bufs=2)
            p1 = work.tile([128, 1024], bf, name="p1", tag="s1")
            p2 = work.tile([128, 1024], bf, name="p2", tag="s2")
            p3 = work.tile([128, 1024], bf, name="p3", tag="s3")
            p4 = work.tile([128, 1024], bf, name="p4", tag="s4")
            nc.vector.tensor_mul(p1[:], zr[:], ar)
            nc.gpsimd.tensor_mul(p2[:], zi[:], ai)
            nc.gpsimd.tensor_mul(p3[:], zcr[:], br)
            nc.vector.tensor_mul(p4[:], zci[:], bi)
            nc.vector.tensor_sub(p1[:], p1[:], p2[:])
            nc.vector.tensor_sub(p3[:], p3[:], p4[:])
            nc.vector.tensor_add(zvr[:], p1[:], p3[:])
            nc.vector.tensor_mul(p1[:], zi[:], ar)
            nc.vector.tensor_mul(p2[:], zr[:], ai)
            nc.vector.tensor_mul(p3[:], zci[:], br)
            nc.vector.tensor_mul(p4[:], zcr[:], bi)
            nc.vector.tensor_add(p1[:], p1[:], p2[:])
            nc.vector.tensor_add(p3[:], p3[:], p4[:])
            nc.vector.tensor_add(zvi[:], p1[:], p3[:])
            if debug_taps:
                nc.sync.dma_start(taps["t_zv_r"][:, cols], zvr[:])
                nc.sync.dma_start(taps["t_zv_i"][:, cols], zvi[:])

            # B'-rev: C3[k1, (h,j2)] per (g, h')
            c3r = work.tile([128, 1024], bf, tag="c3r", bufs=3)
            c3i = work.tile([128, 1024], bf, tag="c3i", bufs=3)
            for gl in range(G):
                for h_ in range(2):
                    rows = slice(h_ * 64, (h_ + 1) * 64)
                    lr = zvr[rows, gl * 128:(gl + 1) * 128]
                    li = zvi[rows, gl * 128:(gl + 1) * 128]
                    oc = (2 * gl + h_) * 64
                    ps_re = pp.tile([128, 64], f32, name="psD_re", tag="pd", bufs=3)
                    nc.tensor.matmul(ps_re[:], lr, C["d2i_r"][rows, :], start=True, stop=False)
                    nc.tensor.matmul(ps_re[:], li, C["d2i_in"][rows, :], start=False, stop=True)
                    nc.scalar.copy(c3r[:, oc:oc + 64], ps_re[:])
                    ps_im = pp.tile([128, 64], f32, name="psD_im", tag="pd", bufs=3)
                    nc.tensor.matmul(ps_im[:], lr, C["d2i_i"][rows, :], start=True, stop=False)
                    nc.tensor.matmul(ps_im[:], li, C["d2i_r"][rows, :], start=False, stop=True)
                    nc.vector.tensor_copy(c3i[:, oc:oc + 64], ps_im[:])
            if debug_taps:
                nc.sync.dma_start(taps["t_c3_r"][:, cols], c3r[:])
                nc.sync.dma_start(taps["t_c3_i"][:, cols], c3i[:])

            # inverse twiddle (broadcast [128,64] over h=16)
            twic_b = C["twic"][:].unsqueeze(1).broadcast_to((128, 16, 64))
            twis_b = C["twis"][:].unsqueeze(1).broadcast_to((128, 16, 64))
            v3h = lambda t: t[:].rearrange("p (a b) -> p a b", a=16)
            c3tr = work.tile([128, 1024], bf, tag="c3tr", bufs=2)
            c3ti = work.tile([128, 1024], bf, tag="c3ti", bufs=2)
            tC = work.tile([128, 1024], bf, name="tC", tag="s1")
            tD = work.tile([128, 1024], bf, name="tD", tag="s2")
            nc.vector.tensor_mul(v3h(tC), v3h(c3r), twic_b)
            nc.gpsimd.tensor_mul(v3h(tD), v3h(c3i), twis_b)
            nc.vector.tensor_sub(c3tr[:], tC[:], tD[:])
            nc.gpsimd.tensor_mul(v3h(tC), v3h(c3r), twis_b)
            nc.vector.tensor_mul(v3h(tD), v3h(c3i), twic_b)
            nc.vector.tensor_add(c3ti[:], tC[:], tD[:])

            # stage A' -> vE, vO [j1<64, (h, j2)]
            ve = work.tile([64, 1024], bf, tag="ve")
            vo = work.tile([64, 1024], bf, tag="vo")
            ve_acts, vo_acts = [], []
            for (dst, acts, ma, mb) in ((ve, ve_acts, "d1i_r", "d1i_in"),
                                        (vo, vo_acts, "d1i_i", "d1i_r")):
                for c in range(2):
                    cols2 = slice(c * 512, (c + 1) * 512)
                    ps = pp.tile([64, 512], f32, name="pse", tag="pbig", bufs=2)
                    nc.tensor.matmul(ps[:], C[ma][:], c3tr[:, cols2], start=True, stop=False)
                    nc.tensor.matmul(ps[:], C[mb][:], c3ti[:, cols2], start=False, stop=True)
                    acts.append(nc.scalar.copy(dst[:, cols2], ps[:]))

            # scatter into DRAM bounce then gather this group's 16 y-rows
            for dvt, dst, (srct, acts) in ((dv_e, yE, (ve, ve_acts)),
                                           (dv_o, yO, (vo, vo_acts))):
                dma = nc.sync.dma_start(dvt[gg], srct[:])
                for a in acts:
                    add_dep_helper(dma.ins, a.ins, reason="scatter after A' evac")
                gat = nc.sync.dma_start(
                    yE[gg * 16:(gg + 1) * 16, :].rearrange("h (j b) -> h j b", j=64)
                    if dst is yE else
                    yO[gg * 16:(gg + 1) * 16, :].rearrange("h (j b) -> h j b", j=64),
                    dvt[gg].rearrange("j (hl b) -> hl j b", hl=16))
                add_dep_helper(gat.ins, dma.ins, reason="y gather after scatter")

        if debug_taps:
            nc.sync.dma_start(taps["t_ye"][:], yE[:])
            nc.sync.dma_start(taps["t_yo"][:], yO[:])

        # ---- gelu + GLU + pool ----------------------------------------
        CG = 0.7978845608028654
        planes = (yE, yO)
        idx = 0
        scratch = work.tile([128, 512], bf, tag="glu_scratch")
        for ch in range(2):
            h0 = ch * 2048
            hc = slice(h0, h0 + 2048)
            SPl = h0 + 1472
            lo = slice(h0, SPl)
            hi = slice(SPl, h0 + 2048)
            gt = {}
            for i in range(2):
                gt[i] = pw.tile([128, 2048], bf, name=f"gel{ch}_{i}", tag="ab")
            gl_ = lambda t: t[:, 0:1472]
            gh_ = lambda t: t[:, 1472:2048]
            for i, pl in enumerate(planes):   # sq = x*x
                nc.vector.tensor_mul(gl_(gt[i]), pl[:, lo], pl[:, lo])
                nc.gpsimd.tensor_mul(gh_(gt[i]), pl[:, hi], pl[:, hi])
            for i in range(2):                # rr = 0.044715*sq + 1
                nc.vector.tensor_scalar(gt[i][:], gt[i][:], 0.044715, 1.0,
                                        op0=ALU.mult, op1=ALU.add)
            for i, pl in enumerate(planes):   # qq = x*rr
                nc.vector.tensor_mul(gl_(gt[i]), pl[:, lo], gl_(gt[i]))
                nc.gpsimd.tensor_mul(gh_(gt[i]), pl[:, hi], gh_(gt[i]))
            for i in range(2):                # tt = tanh(CG*qq)
                nc.scalar.activation(gt[i][:], gt[i][:], AF.Tanh, scale=CG)
            for i in range(2):                # uu = 0.5*tt + 0.5
                nc.vector.tensor_scalar(gt[i][:], gt[i][:], 0.5, 0.5,
                                        op0=ALU.mult, op1=ALU.add)
            for i, pl in enumerate(planes):   # y = x*uu
                nc.vector.tensor_mul(pl[:, lo], pl[:, lo], gl_(gt[i]))
                nc.gpsimd.tensor_mul(pl[:, hi], pl[:, hi], gh_(gt[i]))

            for plane in planes:              # GLU on this column half
                for c in range(4):
                    cols = slice(h0 + c * 512, h0 + (c + 1) * 512)
                    ps_a = pp.tile([128, 512], f32, tag="pbig", bufs=2)
                    ps_g = pp.tile([128, 512], f32, tag="pbig", bufs=2)
                    nc.tensor.matmul(ps_a[:], C["glu_lhsT"][:, 0:128],
                                     plane[:, cols], start=True, stop=True)
                    nc.tensor.matmul(ps_g[:], C["glu_lhsT"][:, 128:256],
                                     plane[:, cols], start=True, stop=True)
                    sig = work.tile([128, 512], bf, tag="glu_sig")
                    nc.scalar.activation(sig[:], ps_g[:], AF.Sigmoid,
                                         bias=C["glu_bg"], scale=1.0)
                    nc.vector.scalar_tensor_tensor(
                        scratch[:], ps_a[:], C["glu_ba"], sig[:],
                        op0=ALU.add, op1=ALU.mult,
                        accum_out=pool_cols[:, idx:idx + 1])
                    idx += 1

        pool_t = work.tile([128, 1], f32, tag="pool_t")
        nc.vector.tensor_reduce(pool_t[:], pool_cols[:],
                                axis=mybir.AxisListType.X, op=ALU.add)
        nc.sync.dma_start(pool_out[:], pool_t[:])

        _stack.close()

    nc.compile()
    return nc


_CACHED_NC = None


def kernel(**inputs):
    global _CACHED_NC
    from concourse.bass_utils import run_bass_kernel_spmd

    shared, per_core = host_prep(inputs)
    if _CACHED_NC is None:
        _CACHED_NC = build_program()
    nc = _CACHED_NC

    in_maps = [{**shared, **pc} for pc in per_core]
    res = run_bass_kernel_spmd(nc, in_maps, list(range(B)))
    pool = np.stack([np.asarray(res.results[b]["pool"][:, 0], np.float64)
                     for b in range(B)])                     # (8, 128)
    pooled = pool / float(L)
    dec_w = np.asarray(inputs["dec_w"], np.float64)
    dec_b = np.asarray(inputs["dec_b"], np.float64)
    return (pooled @ dec_w + dec_b).astype(np.float32)


if __name__ == "__main__":
    ins = {
        "x": np.random.randn(B, L, 2).astype(np.float32),
        "enc_w": np.random.randn(2, H).astype(np.float32),
        "enc_b": np.random.randn(H).astype(np.float32),
        "log_dt": np.random.rand(H).astype(np.float32),
        "log_A_real": np.random.randn(H, 32).astype(np.float32),
        "A_imag": np.random.randn(H, 32).astype(np.float32),
        "C_re": np.random.randn(H, 32).astype(np.float32),
        "C_im": np.random.randn(H, 32).astype(np.float32),
        "D": np.random.randn(H).astype(np.float32),
        "out_w": np.random.randn(2 * H, H).astype(np.float32),
        "out_b": np.random.randn(2 * H).astype(np.float32),
        "dec_w": np.random.randn(H, 1).astype(np.float32),
        "dec_b": np.random.randn(1).astype(np.float32),
    }
    print(kernel(**ins).shape)
